# revision 1
# baseline (speedup 1.0000x reference)
"""Trainium2 Bass kernel for nn_EncoderBlock (dual self-attention + BN + FFN + BN).

Sharding: data-parallel over batch (16 batches -> 2 per core on 8 cores).
Device layout: activations transposed (channels E on partitions, tokens on free
dim) so BatchNorm stats are free-dim reductions. Attention computes transposed
scores sT[h] = k_h @ q_h.T so softmax needs no on-device transposes; a ones
column appended to V produces softmax denominators inside the AV matmul; the
per-query reciprocal denominators are broadcast across partitions with a tiny
K=2 matmul. All big matmuls run in float32r (full PE rate, ~1.5e-4 rel err).
BatchNorm batch stats use a 4KB AllReduce across the 8 cores (twice).
The attention phases are software-pipelined at emission time so the per-pair
softmax-denominator chains and batch transitions hide behind independent
projection matmuls.
"""

import numpy as np
import concourse.bass as bass
import concourse.bacc as bacc
import concourse.tile as tile
from concourse import mybir
from concourse.bass_utils import run_bass_kernel_spmd

dt = mybir.dt
F32 = dt.float32
F32R = dt.float32r
AF = mybir.ActivationFunctionType
OP = mybir.AluOpType

N_CORES = 8
B, N, E, H, DK = 16, 1024, 512, 8, 64
NR, NT = 256, 768          # robot / task sequence lengths
BL = B // N_CORES          # local batches per core
TOK = BL * N               # local tokens per core
EC = E // 128              # channel chunks of 128
N_GLOBAL = B * N           # BN stat count
EPS = 1e-5

W_NAMES = ["rq", "rk", "rv", "ro", "tq", "tk", "tv", "to", "f1", "f2"]
ALL_B = W_NAMES + ["bn1_g", "bn1_b", "bn2_g", "bn2_b"]


def _bank_slices(base, length):
    """Split [base, base+length) into pieces (<=512) that never cross a
    512-col PSUM bank boundary. base/length multiples of 256."""
    out = []
    cur = base
    end = base + length
    while cur < end:
        nb = (cur // 512 + 1) * 512
        fl = min(end, nb) - cur
        out.append((cur - base, fl))
        cur += fl
    return out


def build(for_timing=False):
    nc = bacc.Bacc("TRN2", target_bir_lowering=False, debug=False,
                   num_devices=N_CORES)

    xT_d = nc.dram_tensor("xT", [E, TOK], F32, kind="ExternalInput")
    w_d = {n: nc.dram_tensor(f"{n}_wT", [E, E], F32, kind="ExternalInput")
           for n in W_NAMES}
    bpk_d = nc.dram_tensor("bpk", [128, 14 * EC], F32, kind="ExternalInput")
    vrep_d = {n: nc.dram_tensor(f"{n}_brep", [128, E], F32, kind="ExternalInput")
              for n in ["rv", "tv"]}
    sel2_d = nc.dram_tensor("sel2", [98, 128], F32, kind="ExternalInput")
    ones_d = nc.dram_tensor("onesv", [128, H], F32, kind="ExternalInput")
    yT_d = nc.dram_tensor("yT", [E, TOK], F32, kind="ExternalOutput")

    from contextlib import ExitStack
    with tile.TileContext(nc) as tc, ExitStack() as es:
        const = es.enter_context(tc.tile_pool(name="const", bufs=1))
        wpool = es.enter_context(tc.tile_pool(name="w", bufs=1))
        act = es.enter_context(tc.tile_pool(name="act", bufs=1))
        attn = es.enter_context(tc.tile_pool(name="attn", bufs=1))
        expp = es.enter_context(tc.tile_pool(name="expp", bufs=2))
        small = es.enter_context(tc.tile_pool(name="small", bufs=2))
        dram = es.enter_context(tc.tile_pool(name="dram", bufs=1, space="DRAM"))
        ps_big = es.enter_context(tc.tile_pool(name="ps_big", bufs=2, space="PSUM"))
        ps_av = es.enter_context(tc.tile_pool(name="ps_av", bufs=2, space="PSUM"))
        _body(nc, const, wpool, act, attn, expp, small, dram, ps_big, ps_av,
              xT_d, w_d, bpk_d, vrep_d, sel2_d, ones_d, yT_d, for_timing)
    nc.finalize()
    return nc


def _load_w(nc, wpool, w_dram, tag):
    tiles = []
    for k in range(EC):
        t = wpool.tile([128, E], F32R, tag=f"{tag}{k}", name=f"{tag}{k}")
        nc.sync.dma_start(out=t[:],
                          in_=w_dram.ap()[k * 128:(k + 1) * 128, :].bitcast(F32R))
        tiles.append(t)
    return tiles


class _Ctx:
    pass


def _body(nc, const, wpool, act, attn, expp, small, dram, ps_big, ps_av,
          xT_d, w_d, bpk_d, vrep_d, sel2_d, ones_d, yT_d, for_timing):
    # ---------- constants / inputs resident in SBUF ----------
    # DMA emission order == HWDGE issue order: the first projection needs
    # bpk + wq + xT robot-b0 columns; everything else can trickle in after
    bpk = const.tile([128, 14 * EC], F32, tag="bpk", name="bpk")
    nc.sync.dma_start(out=bpk[:], in_=bpk_d.ap())
    bias = {n: bpk[:, i * EC:(i + 1) * EC] for i, n in enumerate(ALL_B)}

    w_robot = {"rq": _load_w(nc, wpool, w_d["rq"], "wq")}
    xT = [const.tile([128, TOK], F32R, tag=f"xT{k}", name=f"xT{k}")
          for k in range(EC)]
    for k in range(EC):
        nc.sync.dma_start(out=xT[k][:, 0:NR],
                          in_=xT_d.ap()[k * 128:(k + 1) * 128, 0:NR].bitcast(F32R))
    for k in range(EC):
        nc.sync.dma_start(out=xT[k][:, N:N + NR],
                          in_=xT_d.ap()[k * 128:(k + 1) * 128, N:N + NR].bitcast(F32R))
    w_robot["rk"] = _load_w(nc, wpool, w_d["rk"], "wk")
    w_robot["rv"] = _load_w(nc, wpool, w_d["rv"], "wv")
    vrep = {}
    for n in ["rv", "tv"]:
        t = const.tile([128, E], F32, tag=f"vr_{n}", name=f"vr_{n}")
        nc.sync.dma_start(out=t[:], in_=vrep_d[n].ap())
        vrep[n] = t
    w_robot["ro"] = _load_w(nc, wpool, w_d["ro"], "wo")
    sel2 = const.tile([98, 128], F32R, tag="sel2", name="sel2")
    nc.sync.dma_start(out=sel2[:], in_=sel2_d.ap().bitcast(F32R))


    # prefetch the exp ACT table set while input DMAs are in flight
    warm = const.tile([1, 1], F32, tag="warm", name="warm")
    nc.vector.memset(warm[:], 0.0)
    nc.scalar.activation(out=warm[:], in_=warm[:], func=AF.Exp, scale=1.0)

    # persistent V tiles ([128, H, DK+1] per 128-token chunk); the ones
    # column (softmax denominator trick) is initialized once
    v_sb = []
    for t in range(NT // 128):
        vt = attn.tile([128, H, DK + 1], F32R, tag=f"v{t}", name=f"v{t}")
        nc.gpsimd.dma_start(out=vt[:, :, DK:DK + 1],
                            in_=ones_d.ap().bitcast(F32R))
        v_sb.append(vt)

    # h-tilde (pre-BN1 attention output) accumulated across parts/batches
    ht = [act.tile([128, TOK], F32, tag=f"ht{k}", name=f"ht{k}")
          for k in range(EC)]

    # ---------- attention (emission software-pipelined) ----------
    def proj_qkv(P, b):
        """q/k projections into channel-major tiles + v into token-major."""
        tok0 = b * N + (0 if P.part == 0 else NR)
        st = _Ctx()
        st.tok0 = tok0
        st.qT = [attn.tile([128, NT], F32R, tag=f"qT{m}", name=f"qT{m}")
                 for m in range(EC)]
        st.kT = [attn.tile([128, NT], F32R, tag=f"kT{m}", name=f"kT{m}")
                 for m in range(EC)]
        def emit_qk(w_t, o_t, bn_, m):
            ps = ps_big.tile([128, NT], F32, tag="sc", name="psq")
            for off, fl in _bank_slices(0, P.np):
                for k in range(EC):
                    nc.tensor.matmul(
                        ps[:, off:off + fl],
                        w_t[k][:, m * 128:(m + 1) * 128],
                        xT[k][:, tok0 + off:tok0 + off + fl],
                        start=(k == 0), stop=(k == EC - 1))
            nc.vector.tensor_scalar(
                out=o_t[m][:, 0:P.np], in0=ps[:, 0:P.np],
                scalar1=bias[bn_][:, m:m + 1], scalar2=None, op0=OP.add)

        def emit_v(t):
            vt = v_sb[t]
            ps = ps_big.tile([128, E], F32, tag="sc", name="psv")
            for k in range(EC):
                nc.tensor.matmul(
                    ps[:], xT[k][:, tok0 + t * 128:tok0 + (t + 1) * 128],
                    P.wv[k][:], start=(k == 0), stop=(k == EC - 1))
            nc.vector.tensor_tensor(
                out=vt[:, :, 0:DK],
                in0=ps[:].rearrange("p (h d) -> p h d", h=H),
                in1=vrep[P.wn[2]][:].rearrange("p (h d) -> p h d", h=H),
                op=OP.add)

        # interleave so pair-0 scores (qT[0]/kT[0]) and v chunk 0 are ready
        # as early as possible
        v_sched = [[0], [1], [2, 3], [4, 5]] if P.nk == 6 else [[0], [1], [], []]
        for m in range(EC):
            emit_qk(P.wq, st.qT, P.wn[0], m)
            emit_qk(P.wk, st.kT, P.wn[1], m)
            for t in v_sched[m]:
                emit_v(t)
        return st

    def heads(P, st):
        """Per-head scores -> exp -> AV (+denominator row); psum evacuated
        partition-aligned then remapped into pair tiles via SWDGE DMA."""
        Np = P.np
        st.zT = [attn.tile([128, NT], F32R, tag=f"zT{p}", name=f"zT{p}")
                 for p in range(4)]
        rows_all = small.tile([98, NT], F32, tag="rows", name="rows", bufs=1)
        rinv_all = small.tile([98, NT], F32R, tag="rinv", name="rinv", bufs=1)
        st.rows = rows_all
        st.rinv = rinv_all
        for pair in range(4):
            for j in range(2):
                h = 2 * pair + j
                qh = st.qT[h // 2][(h % 2) * 64:(h % 2) * 64 + 64, 0:Np]
                kh = st.kT[h // 2][(h % 2) * 64:(h % 2) * 64 + 64, 0:Np]
                zu = ps_av.tile([65, NT], F32, tag="av", name="av", bufs=1)
                if P.part == 0:
                    sc = ps_big.tile([128, 512], F32, tag="sc", name="sc")
                    for kc in range(P.nk):
                        nc.tensor.matmul(sc[:, kc * Np:(kc + 1) * Np],
                                         kh[:, kc * 128:(kc + 1) * 128], qh,
                                         start=True, stop=True)
                    ex = expp.tile([128, 512], F32R, tag="exp", name="exp")
                    nc.scalar.activation(out=ex[:], in_=sc[:], func=AF.Exp,
                                         scale=0.125)
                    for kc in range(P.nk):
                        for off, fl in _bank_slices(0, Np):
                            nc.tensor.matmul(
                                zu[:, off:off + fl], v_sb[kc][:, h, :],
                                ex[:, kc * Np + off:kc * Np + off + fl],
                                start=(kc == 0), stop=(kc == P.nk - 1))
                else:
                    for g in range(P.nk // 2):
                        sc = ps_big.tile([128, 2, NT], F32, tag="sc", name="sc")
                        for j2 in range(2):
                            kc = 2 * g + j2
                            for off, fl in _bank_slices(j2 * NT, Np):
                                nc.tensor.matmul(sc[:, j2, off:off + fl],
                                                 kh[:, kc * 128:(kc + 1) * 128],
                                                 qh[:, off:off + fl],
                                                 start=True, stop=True)
                        ex = expp.tile([128, 2, NT], F32R, tag="exp", name="exp")
                        nc.scalar.activation(out=ex[:], in_=sc[:], func=AF.Exp,
                                             scale=0.125)
                        for j2 in range(2):
                            kc = 2 * g + j2
                            for off, fl in _bank_slices(0, Np):
                                nc.tensor.matmul(
                                    zu[:, off:off + fl], v_sb[kc][:, h, :],
                                    ex[:, j2, off:off + fl],
                                    start=(kc == 0), stop=(kc == P.nk - 1))
                zst = expp.tile([65, NT], F32R, tag="zst", name="zst", bufs=2)
                nc.vector.tensor_copy(out=zst[:, 0:Np],
                                      in_=zu[:, 0:Np].bitcast(F32R))
                nc.sync.dma_start(out=st.zT[pair][j * 64:(j + 1) * 64, 0:Np],
                                  in_=zst[0:64, 0:Np])
                nc.sync.dma_start(
                    out=rows_all[32 * pair + j:32 * pair + j + 1, 0:Np],
                    in_=zst[64:65, 0:Np].bitcast(F32))
                if j == 1:
                    # hoisted reciprocal: right behind the rowsum DMAs in the
                    # DVE queue instead of behind the next batch's evacs
                    with nc.allow_low_precision(reason="f32r feeds f32r mm"):
                        nc.vector.reciprocal(
                            out=rinv_all[32 * pair:32 * pair + 2, 0:Np],
                            in_=rows_all[32 * pair:32 * pair + 2, 0:Np])

    def denom_outproj(P, st):
        """Softmax denominators (reciprocal + K=2 broadcast matmul), then the
        output projection with bias + residual into ht."""
        Np = P.np
        tok0 = st.tok0
        for pair in range(4):
            rinv = st.rinv[32 * pair:32 * pair + 2, 0:Np]
            rep = ps_big.tile([128, NT], F32, tag="sc", name="rep")
            tp = (96, 0) if pair == 3 else None
            for off, fl in _bank_slices(0, Np):
                nc.tensor.matmul(rep[:, off:off + fl],
                                 sel2[32 * pair:32 * pair + 2, :],
                                 rinv[:, off:off + fl], start=True, stop=True,
                                 tile_position=tp)
            nc.vector.tensor_tensor(out=st.zT[pair][:, 0:Np],
                                    in0=st.zT[pair][:, 0:Np].bitcast(F32),
                                    in1=rep[:, 0:Np], op=OP.mult)
        for m in range(EC):
            ps = ps_big.tile([128, NT], F32, tag="sc", name="pso")
            for off, fl in _bank_slices(0, Np):
                for k in range(EC):
                    nc.tensor.matmul(ps[:, off:off + fl],
                                     P.wo[k][:, m * 128:(m + 1) * 128],
                                     st.zT[k][:, off:off + fl],
                                     start=(k == 0), stop=(k == EC - 1))
            dst = ht[m][:, tok0:tok0 + Np]
            nc.scalar.activation(out=dst, in_=ps[:, 0:Np], func=AF.Identity,
                                 bias=bias[P.wn[3]][:, m:m + 1], scale=1.0)
            nc.vector.tensor_tensor(
                out=dst, in0=dst,
                in1=xT[m][:, tok0:tok0 + Np].bitcast(F32), op=OP.add)
            if P.part == 1 and tok0 >= N:
                # batch-1 token chunks complete for this m: emit BN1 stats
                for c in (2, 3):
                    nc.vector.bn_stats(out=st1_tiles[m][:, c, :],
                                       in_=ht[m][:, c * 512:(c + 1) * 512])

    st1_tiles = _bn_stats_tiles(small, "bn1")
    st2_tiles = _bn_stats_tiles(small, "bn2")
    f1 = f2 = None
    for part in range(2):
        P = _Ctx()
        P.part = part
        P.wn = ["rq", "rk", "rv", "ro"] if part == 0 else ["tq", "tk", "tv", "to"]
        P.np = NR if part == 0 else NT
        P.nk = P.np // 128
        if part == 0:
            P.wq, P.wk, P.wv, P.wo = (w_robot["rq"], w_robot["rk"],
                                      w_robot["rv"], w_robot["ro"])
        else:
            P.wq = _load_w(nc, wpool, w_d[P.wn[0]], "wq")
            P.wk = _load_w(nc, wpool, w_d[P.wn[1]], "wk")
            P.wv = _load_w(nc, wpool, w_d[P.wn[2]], "wv")
            P.wo = _load_w(nc, wpool, w_d[P.wn[3]], "wo")

        st0 = proj_qkv(P, 0)
        heads(P, st0)
        st1 = proj_qkv(P, 1)
        if part == 0:
            # task xT columns load during robot attention, queued behind the
            # latency-critical robot z/rows transfers
            for off, ln in [(NR, NT), (N + NR, NT)]:
                for k in range(EC):
                    nc.sync.dma_start(
                        out=xT[k][:, off:off + ln],
                        in_=xT_d.ap()[k * 128:(k + 1) * 128,
                                      off:off + ln].bitcast(F32R))
        denom_outproj(P, st0)
        if part == 1:
            # ht token chunks 0,1 (batch 0) are complete: emit their BN1 stats
            for m in range(EC):
                for c in (0, 1):
                    nc.vector.bn_stats(
                        out=st1_tiles[m][:, c, :],
                        in_=ht[m][:, c * 512:(c + 1) * 512])
        heads(P, st1)
        if part == 1:
            # prefetch FFN weights into slots whose last readers are done
            f1 = _load_w(nc, wpool, w_d["f1"], "wq")
            f2 = _load_w(nc, wpool, w_d["f2"], "wk")
            # all exps done: swap the ACT table set to sqrt ahead of BN1
            warm2 = const.tile([1, 1], F32, tag="warm", name="warm2")
            nc.vector.memset(warm2[:], 1.0)
            nc.scalar.activation(out=warm2[:], in_=warm2[:], func=AF.Sqrt,
                                 scale=1.0)
        denom_outproj(P, st1)

    # ---------- BN1 ----------
    s1, t1 = _bn_params(nc, small, dram, st1_tiles, bias["bn1_g"],
                        bias["bn1_b"], "bn1", for_timing)
    hn = [act.tile([128, TOK], F32R, tag=f"hn{k}", name=f"hn{k}")
          for k in range(EC)]
    for m in range(EC):
        if m % 2 == 0:
            nc.vector.tensor_scalar(out=hn[m][:], in0=ht[m][:],
                                    scalar1=s1[m], scalar2=t1[m],
                                    op0=OP.mult, op1=OP.add)
        else:
            nc.scalar.activation(out=hn[m][:], in_=ht[m][:], func=AF.Identity,
                                 bias=t1[m], scale=s1[m])

    # ---------- FFN ----------
    h1 = [const.tile([128, TOK], F32R, tag=f"xT{k}", name=f"h1_{k}")
          for k in range(EC)]
    for m in range(EC):
        for off, fl in _bank_slices(0, TOK):
            ps = ps_big.tile([128, 512], F32, tag="sc", name="psf1")
            for k in range(EC):
                nc.tensor.matmul(ps[:, 0:fl], f1[k][:, m * 128:(m + 1) * 128],
                                 hn[k][:, off:off + fl],
                                 start=(k == 0), stop=(k == EC - 1))
            nc.scalar.activation(out=h1[m][:, off:off + fl], in_=ps[:, 0:fl],
                                 func=AF.Relu, bias=bias["f1"][:, m:m + 1],
                                 scale=1.0)
    ho = [act.tile([128, TOK], F32, tag=f"ht{k}", name=f"ho{k}")
          for k in range(EC)]
    for m in range(EC):
        for off, fl in _bank_slices(0, TOK):
            ps = ps_big.tile([128, 512], F32, tag="sc", name="psf2")
            for k in range(EC):
                nc.tensor.matmul(ps[:, 0:fl], f2[k][:, m * 128:(m + 1) * 128],
                                 h1[k][:, off:off + fl],
                                 start=(k == 0), stop=(k == EC - 1))
            dst = ho[m][:, off:off + fl]
            nc.scalar.activation(out=dst, in_=ps[:, 0:fl], func=AF.Identity,
                                 bias=bias["f2"][:, m:m + 1], scale=1.0)
            nc.vector.tensor_tensor(out=dst, in0=dst,
                                    in1=hn[m][:, off:off + fl].bitcast(F32),
                                    op=OP.add)
            nc.vector.bn_stats(out=st2_tiles[m][:, off // 512, :], in_=dst)

    # ---------- BN2 + output (pipelined per 512-token slice) ----------
    s2, t2 = _bn_params(nc, small, dram, st2_tiles, bias["bn2_g"],
                        bias["bn2_b"], "bn2", for_timing)
    for m in range(EC):
        if m % 2 == 0:
            nc.vector.tensor_scalar(out=ho[m][:], in0=ho[m][:],
                                    scalar1=s2[m], scalar2=t2[m],
                                    op0=OP.mult, op1=OP.add)
        else:
            nc.scalar.activation(out=ho[m][:], in_=ho[m][:],
                                 func=AF.Identity, bias=t2[m], scale=s2[m])
        nc.sync.dma_start(out=yT_d.ap()[m * 128:(m + 1) * 128, :], in_=ho[m][:])


def _bn_stats_tiles(small, name):
    return [small.tile([128, 4, 6], F32, tag=f"st_{name}{m}",
                       name=f"st_{name}{m}", bufs=1) for m in range(EC)]


def _bn_params(nc, small, dram, sts, g_sb, b_sb, name, for_timing=False):
    """Per-channel scale/shift for training-mode BN over all B*N tokens:
    local sums (bn_stats emitted earlier into sts) -> 8-core AllReduce ->
    mu/var -> sqrt+recip (+1 Newton step).
    Returns ([EC] scale APs, [EC] shift APs), each [128, 1]."""
    ccin = dram.tile([128, 2 * EC], F32, tag=f"cci_{name}", name=f"cci_{name}")
    ccout = dram.tile([128, 2 * EC], F32, tag=f"cco_{name}", name=f"cco_{name}")
    su = small.tile([128, 2 * EC], F32, tag=f"su_{name}", name=f"su_{name}")
    mva = small.tile([128, EC, 2], F32, tag=f"mv_{name}", name=f"mv_{name}",
                     bufs=1)
    for m in range(EC):
        nc.vector.bn_aggr(out=mva[:, m, :], in_=sts[m][:])
    # su0 = sum(h) = mean * TOK ; su1 = sum(h^2) = (var + mean^2) * TOK
    suv = su[:].rearrange("p (c two) -> p c two", two=2)
    t = small.tile([128, EC], F32, tag=f"tmp_{name}", name=f"tmp_{name}",
                   bufs=1)
    nc.vector.tensor_scalar(out=suv[:, :, 0], in0=mva[:, :, 0],
                            scalar1=float(TOK), scalar2=None, op0=OP.mult)
    nc.vector.tensor_tensor(out=t[:], in0=mva[:, :, 0], in1=mva[:, :, 0],
                            op=OP.mult)
    nc.vector.tensor_tensor(out=t[:], in0=t[:], in1=mva[:, :, 1], op=OP.add)
    nc.vector.tensor_scalar(out=suv[:, :, 1], in0=t[:], scalar1=float(TOK),
                            scalar2=None, op0=OP.mult)
    nc.sync.dma_start(out=ccin[:], in_=su[:])
    if for_timing:
        # TimelineSim cannot model collectives; substitute a same-shape copy
        nc.gpsimd.dma_start(out=ccout[:], in_=ccin[:])
    else:
        nc.gpsimd.collective_compute(
            "AllReduce", OP.add, replica_groups=[list(range(N_CORES))],
            ins=[ccin.opt()], outs=[ccout.opt()])
    scales, shifts = [], []
    gsa = small.tile([128, 2 * EC], F32, tag=f"gs_{name}", name=f"gs_{name}")
    nc.sync.dma_start(out=gsa[:], in_=ccout[:])
    gv = gsa[:].rearrange("p (c two) -> p c two", two=2)
    mu = small.tile([128, EC], F32, tag=f"mu_{name}", name=f"mu_{name}", bufs=1)
    var = small.tile([128, EC], F32, tag=f"var_{name}", name=f"var_{name}",
                     bufs=1)
    t2 = small.tile([128, EC], F32, tag=f"t2_{name}", name=f"t2_{name}", bufs=1)
    nc.vector.tensor_scalar(out=mu[:], in0=gv[:, :, 0],
                            scalar1=1.0 / N_GLOBAL, scalar2=None, op0=OP.mult)
    nc.vector.tensor_scalar(out=t2[:], in0=gv[:, :, 1],
                            scalar1=1.0 / N_GLOBAL, scalar2=None, op0=OP.mult)
    nc.vector.tensor_tensor(out=var[:], in0=mu[:], in1=mu[:], op=OP.mult)
    nc.vector.tensor_tensor(out=var[:], in0=t2[:], in1=var[:], op=OP.subtract)
    # r = 1/sqrt(var + eps): ACT Sqrt + DVE reciprocal, then one Newton step
    # to wash out the sqrt table's loose ULP budget
    epst = small.tile([128, 1], F32, tag=f"eps_{name}", name=f"eps_{name}",
                      bufs=1)
    nc.vector.memset(epst[:], EPS)
    sq = small.tile([128, EC], F32, tag=f"sq_{name}", name=f"sq_{name}", bufs=1)
    nc.scalar.activation(out=sq[:], in_=var[:], func=AF.Sqrt, bias=epst[:],
                         scale=1.0)
    r0 = small.tile([128, EC], F32, tag=f"r0_{name}", name=f"r0_{name}", bufs=1)
    nc.vector.reciprocal(out=r0[:], in_=sq[:])
    av_ = small.tile([128, EC], F32, tag=f"a_{name}", name=f"a_{name}", bufs=1)
    nc.vector.tensor_scalar(out=av_[:], in0=var[:], scalar1=EPS, scalar2=None,
                            op0=OP.add)
    nt = small.tile([128, EC], F32, tag=f"nt_{name}", name=f"nt_{name}", bufs=1)
    nc.vector.tensor_tensor(out=nt[:], in0=r0[:], in1=r0[:], op=OP.mult)
    nc.vector.tensor_tensor(out=nt[:], in0=nt[:], in1=av_[:], op=OP.mult)
    nc.vector.tensor_scalar(out=nt[:], in0=nt[:], scalar1=-0.5, scalar2=1.5,
                            op0=OP.mult, op1=OP.add)
    r = small.tile([128, EC], F32, tag=f"r_{name}", name=f"r_{name}", bufs=1)
    nc.vector.tensor_tensor(out=r[:], in0=r0[:], in1=nt[:], op=OP.mult)
    s_all = small.tile([128, EC], F32, tag=f"s_{name}", name=f"s_{name}",
                       bufs=1)
    sh_all = small.tile([128, EC], F32, tag=f"sh_{name}", name=f"sh_{name}",
                        bufs=1)
    nc.vector.tensor_tensor(out=s_all[:], in0=r[:], in1=g_sb, op=OP.mult)
    nc.vector.tensor_tensor(out=sh_all[:], in0=mu[:], in1=s_all[:], op=OP.mult)
    nc.vector.tensor_tensor(out=sh_all[:], in0=b_sb, in1=sh_all[:],
                            op=OP.subtract)
    for m in range(EC):
        scales.append(s_all[:, m:m + 1])
        shifts.append(sh_all[:, m:m + 1])
    return scales, shifts


_NC_CACHE = None


def _get_nc():
    global _NC_CACHE
    if _NC_CACHE is None:
        _NC_CACHE = build()
    return _NC_CACHE


def make_in_maps(inputs):
    shared = {}
    for n in W_NAMES:
        shared[f"{n}_wT"] = np.ascontiguousarray(inputs[f"{n}_w"].T)
    for n in ["rv", "tv"]:
        shared[f"{n}_brep"] = np.ascontiguousarray(
            np.broadcast_to(inputs[f"{n}_b"], (128, E)))
    bpk = np.empty((128, 14 * EC), dtype=np.float32)
    for i, n in enumerate(ALL_B):
        vec = inputs[f"{n}_b"] if n in W_NAMES else inputs[n]
        bpk[:, i * EC:(i + 1) * EC] = np.asarray(vec).reshape(EC, 128).T
    shared["bpk"] = bpk
    sel2 = np.zeros((98, 128), dtype=np.float32)
    for p in range(4):
        sel2[32 * p, 0:64] = 1.0
        sel2[32 * p + 1, 64:128] = 1.0
    shared["sel2"] = sel2
    shared["onesv"] = np.ones((128, H), dtype=np.float32)

    x = np.asarray(inputs["x"], dtype=np.float32)
    in_maps = []
    for i in range(N_CORES):
        xc = x[BL * i:BL * (i + 1)]                      # [BL, N, E]
        xT = np.ascontiguousarray(xc.transpose(2, 0, 1).reshape(E, TOK))
        in_maps.append({"xT": xT, **shared})
    return in_maps


def assemble_output(results):
    y = np.empty((B, N, E), dtype=np.float32)
    for i in range(N_CORES):
        yT = results[i]["yT"]                            # [E, TOK]
        y[BL * i:BL * (i + 1)] = yT.reshape(E, BL, N).transpose(1, 2, 0)
    return y


def kernel(**inputs):
    nc = _get_nc()
    in_maps = make_in_maps(inputs)
    res = run_bass_kernel_spmd(nc, in_maps, core_ids=list(range(N_CORES)))
    return assemble_output(res.results)


if __name__ == "__main__":
    nc = build()
    print("build ok")



# revision 21
# speedup vs baseline: 1.1223x; 1.1223x over previous
"""Trainium2 Bass kernel for nn_EncoderBlock (dual self-attention + BN + FFN + BN).

Sharding: data-parallel over batch (16 batches -> 2 per core on 8 cores).

v2: the attention block runs in fp8e4m3 with DoubleRow matmuls (4x PE rate on
the qkv/out projections and AV), validated to ~8e-4 end-to-end rel err.
Scores stay bf16 (64-deep contraction can't DoubleRow). exp outputs fp8
directly from ACT with the softmax /8 range shift folded into the exp bias;
the ones-column (value 1/64) inside V yields denominators from the AV matmul.
K/Q/O biases are per-partition scalars in the evacuation ops (K's provably
cancels in softmax but is applied anyway); V's bias rides the existing
psum->v8 add. The out-projection evacuation is a fused scalar_tensor_tensor
(psum + bias + residual) whose accum_out doubles as the BN1 channel sums;
sumsq comes from a square pass split across DVE/ACT. BN1's scale is folded
into the F1 weights (in-place) so the FFN starts right after the AllReduce;
BN1's shift becomes an F1 bias correction via a tiny matvec. BN2 stats use
the same accum trick; the final normalize+store is pipelined per 512-column
slice across DVE/ACT/Pool with immediate per-slice DMA.
BatchNorm batch stats use a 4KB AllReduce across the 8 cores (twice).
"""

import numpy as np
import concourse.bass as bass
import concourse.bacc as bacc
import concourse.tile as tile
from concourse import mybir
from concourse.bass_utils import run_bass_kernel_spmd

dt = mybir.dt
F32 = dt.float32
F32R = dt.float32r
BF16 = dt.bfloat16
F8 = dt.float8e4
AF = mybir.ActivationFunctionType
OP = mybir.AluOpType
DR = mybir.MatmulPerfMode.DoubleRow

N_CORES = 8
B, N, E, H, DK = 16, 1024, 512, 8, 64
NR, NT = 256, 768          # robot / task sequence lengths
BL = B // N_CORES          # local batches per core
TOK = BL * N               # local tokens per core
EC = E // 128              # channel chunks of 128
N_GLOBAL = B * N           # BN stat count
EPS = 1e-5
LN8 = 2.0794415416798357   # exp range shift: ex = exp(s)/8
IVS = 1.0 / 64.0           # ones-column value (denominator scale)

W8_NAMES = ["rq", "rk", "rv", "ro", "tq", "tk", "tv", "to"]
ALL_B = W8_NAMES + ["f1", "f2", "bn1_g", "bn1_b", "bn2_g", "bn2_b"]


def _bank_slices(base, length, maxlen=512):
    """Split [base, base+length) into pieces (<=maxlen) that never cross a
    512-col PSUM bank boundary."""
    out = []
    cur = base
    end = base + length
    while cur < end:
        nb = (cur // 512 + 1) * 512
        fl = min(end, min(nb, cur + maxlen)) - cur
        out.append((cur - base, fl))
        cur += fl
    return out


def build(for_timing=False):
    nc = bacc.Bacc("TRN2", target_bir_lowering=False, debug=False,
                   num_devices=N_CORES)

    xT_d = nc.dram_tensor("xT", [E, TOK], F32, kind="ExternalInput")
    x8_d = nc.dram_tensor("x8", [128, EC * TOK], F8, kind="ExternalInput")
    w8_d = {n: nc.dram_tensor(f"{n}_w8", [128, 4 * E], F8, kind="ExternalInput")
            for n in W8_NAMES}
    f_d = {"f1": nc.dram_tensor("f1_wT", [E, E], F32, kind="ExternalInput"),
           "f2": nc.dram_tensor("f2_wT", [E, E], BF16, kind="ExternalInput")}
    bpk_d = nc.dram_tensor("bpk", [128, len(ALL_B) * EC], F32,
                           kind="ExternalInput")
    vrep_d = {n: nc.dram_tensor(f"{n}_brep", [128, E], F32, kind="ExternalInput")
              for n in ["rv", "tv"]}
    sel2_d = nc.dram_tensor("sel2", [98, 128], F32, kind="ExternalInput")
    yT_d = nc.dram_tensor("yT", [E, TOK], F32, kind="ExternalOutput")

    from contextlib import ExitStack
    with tile.TileContext(nc) as tc, ExitStack() as es:
        const = es.enter_context(tc.tile_pool(name="const", bufs=1))
        wpool = es.enter_context(tc.tile_pool(name="w", bufs=1))
        act = es.enter_context(tc.tile_pool(name="act", bufs=1))
        attn = es.enter_context(tc.tile_pool(name="attn", bufs=2))
        expp = es.enter_context(tc.tile_pool(name="expp", bufs=2))
        small = es.enter_context(tc.tile_pool(name="small", bufs=2))
        dram = es.enter_context(tc.tile_pool(name="dram", bufs=1, space="DRAM"))
        ps_big = es.enter_context(tc.tile_pool(name="ps_big", bufs=2, space="PSUM"))
        ps_av = es.enter_context(tc.tile_pool(name="ps_av", bufs=2, space="PSUM"))
        _body(nc, const, wpool, act, attn, expp, small, dram, ps_big, ps_av,
              xT_d, x8_d, w8_d, f_d, bpk_d, vrep_d, sel2_d, yT_d, for_timing)
    nc.finalize()
    return nc


class _Ctx:
    pass


def _body(nc, const, wpool, act, attn, expp, small, dram, ps_big, ps_av,
          xT_d, x8_d, w8_d, f_d, bpk_d, vrep_d, sel2_d, yT_d, for_timing):
    # ---------- constants / inputs resident in SBUF ----------
    # DMA emission order == issue order on the sync queue: the first
    # projection needs rq/rk weights + robot x8 columns; bulk (xT, task x8,
    # FFN weights) trickles in behind.
    w8 = {}
    for n in ["rq", "rk"]:
        t = wpool.tile([128, 2, 2, E], F8, tag=f"w8{n}", name=f"w8{n}")
        nc.sync.dma_start(out=t[:], in_=w8_d[n].ap())
        w8[n] = t
    x8 = const.tile([128, EC, TOK], F8, tag="x8", name="x8")
    x8d = x8_d.ap().rearrange("p (k t) -> p k t", k=EC)
    nc.sync.dma_start(out=x8[:, :, 0:NR], in_=x8d[:, :, 0:NR])
    nc.sync.dma_start(out=x8[:, :, N:N + NR], in_=x8d[:, :, N:N + NR])
    bpk = const.tile([128, len(ALL_B) * EC], F32, tag="bpk", name="bpk")
    nc.sync.dma_start(out=bpk[:], in_=bpk_d.ap())
    bias = {n: bpk[:, i * EC:(i + 1) * EC] for i, n in enumerate(ALL_B)}
    for n in ["rv", "ro"]:
        t = wpool.tile([128, 2, 2, E], F8, tag=f"w8{n}", name=f"w8{n}")
        nc.sync.dma_start(out=t[:], in_=w8_d[n].ap())
        w8[n] = t
    vrep = {}
    t = const.tile([128, E], F32, tag="vr_rv", name="vr_rv")
    nc.sync.dma_start(out=t[:], in_=vrep_d["rv"].ap())
    vrep["rv"] = t
    sel2 = const.tile([98, 128], F32R, tag="sel2", name="sel2")
    nc.sync.dma_start(out=sel2[:], in_=sel2_d.ap().bitcast(F32R))
    # task-side x8 + weights and all bulk loads ride the ACT HWDGE queue so
    # they never head-of-line-block the latency-critical z8/rows transfers
    # on the sync (SP) queue.
    nc.scalar.dma_start(out=x8[:, :, NR:N], in_=x8d[:, :, NR:N])
    nc.scalar.dma_start(out=x8[:, :, N + NR:], in_=x8d[:, :, N + NR:])
    for n in ["tq", "tk", "tv", "to"]:
        t = wpool.tile([128, 2, 2, E], F8, tag=f"w8{n}", name=f"w8{n}")
        nc.scalar.dma_start(out=t[:], in_=w8_d[n].ap())
        w8[n] = t
    t = const.tile([128, E], F32, tag="vr_tv", name="vr_tv")
    nc.scalar.dma_start(out=t[:], in_=vrep_d["tv"].ap())
    vrep["tv"] = t
    # residual xT: robot columns first
    xT = [const.tile([128, TOK], F32R, tag=f"xT{k}", name=f"xT{k}")
          for k in range(EC)]
    for k in range(EC):
        nc.scalar.dma_start(out=xT[k][:, 0:NR],
                            in_=xT_d.ap()[k * 128:(k + 1) * 128, 0:NR].bitcast(F32R))
        nc.scalar.dma_start(out=xT[k][:, N:N + NR],
                            in_=xT_d.ap()[k * 128:(k + 1) * 128, N:N + NR].bitcast(F32R))
    for k in range(EC):
        nc.scalar.dma_start(out=xT[k][:, NR:N],
                            in_=xT_d.ap()[k * 128:(k + 1) * 128, NR:N].bitcast(F32R))
        nc.scalar.dma_start(out=xT[k][:, N + NR:],
                            in_=xT_d.ap()[k * 128:(k + 1) * 128, N + NR:].bitcast(F32R))
    # FFN weights, loaded during attention (f1 f32r: folded in place later;
    # f2 bf16 to match the bf16 h1 moving operand)
    f1 = [wpool.tile([128, E], F32R, tag=f"f1_{k}", name=f"f1_{k}")
          for k in range(EC)]
    f2 = [wpool.tile([128, E], BF16, tag=f"f2_{k}", name=f"f2_{k}")
          for k in range(EC)]
    for k in range(EC):
        nc.scalar.dma_start(out=f1[k][:],
                            in_=f_d["f1"].ap()[k * 128:(k + 1) * 128, :].bitcast(F32R))
    for k in range(EC):
        nc.scalar.dma_start(out=f2[k][:],
                            in_=f_d["f2"].ap()[k * 128:(k + 1) * 128, :])

    # prefetch the exp ACT table set while input DMAs are in flight
    warm = const.tile([1, 1], F32, tag="warm", name="warm")
    nc.vector.memset(warm[:], 0.0)
    nc.scalar.activation(out=warm[:], in_=warm[:], func=AF.Exp, scale=1.0)
    negln8 = const.tile([128, 1], F32, tag="negln8", name="negln8")
    nc.gpsimd.memset(negln8[:], -LN8)

    # h-tilde (pre-BN1 attention output) accumulated across parts/batches,
    # with per-(m, batch-part) channel sums / sumsq for BN1
    ht = [act.tile([128, TOK], F32R, tag=f"ht{k}", name=f"ht{k}")
          for k in range(EC)]
    acc1 = [small.tile([128, 4], F32, tag=f"acc1_{m}", name=f"acc1_{m}", bufs=1)
            for m in range(EC)]
    sq1 = [small.tile([128, 4], F32, tag=f"sq1_{m}", name=f"sq1_{m}", bufs=1)
           for m in range(EC)]

    # ---------- attention ----------
    def proj_qkv(P):
        """q/k DR projections into bf16 channel-major tiles + v (DR) into the
        fp8 AV-layout tile. Returns per-part state."""
        st = _Ctx()
        st.tok0 = P.tok0
        Np = P.np
        st.qT = [attn.tile([128, NT], BF16, tag=f"qT{m}", name=f"qT{m}")
                 for m in range(EC)]
        st.kT = [attn.tile([128, NT], BF16, tag=f"kT{m}", name=f"kT{m}")
                 for m in range(EC)]
        # v8: [128 tok, kc, h, dk+1]; ones column = 1/64 for denominators
        st.v8 = attn.tile([128, 6, H, DK + 2], F8, tag="v8", name="v8")
        with nc.allow_low_precision(reason="fp8 attention"):
            nc.gpsimd.memset(st.v8[:, :, :, DK:DK + 1], IVS)
            nc.gpsimd.memset(st.v8[:, :, :, DK + 1:DK + 2], 0.0)

        def emit_qk(wt, o_t, bname, m):
            ps = ps_big.tile([128, 2, NT], F32, tag="sc", name="psq")
            for g in range(2):
                for off, fl in _bank_slices(0, Np, 256):
                    nc.tensor.matmul(
                        ps[:, 0, off:off + fl],
                        wt[:, g, :, m * 128:(m + 1) * 128],
                        x8[:, 2 * g:2 * g + 2, P.tok0 + off:P.tok0 + off + fl],
                        start=(g == 0), stop=(g == 1), perf_mode=DR)
            with nc.allow_low_precision(reason="bf16 qk"):
                nc.vector.tensor_scalar(
                    out=o_t[m][:, 0:Np], in0=ps[:, 0, 0:Np],
                    scalar1=bias[bname][:, m:m + 1], scalar2=None, op0=OP.add)

        def emit_qk_all():
            for m in range(EC):
                emit_qk(P.wq, st.qT, P.wn[0], m)
                emit_qk(P.wk, st.kT, P.wn[1], m)
                for t in v_sched[m]:
                    emit_v(t)

        def emit_v(t):
            ps = ps_big.tile([128, 2, NT], F32, tag="sc", name="psv")
            for g in range(2):
                for j0 in (0, 256):
                    nc.tensor.matmul(
                        ps[:, 0, j0:j0 + 256],
                        x8[:, 2 * g:2 * g + 2,
                           P.tok0 + t * 128:P.tok0 + (t + 1) * 128],
                        P.wv[:, g, :, j0:j0 + 256],
                        start=(g == 0), stop=(g == 1), perf_mode=DR)
            with nc.allow_low_precision(reason="fp8 v"):
                nc.vector.tensor_tensor(
                    out=st.v8[:, t, :, 0:DK],
                    in0=ps[:, 0, 0:E].rearrange("p (h d) -> p h d", h=H),
                    in1=vrep[P.wn[2]][:].rearrange("p (h d) -> p h d", h=H),
                    op=OP.add)

        v_sched = [[0], [1], [2, 3], [4, 5]] if P.nk == 6 else [[0], [1], [], []]
        emit_qk_all()
        return st

    def heads(P, st):
        """Per-head bf16 scores -> exp (fp8, /8 shift) -> DR AV with
        denominator row; evacuated fp8 and remapped into z8 via DMA."""
        Np = P.np
        st.z8 = attn.tile([128, 4, NT], F8, tag="z8", name="z8")
        rows_all = small.tile([98, NT], F8, tag="rows", name="rows")
        rinv_all = small.tile([98, NT], F32R, tag="rinv", name="rinv")
        st.rinv = rinv_all
        for pair in range(4):
            for j in range(2):
                h = 2 * pair + j
                qh = st.qT[h // 2][(h % 2) * 64:(h % 2) * 64 + 64, 0:Np]
                kh = st.kT[h // 2][(h % 2) * 64:(h % 2) * 64 + 64, 0:Np]
                zu = ps_av.tile([66, NT], F32, tag="av", name="av", bufs=1)
                for g in range(P.nk // 2):
                    sc = ps_big.tile([128, 2, NT], F32, tag="sc", name="sc")
                    for j2 in range(2):
                        kc = 2 * g + j2
                        for off, fl in _bank_slices(j2 * NT, Np):
                            nc.tensor.matmul(sc[:, j2, off:off + fl],
                                             kh[:, kc * 128:(kc + 1) * 128],
                                             qh[:, off:off + fl],
                                             start=True, stop=True)
                    ex = expp.tile([128, 2, NT], F8, tag="exp", name="exp",
                                   bufs=3)
                    with nc.allow_low_precision(reason="fp8 exp"):
                        nc.scalar.activation(
                            out=ex[:, :, 0:Np], in_=sc[:, :, 0:Np],
                            func=AF.Exp, scale=0.125, bias=negln8[:])
                    for off, fl in _bank_slices(0, Np, 256):
                        nc.tensor.matmul(
                            zu[:, off:off + fl], st.v8[:, 2 * g:2 * g + 2, h, :],
                            ex[:, :, off:off + fl],
                            start=(g == 0), stop=(g == P.nk // 2 - 1),
                            perf_mode=DR)
                zst = expp.tile([65, NT], F8, tag="zst", name="zst", bufs=2)
                with nc.allow_low_precision(reason="fp8 z"):
                    nc.vector.tensor_copy(out=zst[:, 0:Np], in_=zu[0:65, 0:Np])
                nc.sync.dma_start(out=st.z8[64 * j:64 * (j + 1), pair, 0:Np],
                                  in_=zst[0:64, 0:Np])
                nc.sync.dma_start(
                    out=rows_all[32 * pair + j:32 * pair + j + 1, 0:Np],
                    in_=zst[64:65, 0:Np])
                if j == 1:
                    with nc.allow_low_precision(reason="f32r feeds f32r mm"):
                        nc.vector.reciprocal(
                            out=rinv_all[32 * pair:32 * pair + 2, 0:Np],
                            in_=rows_all[32 * pair:32 * pair + 2, 0:Np])

    def denom_outproj(P, st, bp_idx):
        """Denominator broadcast (K=2 matmul of reciprocals), z8 scale, DR
        output projection, fused evac: (psum + bo) + x residual -> ht with
        BN1 channel sums via accum; sumsq square-pass split DVE/ACT."""
        Np = P.np
        tok0 = st.tok0
        for pair in range(4):
            rinv = st.rinv[32 * pair:32 * pair + 2, 0:Np]
            rep = ps_big.tile([128, 2, NT], F32, tag="sc", name="rep")
            tp = (96, 0) if pair == 3 else None
            for off, fl in _bank_slices(0, Np):
                nc.tensor.matmul(rep[:, 0, off:off + fl],
                                 sel2[32 * pair:32 * pair + 2, :],
                                 rinv[:, off:off + fl], start=True, stop=True,
                                 tile_position=tp)
            with nc.allow_low_precision(reason="fp8 z scale"):
                nc.vector.tensor_tensor(out=st.z8[:, pair, 0:Np],
                                        in0=st.z8[:, pair, 0:Np],
                                        in1=rep[:, 0, 0:Np], op=OP.mult)
        for m in range(EC):
            ps = ps_big.tile([128, 2, NT], F32, tag="sc", name="pso")
            for g in range(2):
                for off, fl in _bank_slices(0, Np, 256):
                    nc.tensor.matmul(
                        ps[:, 0, off:off + fl],
                        P.wo[:, g, :, m * 128:(m + 1) * 128],
                        st.z8[:, 2 * g:2 * g + 2, off:off + fl],
                        start=(g == 0), stop=(g == 1), perf_mode=DR)
            dst = ht[m][:, tok0:tok0 + Np]
            with nc.allow_low_precision(reason="f32r ht"):
                nc.vector.scalar_tensor_tensor(
                    out=dst, in0=ps[:, 0, 0:Np],
                    scalar=bias[P.wn[3]][:, m:m + 1],
                    in1=xT[m][:, tok0:tok0 + Np],
                    op0=OP.add, op1=OP.add,
                    accum_out=acc1[m][:, bp_idx:bp_idx + 1])
            # sumsq for BN1, alternating engines; scratch is write-only
            dv = dst.bitcast(F32)
            if m % 2 == 0:
                scr = small.tile([128, NT], F32, tag="sqd", name="sqd", bufs=2)
                nc.vector.scalar_tensor_tensor(
                    out=scr[:, 0:Np], in0=dv, scalar=1.0, in1=dv,
                    op0=OP.mult, op1=OP.mult,
                    accum_out=sq1[m][:, bp_idx:bp_idx + 1])
            else:
                scr = small.tile([128, NT], F32, tag="sqa", name="sqa", bufs=2)
                nc.scalar.activation(
                    out=scr[:, 0:Np], in_=dv, func=AF.Square,
                    accum_out=sq1[m][:, bp_idx:bp_idx + 1])

    parts = []
    for bp_idx, (part, b) in enumerate([(0, 0), (0, 1), (1, 0), (1, 1)]):
        P = _Ctx()
        P.part = part
        P.b = b
        P.bp_idx = bp_idx
        P.wn = ["rq", "rk", "rv", "ro"] if part == 0 else ["tq", "tk", "tv", "to"]
        P.np = NR if part == 0 else NT
        P.nk = P.np // 128
        P.tok0 = b * N + (0 if part == 0 else NR)
        P.wq, P.wk, P.wv, P.wo = (w8[P.wn[0]], w8[P.wn[1]], w8[P.wn[2]],
                                  w8[P.wn[3]])
        parts.append(P)
    # order: R0 R1 T0 T1 (robot both batches while task x8/weights load)
    order = [parts[0], parts[1], parts[2], parts[3]]
    sts = {}
    prev = None
    for P in order:
        sts[P.bp_idx] = proj_qkv(P)
        if prev is not None:
            denom_outproj(prev, sts[prev.bp_idx], prev.bp_idx)
        heads(P, sts[P.bp_idx])
        prev = P
    denom_outproj(prev, sts[prev.bp_idx], prev.bp_idx)

    # all exps done: swap the ACT table set to sqrt ahead of BN1
    warm2 = const.tile([1, 1], F32, tag="warm", name="warm2")
    nc.vector.memset(warm2[:], 1.0)
    nc.scalar.activation(out=warm2[:], in_=warm2[:], func=AF.Sqrt, scale=1.0)

    # ---------- BN1 (sums -> AllReduce -> params; fold into F1) ----------
    s1, t1 = _bn_params(nc, small, dram, acc1, sq1, bias["bn1_g"],
                        bias["bn1_b"], "bn1", for_timing)
    # b1' = f1_b + f1_w @ t1 (tiny matvec on original f1 tiles)
    b1p = small.tile([128, EC], F32, tag="b1p", name="b1p", bufs=1)
    t1r = small.tile([128, EC], F32R, tag="t1r", name="t1r", bufs=1)
    with nc.allow_low_precision(reason="f32r matvec input"):
        nc.vector.tensor_copy(out=t1r[:], in_=t1[:])
    psb = ps_big.tile([128, 2, NT], F32, tag="sc", name="psb1")
    for m in range(EC):
        for k in range(EC):
            nc.tensor.matmul(psb[:, 0, 2 * m:2 * m + 2],
                             f1[k][:, m * 128:(m + 1) * 128],
                             t1r[:, k:k + 1].to_broadcast((128, 2)),
                             start=(k == 0), stop=(k == EC - 1))
    nc.vector.tensor_tensor(out=b1p[:],
                            in0=psb[:, 0, 0:2 * EC:2], in1=bias["f1"],
                            op=OP.add)
    # fold BN1 scale into f1 (in place, per input-channel partition)
    for k in range(EC):
        with nc.allow_low_precision(reason="f32r weights"):
            nc.vector.tensor_scalar(out=f1[k][:], in0=f1[k][:],
                                    scalar1=s1[:, k:k + 1], scalar2=None,
                                    op0=OP.mult)

    # hn = s1*ht + t1 (BN1 output, residual only) -> xT slots
    hn = [const.tile([128, TOK], F32R, tag=f"xT{k}", name=f"hn{k}")
          for k in range(EC)]
    for m in range(EC):
        for i, (off, fl) in enumerate(_bank_slices(0, TOK)):
            src = ht[m][:, off:off + fl].bitcast(F32)
            dstv = hn[m][:, off:off + fl]
            with nc.allow_low_precision(reason="f32r hn"):
                if i % 2 == 0:
                    nc.gpsimd.tensor_scalar(out=dstv, in0=src,
                                            scalar1=s1[:, m:m + 1],
                                            scalar2=t1[:, m:m + 1],
                                            op0=OP.mult, op1=OP.add)
                else:
                    nc.scalar.activation(out=dstv, in_=src, func=AF.Identity,
                                         bias=t1[:, m:m + 1],
                                         scale=s1[:, m:m + 1])

    # ---------- FFN ----------
    h1 = [act.tile([128, TOK], BF16, tag=f"h1_{k}", name=f"h1_{k}")
          for k in range(EC)]
    for m in range(EC):
        for off, fl in _bank_slices(0, TOK):
            ps = ps_big.tile([128, 2, NT], F32, tag="sc", name="psf1")
            for k in range(EC):
                nc.tensor.matmul(ps[:, 0, 0:fl], f1[k][:, m * 128:(m + 1) * 128],
                                 ht[k][:, off:off + fl],
                                 start=(k == 0), stop=(k == EC - 1))
            with nc.allow_low_precision(reason="bf16 h1"):
                if m % 2 == 0:
                    nc.scalar.activation(out=h1[m][:, off:off + fl],
                                         in_=ps[:, 0, 0:fl], func=AF.Relu,
                                         bias=b1p[:, m:m + 1], scale=1.0)
                else:
                    nc.vector.tensor_scalar(out=h1[m][:, off:off + fl],
                                            in0=ps[:, 0, 0:fl],
                                            scalar1=b1p[:, m:m + 1],
                                            scalar2=0.0, op0=OP.add, op1=OP.max)
    ho = [act.tile([128, TOK], F32, tag=f"ht{k}", name=f"ho{k}")
          for k in range(EC)]
    acc2 = [small.tile([128, 4], F32, tag=f"acc2_{m}", name=f"acc2_{m}", bufs=1)
            for m in range(EC)]
    sq2 = [small.tile([128, 4], F32, tag=f"sq2_{m}", name=f"sq2_{m}", bufs=1)
           for m in range(EC)]
    for m in range(EC):
        for i, (off, fl) in enumerate(_bank_slices(0, TOK)):
            ps = ps_big.tile([128, 2, NT], F32, tag="sc", name="psf2")
            for k in range(EC):
                nc.tensor.matmul(ps[:, 0, 0:fl], f2[k][:, m * 128:(m + 1) * 128],
                                 h1[k][:, off:off + fl],
                                 start=(k == 0), stop=(k == EC - 1))
            dst = ho[m][:, off:off + fl]
            nc.vector.scalar_tensor_tensor(
                out=dst, in0=ps[:, 0, 0:fl], scalar=bias["f2"][:, m:m + 1],
                in1=hn[m][:, off:off + fl].bitcast(F32),
                op0=OP.add, op1=OP.add,
                accum_out=acc2[m][:, i:i + 1])
            # sumsq for BN2, alternating engines
            if i % 2 == 0:
                scr = small.tile([128, NT], F32, tag="sqd", name="sqd2", bufs=2)
                nc.vector.scalar_tensor_tensor(
                    out=scr[:, 0:fl], in0=dst, scalar=1.0, in1=dst,
                    op0=OP.mult, op1=OP.mult,
                    accum_out=sq2[m][:, i:i + 1])
            else:
                scr = small.tile([128, NT], F32, tag="sqa", name="sqa2", bufs=2)
                nc.scalar.activation(
                    out=scr[:, 0:fl], in_=dst, func=AF.Square,
                    accum_out=sq2[m][:, i:i + 1])

    # ---------- BN2 + output (pipelined per 512-token slice) ----------
    s2, t2 = _bn_params(nc, small, dram, acc2, sq2, bias["bn2_g"],
                        bias["bn2_b"], "bn2", for_timing)
    for i, (off, fl) in enumerate(_bank_slices(0, TOK)):
        for m in range(EC):
            dst = ho[m][:, off:off + fl]
            eng = (i * EC + m) % 3
            if eng == 0:
                nc.vector.tensor_scalar(out=dst, in0=dst,
                                        scalar1=s2[:, m:m + 1],
                                        scalar2=t2[:, m:m + 1],
                                        op0=OP.mult, op1=OP.add)
            elif eng == 1:
                nc.scalar.activation(out=dst, in_=dst, func=AF.Identity,
                                     bias=t2[:, m:m + 1], scale=s2[:, m:m + 1])
            else:
                nc.gpsimd.tensor_scalar(out=dst, in0=dst,
                                        scalar1=s2[:, m:m + 1],
                                        scalar2=t2[:, m:m + 1],
                                        op0=OP.mult, op1=OP.add)
            nc.sync.dma_start(out=yT_d.ap()[m * 128:(m + 1) * 128, off:off + fl],
                              in_=dst)


def _bn_params(nc, small, dram, accs, sqs, g_sb, b_sb, name, for_timing=False):
    """Per-channel scale/shift for training-mode BN over all B*N tokens from
    raw per-(m, slice) sums: reduce -> 8-core AllReduce -> mu/var ->
    sqrt+recip (+1 Newton step). Returns (s [128, EC], t [128, EC])."""
    ccin = dram.tile([128, 2 * EC], F32, tag=f"cci_{name}", name=f"cci_{name}")
    ccout = dram.tile([128, 2 * EC], F32, tag=f"cco_{name}", name=f"cco_{name}")
    su = small.tile([128, 2 * EC], F32, tag=f"su_{name}", name=f"su_{name}")
    suv = su[:].rearrange("p (c two) -> p c two", two=2)
    for m in range(EC):
        nc.vector.tensor_reduce(out=suv[:, m, 0:1], in_=accs[m][:],
                                axis=mybir.AxisListType.X, op=OP.add)
        nc.vector.tensor_reduce(out=suv[:, m, 1:2], in_=sqs[m][:],
                                axis=mybir.AxisListType.X, op=OP.add)
    nc.sync.dma_start(out=ccin[:], in_=su[:])
    if for_timing:
        # TimelineSim cannot model collectives; substitute a same-shape copy
        nc.gpsimd.dma_start(out=ccout[:], in_=ccin[:])
    else:
        nc.gpsimd.collective_compute(
            "AllReduce", OP.add, replica_groups=[list(range(N_CORES))],
            ins=[ccin.opt()], outs=[ccout.opt()])
    gsa = small.tile([128, 2 * EC], F32, tag=f"gs_{name}", name=f"gs_{name}")
    nc.sync.dma_start(out=gsa[:], in_=ccout[:])
    gv = gsa[:].rearrange("p (c two) -> p c two", two=2)
    mu = small.tile([128, EC], F32, tag=f"mu_{name}", name=f"mu_{name}", bufs=1)
    var = small.tile([128, EC], F32, tag=f"var_{name}", name=f"var_{name}",
                     bufs=1)
    t2 = small.tile([128, EC], F32, tag=f"t2_{name}", name=f"t2_{name}", bufs=1)
    nc.vector.tensor_scalar(out=mu[:], in0=gv[:, :, 0],
                            scalar1=1.0 / N_GLOBAL, scalar2=None, op0=OP.mult)
    nc.vector.tensor_scalar(out=t2[:], in0=gv[:, :, 1],
                            scalar1=1.0 / N_GLOBAL, scalar2=None, op0=OP.mult)
    nc.vector.tensor_tensor(out=var[:], in0=mu[:], in1=mu[:], op=OP.mult)
    nc.vector.tensor_tensor(out=var[:], in0=t2[:], in1=var[:], op=OP.subtract)
    # r = 1/sqrt(var + eps): ACT Sqrt + DVE reciprocal, then one Newton step
    epst = small.tile([128, 1], F32, tag=f"eps_{name}", name=f"eps_{name}",
                      bufs=1)
    nc.vector.memset(epst[:], EPS)
    sq = small.tile([128, EC], F32, tag=f"sq_{name}", name=f"sq_{name}", bufs=1)
    nc.scalar.activation(out=sq[:], in_=var[:], func=AF.Sqrt, bias=epst[:],
                         scale=1.0)
    r0 = small.tile([128, EC], F32, tag=f"r0_{name}", name=f"r0_{name}", bufs=1)
    nc.vector.reciprocal(out=r0[:], in_=sq[:])
    av_ = small.tile([128, EC], F32, tag=f"a_{name}", name=f"a_{name}", bufs=1)
    nc.vector.tensor_scalar(out=av_[:], in0=var[:], scalar1=EPS, scalar2=None,
                            op0=OP.add)
    nt = small.tile([128, EC], F32, tag=f"nt_{name}", name=f"nt_{name}", bufs=1)
    nc.vector.tensor_tensor(out=nt[:], in0=r0[:], in1=r0[:], op=OP.mult)
    nc.vector.tensor_tensor(out=nt[:], in0=nt[:], in1=av_[:], op=OP.mult)
    nc.vector.tensor_scalar(out=nt[:], in0=nt[:], scalar1=-0.5, scalar2=1.5,
                            op0=OP.mult, op1=OP.add)
    r = small.tile([128, EC], F32, tag=f"r_{name}", name=f"r_{name}", bufs=1)
    nc.vector.tensor_tensor(out=r[:], in0=r0[:], in1=nt[:], op=OP.mult)
    s_all = small.tile([128, EC], F32, tag=f"s_{name}", name=f"s_{name}",
                       bufs=1)
    sh_all = small.tile([128, EC], F32, tag=f"sh_{name}", name=f"sh_{name}",
                        bufs=1)
    nc.vector.tensor_tensor(out=s_all[:], in0=r[:], in1=g_sb, op=OP.mult)
    nc.vector.tensor_tensor(out=sh_all[:], in0=mu[:], in1=s_all[:], op=OP.mult)
    nc.vector.tensor_tensor(out=sh_all[:], in0=b_sb, in1=sh_all[:],
                            op=OP.subtract)
    return s_all, sh_all


_NC_CACHE = None


def _get_nc():
    global _NC_CACHE
    if _NC_CACHE is None:
        _NC_CACHE = build()
    return _NC_CACHE


def make_in_maps(inputs):
    import ml_dtypes
    f8 = ml_dtypes.float8_e4m3
    shared = {}
    for n in W8_NAMES:
        w = np.asarray(inputs[f"{n}_w"], dtype=np.float32)      # [E, E]
        # w8[p, g, jt, j] = W[j, (2g+jt)*128 + p]
        w8 = np.ascontiguousarray(
            w.T.reshape(2, 2, 128, E).transpose(2, 0, 1, 3)).astype(f8)
        shared[f"{n}_w8"] = w8.reshape(128, 4 * E)
    shared["f1_wT"] = np.ascontiguousarray(
        np.asarray(inputs["f1_w"], dtype=np.float32).T)
    shared["f2_wT"] = np.ascontiguousarray(
        np.asarray(inputs["f2_w"], dtype=np.float32).T).astype(
            ml_dtypes.bfloat16)
    for n in ["rv", "tv"]:
        shared[f"{n}_brep"] = np.ascontiguousarray(
            np.broadcast_to(np.asarray(inputs[f"{n}_b"], dtype=np.float32),
                            (128, E)))
    bpk = np.empty((128, len(ALL_B) * EC), dtype=np.float32)
    for i, n in enumerate(ALL_B):
        vec = inputs[f"{n}_b"] if n in W8_NAMES + ["f1", "f2"] else inputs[n]
        bpk[:, i * EC:(i + 1) * EC] = np.asarray(vec).reshape(EC, 128).T
    shared["bpk"] = bpk
    sel2 = np.zeros((98, 128), dtype=np.float32)
    for p in range(4):
        sel2[32 * p, 0:64] = IVS
        sel2[32 * p + 1, 64:128] = IVS
    shared["sel2"] = sel2

    x = np.asarray(inputs["x"], dtype=np.float32)
    in_maps = []
    for i in range(N_CORES):
        xc = x[BL * i:BL * (i + 1)]                      # [BL, N, E]
        xT = np.ascontiguousarray(xc.transpose(2, 0, 1).reshape(E, TOK))
        x8 = np.ascontiguousarray(
            xT.reshape(EC, 128, TOK).transpose(1, 0, 2)).astype(f8)
        in_maps.append({"xT": xT, "x8": x8.reshape(128, EC * TOK), **shared})
    return in_maps


def assemble_output(results):
    y = np.empty((B, N, E), dtype=np.float32)
    for i in range(N_CORES):
        yT = results[i]["yT"]                            # [E, TOK]
        y[BL * i:BL * (i + 1)] = yT.reshape(E, BL, N).transpose(1, 2, 0)
    return y


def kernel(**inputs):
    nc = _get_nc()
    in_maps = make_in_maps(inputs)
    res = run_bass_kernel_spmd(nc, in_maps, core_ids=list(range(N_CORES)))
    return assemble_output(res.results)


if __name__ == "__main__":
    nc = build()
    print("build ok")


# revision 36
# speedup vs baseline: 1.1757x; 1.0476x over previous
"""Trainium2 Bass kernel for nn_EncoderBlock (dual self-attention + BN + FFN + BN).

Sharding: data-parallel over batch (16 batches -> 2 per core on 8 cores).

v2: the attention block runs in fp8e4m3 with DoubleRow matmuls (4x PE rate on
the qkv/out projections and AV), validated to ~8e-4 end-to-end rel err.
Scores stay bf16 (64-deep contraction can't DoubleRow). exp outputs fp8
directly from ACT with the softmax /8 range shift folded into the exp bias;
the ones-column (value 1/64) inside V yields denominators from the AV matmul.
K/Q/O biases are per-partition scalars in the evacuation ops (K's provably
cancels in softmax but is applied anyway); V's bias rides the existing
psum->v8 add. The out-projection evacuation is a fused scalar_tensor_tensor
(psum + bias + residual) whose accum_out doubles as the BN1 channel sums;
sumsq comes from a square pass split across DVE/ACT. BN1's scale is folded
into the F1 weights (in-place) so the FFN starts right after the AllReduce;
BN1's shift becomes an F1 bias correction via a tiny matvec. BN2 stats use
the same accum trick; the final normalize+store is pipelined per 512-column
slice across DVE/ACT/Pool with immediate per-slice DMA.
BatchNorm batch stats use a 4KB AllReduce across the 8 cores (twice).
"""

import numpy as np
import concourse.bass as bass
import concourse.bacc as bacc
import concourse.tile as tile
from concourse import mybir
from concourse.bass_utils import run_bass_kernel_spmd

dt = mybir.dt
F32 = dt.float32
F32R = dt.float32r
BF16 = dt.bfloat16
F8 = dt.float8e4
AF = mybir.ActivationFunctionType
OP = mybir.AluOpType
DR = mybir.MatmulPerfMode.DoubleRow

N_CORES = 8
B, N, E, H, DK = 16, 1024, 512, 8, 64
NR, NT = 256, 768          # robot / task sequence lengths
BL = B // N_CORES          # local batches per core
TOK = BL * N               # local tokens per core
EC = E // 128              # channel chunks of 128
N_GLOBAL = B * N           # BN stat count
EPS = 1e-5
LN8 = 2.0794415416798357   # exp range shift: ex = exp(s)/8
IVS = 1.0 / 64.0           # ones-column value (denominator scale)

W8_NAMES = ["rq", "rk", "rv", "ro", "tq", "tk", "tv", "to"]
ALL_B = W8_NAMES + ["f1", "f2", "bn1_g", "bn1_b", "bn2_g", "bn2_b"]


def _bank_slices(base, length, maxlen=512):
    """Split [base, base+length) into pieces (<=maxlen) that never cross a
    512-col PSUM bank boundary."""
    out = []
    cur = base
    end = base + length
    while cur < end:
        nb = (cur // 512 + 1) * 512
        fl = min(end, min(nb, cur + maxlen)) - cur
        out.append((cur - base, fl))
        cur += fl
    return out


def build(for_timing=False):
    nc = bacc.Bacc("TRN2", target_bir_lowering=False, debug=False,
                   num_devices=N_CORES)

    xT_d = nc.dram_tensor("xT", [E, TOK], F32, kind="ExternalInput")
    x8r_d = nc.dram_tensor("x8r", [128, EC * 2 * NR], F8, kind="ExternalInput")
    x8t_d = nc.dram_tensor("x8t", [128, EC * 2 * NT], F8, kind="ExternalInput")
    w8_d = {n: nc.dram_tensor(f"{n}_w8", [128, 4 * E], F8, kind="ExternalInput")
            for n in W8_NAMES}
    f_d = {"f1": nc.dram_tensor("f1_wT", [E, E], F32, kind="ExternalInput"),
           "f2": nc.dram_tensor("f2_wT", [E, E], BF16, kind="ExternalInput")}
    bpk_d = nc.dram_tensor("bpk", [128, len(ALL_B) * EC], F32,
                           kind="ExternalInput")
    vrep_d = {n: nc.dram_tensor(f"{n}_brep", [128, E], F32, kind="ExternalInput")
              for n in ["rv", "tv"]}
    sel2_d = nc.dram_tensor("sel2", [98, 128], F32, kind="ExternalInput")
    yT_d = nc.dram_tensor("yT", [E, TOK], F32, kind="ExternalOutput")

    from contextlib import ExitStack
    with tile.TileContext(nc) as tc, ExitStack() as es:
        const = es.enter_context(tc.tile_pool(name="const", bufs=1))
        wpool = es.enter_context(tc.tile_pool(name="w", bufs=1))
        act = es.enter_context(tc.tile_pool(name="act", bufs=1))
        attn = es.enter_context(tc.tile_pool(name="attn", bufs=2))
        expp = es.enter_context(tc.tile_pool(name="expp", bufs=2))
        small = es.enter_context(tc.tile_pool(name="small", bufs=2))
        dram = es.enter_context(tc.tile_pool(name="dram", bufs=1, space="DRAM"))
        ps_big = es.enter_context(tc.tile_pool(name="ps_big", bufs=2, space="PSUM"))
        ps_av = es.enter_context(tc.tile_pool(name="ps_av", bufs=2, space="PSUM"))
        _body(nc, const, wpool, act, attn, expp, small, dram, ps_big, ps_av,
              xT_d, x8r_d, x8t_d, w8_d, f_d, bpk_d, vrep_d, sel2_d, yT_d,
              for_timing)
    nc.finalize()
    return nc


class _Ctx:
    pass


def _body(nc, const, wpool, act, attn, expp, small, dram, ps_big, ps_av,
          xT_d, x8r_d, x8t_d, w8_d, f_d, bpk_d, vrep_d, sel2_d, yT_d,
          for_timing):
    # ---------- constants / inputs resident in SBUF ----------
    # DMA emission order == issue order on the sync queue: the first
    # projection needs rq/rk weights + robot x8 columns; bulk (xT, task x8,
    # FFN weights) trickles in behind.
    w8 = {}
    for n in ["rq", "rk"]:
        t = wpool.tile([128, 2, 2, E], F8, tag=f"w8{n}", name=f"w8{n}")
        nc.sync.dma_start(out=t[:], in_=w8_d[n].ap())
        w8[n] = t
    x8r = const.tile([128, EC, 2, NR], F8, tag="x8r", name="x8r")
    x8t = const.tile([128, EC, 2, NT], F8, tag="x8t", name="x8t")
    nc.sync.dma_start(out=x8r[:], in_=x8r_d.ap())
    bpk = const.tile([128, len(ALL_B) * EC], F32, tag="bpk", name="bpk")
    nc.sync.dma_start(out=bpk[:], in_=bpk_d.ap())
    bias = {n: bpk[:, i * EC:(i + 1) * EC] for i, n in enumerate(ALL_B)}
    for n in ["rv", "ro"]:
        t = wpool.tile([128, 2, 2, E], F8, tag=f"w8{n}", name=f"w8{n}")
        nc.sync.dma_start(out=t[:], in_=w8_d[n].ap())
        w8[n] = t
    vrep = {}
    t = const.tile([128, E], F32, tag="vr_rv", name="vr_rv")
    nc.sync.dma_start(out=t[:], in_=vrep_d["rv"].ap())
    vrep["rv"] = t
    sel2 = const.tile([98, 128], F32R, tag="sel2", name="sel2")
    nc.sync.dma_start(out=sel2[:], in_=sel2_d.ap().bitcast(F32R))
    # Bulk loads ride the Pool SWDGE train (no HWDGE contention with the
    # latency-critical z8/rows transfers on sync), ordered by need time:
    # xT robot (outproj R0 ~18us) -> task x8/weights (T0 proj ~30us) ->
    # xT task (outproj T0 ~110us) -> FFN weights (BN1 ~170us).
    xT = [const.tile([128, TOK], F32R, tag=f"xT{k}", name=f"xT{k}")
          for k in range(EC)]
    for k in range(EC):
        nc.sync.dma_start(out=xT[k][:, 0:NR],
                          in_=xT_d.ap()[k * 128:(k + 1) * 128, 0:NR].bitcast(F32R))
        nc.sync.dma_start(out=xT[k][:, N:N + NR],
                          in_=xT_d.ap()[k * 128:(k + 1) * 128, N:N + NR].bitcast(F32R))
    x8td = x8t_d.ap().rearrange("p (k b t) -> p k b t", k=EC, b=2)
    nc.gpsimd.dma_start(out=x8t[:, :, 0, :], in_=x8td[:, :, 0, :])
    nc.gpsimd.dma_start(out=x8t[:, :, 1, :], in_=x8td[:, :, 1, :])
    for n in ["tq", "tk", "tv", "to"]:
        t = wpool.tile([128, 2, 2, E], F8, tag=f"w8{n}", name=f"w8{n}")
        nc.gpsimd.dma_start(out=t[:], in_=w8_d[n].ap())
        w8[n] = t
    t = const.tile([128, E], F32, tag="vr_tv", name="vr_tv")
    nc.gpsimd.dma_start(out=t[:], in_=vrep_d["tv"].ap())
    vrep["tv"] = t
    for k in range(EC):
        nc.sync.dma_start(out=xT[k][:, NR:N],
                          in_=xT_d.ap()[k * 128:(k + 1) * 128, NR:N].bitcast(F32R))
        nc.sync.dma_start(out=xT[k][:, N + NR:],
                          in_=xT_d.ap()[k * 128:(k + 1) * 128, N + NR:].bitcast(F32R))
    # FFN weights (f1 f32r: folded in place later; f2 bf16 to match bf16 h1)
    f1 = [wpool.tile([128, E], F32R, tag=f"f1_{k}", name=f"f1_{k}")
          for k in range(EC)]
    f2 = [wpool.tile([128, E], BF16, tag=f"f2_{k}", name=f"f2_{k}")
          for k in range(EC)]
    for k in range(EC):
        nc.gpsimd.dma_start(out=f1[k][:],
                            in_=f_d["f1"].ap()[k * 128:(k + 1) * 128, :].bitcast(F32R))
    for k in range(EC):
        nc.gpsimd.dma_start(out=f2[k][:],
                            in_=f_d["f2"].ap()[k * 128:(k + 1) * 128, :])

    # prefetch the exp ACT table set while input DMAs are in flight
    warm = const.tile([1, 1], F32, tag="warm", name="warm")
    nc.vector.memset(warm[:], 0.0)
    nc.scalar.activation(out=warm[:], in_=warm[:], func=AF.Exp, scale=1.0)
    negln8 = const.tile([128, 1], F32, tag="negln8", name="negln8")
    nc.gpsimd.memset(negln8[:], -LN8)
    epst = const.tile([128, 1], F32, tag="epst", name="epst")
    nc.gpsimd.memset(epst[:], EPS)

    # h-tilde (pre-BN1 attention output) accumulated across parts/batches,
    # with per-(m, batch-part) channel sums / sumsq for BN1
    ht = [act.tile([128, TOK], F32R, tag=f"ht{k}", name=f"ht{k}")
          for k in range(EC)]
    acc1 = small.tile([128, EC, 4], F32, tag="acc1", name="acc1", bufs=1)
    sq1 = small.tile([128, EC, 4], F32, tag="sq1", name="sq1", bufs=1)

    # ---------- attention (fine-grained interleaved emission) ----------
    # Per part, per m-chunk: project q/k chunk m, interleave the PREVIOUS
    # part's output-projection chunk m, then run head pair m (scores -> exp
    # -> AV -> evac) and its denominator broadcast + z8 scale. The exp stream
    # on ACT paces everything; PE/DVE work rides underneath it. Robot parts
    # are DVE-bound instead, so their k/zu evacuations go to ACT.
    def make_state(P):
        st = _Ctx()
        st.tok0 = P.tok0
        st.qT = [attn.tile([128, NT], BF16, tag=f"qT{m}", name=f"qT{m}")
                 for m in range(EC)]
        st.kT = [attn.tile([128, NT], BF16, tag=f"kT{m}", name=f"kT{m}")
                 for m in range(EC)]
        st.v8 = attn.tile([128, 6, H, DK + 2], F8, tag="v8", name="v8")
        st.z8 = attn.tile([128, 4, NT], F8, tag="z8", name="z8")
        st.rows = small.tile([98, NT], F8, tag="rows", name="rows")
        st.rinv = small.tile([98, NT], F32R, tag="rinv", name="rinv")
        st.x8p = x8r[:, :, P.b, :] if P.part == 0 else x8t[:, :, P.b, :]
        with nc.allow_low_precision(reason="fp8 attention"):
            nc.vector.memset(st.v8[:, :, :, DK:DK + 1], IVS)
            nc.vector.memset(st.v8[:, :, :, DK + 1:DK + 2], 0.0)
        return st

    def emit_qk(P, st, which, m):
        Np = P.np
        wt, o_t, bname = ((P.wq, st.qT, P.wn[0]) if which == "q"
                          else (P.wk, st.kT, P.wn[1]))
        ps = ps_big.tile([128, 2, NT], F32, tag="sc", name="psq")
        for g in range(2):
            for off, fl in _bank_slices(0, Np, 256):
                nc.tensor.matmul(
                    ps[:, 0, off:off + fl],
                    wt[:, g, :, m * 128:(m + 1) * 128],
                    st.x8p[:, 2 * g:2 * g + 2, off:off + fl],
                    start=(g == 0), stop=(g == 1), perf_mode=DR)
        with nc.allow_low_precision(reason="bf16 qk"):
            nc.vector.tensor_scalar(
                out=o_t[m][:, 0:Np], in0=ps[:, 0, 0:Np],
                scalar1=bias[bname][:, m:m + 1], scalar2=None, op0=OP.add)

    def emit_v(P, st, t):
        ps = ps_big.tile([128, 2, NT], F32, tag="sc", name="psv")
        for g in range(2):
            for j0 in (0, 256):
                nc.tensor.matmul(
                    ps[:, 0, j0:j0 + 256],
                    st.x8p[:, 2 * g:2 * g + 2, t * 128:(t + 1) * 128],
                    P.wv[:, g, :, j0:j0 + 256],
                    start=(g == 0), stop=(g == 1), perf_mode=DR)
        with nc.allow_low_precision(reason="fp8 v"):
            nc.vector.tensor_tensor(
                out=st.v8[:, t, :, 0:DK],
                in0=ps[:, 0, 0:E].rearrange("p (h d) -> p h d", h=H),
                in1=vrep[P.wn[2]][:].rearrange("p (h d) -> p h d", h=H),
                op=OP.add)

    def emit_scores(P, st, pair, j):
        Np = P.np
        qh = st.qT[pair][j * 64:j * 64 + 64, 0:Np]
        kh = st.kT[pair][j * 64:j * 64 + 64, 0:Np]
        exs = []
        for g in range(P.nk // 2):
            sc = ps_big.tile([128, 2, NT], F32, tag="sc", name="sc")
            for j2 in range(2):
                kc = 2 * g + j2
                for off, fl in _bank_slices(j2 * NT, Np):
                    nc.tensor.matmul(sc[:, j2, off:off + fl],
                                     kh[:, kc * 128:(kc + 1) * 128],
                                     qh[:, off:off + fl],
                                     start=True, stop=True)
            ex = expp.tile([128, 2, NT], F8, tag="exp", name="exp", bufs=6)
            with nc.allow_low_precision(reason="fp8 exp"):
                nc.scalar.activation(
                    out=ex[:, :, 0:Np], in_=sc[:, :, 0:Np],
                    func=AF.Exp, scale=0.125, bias=negln8[:])
            exs.append(ex)
        return exs

    def emit_avs(P, st, pair, j, exs, zst_box):
        Np = P.np
        h = 2 * pair + j
        zu = ps_av.tile([66, NT], F32, tag="av", name="av", bufs=1)
        for g in range(P.nk // 2):
            for off, fl in _bank_slices(0, Np, 256):
                nc.tensor.matmul(
                    zu[:, off:off + fl], st.v8[:, 2 * g:2 * g + 2, h, :],
                    exs[g][:, :, off:off + fl],
                    start=(g == 0), stop=(g == P.nk // 2 - 1), perf_mode=DR)
        if j == 0:
            zst_box[0] = expp.tile([65, 2, NT], F8, tag="zst", name="zst",
                                   bufs=2)
        zst = zst_box[0]
        with nc.allow_low_precision(reason="fp8 z"):
            nc.vector.tensor_copy(out=zst[:, j, 0:Np], in_=zu[0:65, 0:Np])
        nc.sync.dma_start(out=st.z8[64 * j:64 * (j + 1), pair, 0:Np],
                          in_=zst[0:64, j, 0:Np])
        if j == 1:
            nc.sync.dma_start(out=st.rows[32 * pair:32 * pair + 2, 0:Np],
                              in_=zst[64:65, :, 0:Np])
            with nc.allow_low_precision(reason="f32r feeds f32r mm"):
                nc.vector.reciprocal(
                    out=st.rinv[32 * pair:32 * pair + 2, 0:Np],
                    in_=st.rows[32 * pair:32 * pair + 2, 0:Np])

    def emit_pair_denom(P, st, pair):
        Np = P.np
        rinv = st.rinv[32 * pair:32 * pair + 2, 0:Np]
        rep = ps_big.tile([128, 2, NT], F32, tag="sc", name="rep")
        tp = (96, 0) if pair == 3 else None
        for off, fl in _bank_slices(0, Np):
            nc.tensor.matmul(rep[:, 0, off:off + fl],
                             sel2[32 * pair:32 * pair + 2, :],
                             rinv[:, off:off + fl], start=True, stop=True,
                             tile_position=tp)
        with nc.allow_low_precision(reason="fp8 z scale"):
            nc.vector.tensor_tensor(out=st.z8[:, pair, 0:Np],
                                    in0=st.z8[:, pair, 0:Np],
                                    in1=rep[:, 0, 0:Np], op=OP.mult)

    def emit_outproj(P, st, m):
        Np = P.np
        tok0 = st.tok0
        ps = ps_big.tile([128, 2, NT], F32, tag="sc", name="pso")
        for g in range(2):
            for off, fl in _bank_slices(0, Np, 256):
                nc.tensor.matmul(
                    ps[:, 0, off:off + fl],
                    P.wo[:, g, :, m * 128:(m + 1) * 128],
                    st.z8[:, 2 * g:2 * g + 2, off:off + fl],
                    start=(g == 0), stop=(g == 1), perf_mode=DR)
        dst = ht[m][:, tok0:tok0 + Np]
        with nc.allow_low_precision(reason="f32r ht"):
            nc.vector.scalar_tensor_tensor(
                out=dst, in0=ps[:, 0, 0:Np],
                scalar=bias[P.wn[3]][:, m:m + 1],
                in1=xT[m][:, tok0:tok0 + Np],
                op0=OP.add, op1=OP.add,
                accum_out=acc1[:, m, P.bp_idx:P.bp_idx + 1])
        dv = dst.bitcast(F32)
        if not getattr(P, "sq_act", False):
            scr = small.tile([128, NT], F32, tag="sqd", name="sqd", bufs=2)
            nc.vector.scalar_tensor_tensor(
                out=scr[:, 0:Np], in0=dv, scalar=1.0, in1=dv,
                op0=OP.mult, op1=OP.mult,
                accum_out=sq1[:, m, P.bp_idx:P.bp_idx + 1])
        else:
            scr = small.tile([128, NT], F32, tag="sqa", name="sqa", bufs=2)
            nc.scalar.activation(
                out=scr[:, 0:Np], in_=dv, func=AF.Square,
                accum_out=sq1[:, m, P.bp_idx:P.bp_idx + 1])

    parts = []
    for bp_idx, (part, b) in enumerate([(0, 0), (0, 1), (1, 0), (1, 1)]):
        P = _Ctx()
        P.part = part
        P.b = b
        P.bp_idx = bp_idx
        P.wn = ["rq", "rk", "rv", "ro"] if part == 0 else ["tq", "tk", "tv", "to"]
        P.np = NR if part == 0 else NT
        P.nk = P.np // 128
        P.tok0 = b * N + (0 if part == 0 else NR)
        P.wq, P.wk, P.wv, P.wo = (w8[P.wn[0]], w8[P.wn[1]], w8[P.wn[2]],
                                  w8[P.wn[3]])
        parts.append(P)

    v_scheds = {6: [[0], [1], [2, 3], [4, 5]], 2: [[0], [1], [], []]}
    sts = {0: make_state(parts[0])}
    P0 = parts[0]
    for m in range(EC):
        emit_qk(P0, sts[0], "q", m)
        emit_qk(P0, sts[0], "k", m)
        for t in v_scheds[P0.nk][m]:
            emit_v(P0, sts[0], t)
    pend = [None]

    def flush():
        if pend[0] is not None:
            pend[0]()
            pend[0] = None

    prev = None
    for i, P in enumerate(parts):
        st = sts[i]
        P.sq_act = (i == 0 or i == 3)  # outproj runs during R1/trailing
        nxt = parts[i + 1] if i + 1 < len(parts) else None
        if nxt is not None:
            sts[i + 1] = make_state(nxt)
        zst_box = [None]
        for pair in range(4):
            for j in (0, 1):
                exs = emit_scores(P, st, pair, j)
                flush()

                def mk(P=P, st=st, pair=pair, j=j, exs=exs, zb=zst_box,
                       prev=prev, nxt=nxt, i=i):
                    def run():
                        emit_avs(P, st, pair, j, exs, zb)
                        if j == 1:
                            # per-pair boundary work, all dependency-ready:
                            # previous pair's denominators, the previous
                            # part's output projection, next part's proj
                            if pair >= 1:
                                emit_pair_denom(P, st, pair - 1)
                            elif prev is not None:
                                emit_pair_denom(prev[0], prev[1], 3)
                            if prev is not None:
                                emit_outproj(prev[0], prev[1], pair)
                            if nxt is not None:
                                emit_qk(nxt, sts[i + 1], "q", pair)
                                emit_qk(nxt, sts[i + 1], "k", pair)
                                for t in v_scheds[nxt.nk][pair]:
                                    emit_v(nxt, sts[i + 1], t)
                    return run
                pend[0] = mk()
        prev = (P, st)
    flush()
    emit_pair_denom(prev[0], prev[1], 3)
    for m in range(EC):
        emit_outproj(prev[0], prev[1], m)

    # all exps done: swap the ACT table set to sqrt ahead of BN1
    warm2 = const.tile([1, 1], F32, tag="warm", name="warm2")
    nc.vector.memset(warm2[:], 1.0)
    nc.scalar.activation(out=warm2[:], in_=warm2[:], func=AF.Sqrt, scale=1.0)

    # ---------- BN1 (sums -> AllReduce -> params; fold into F1) ----------
    s1, t1 = _bn_params(nc, small, dram, acc1, sq1, bias["bn1_g"],
                        bias["bn1_b"], epst[:], "bn1", for_timing)
    # b1' = f1_b + f1_w @ t1 (tiny matvec on original f1 tiles)
    b1p = small.tile([128, EC], F32, tag="b1p", name="b1p", bufs=1)
    t1r = small.tile([128, EC], F32R, tag="t1r", name="t1r", bufs=1)
    with nc.allow_low_precision(reason="f32r matvec input"):
        nc.vector.tensor_copy(out=t1r[:], in_=t1[:])
    psb = ps_big.tile([128, 2, NT], F32, tag="sc", name="psb1")
    for m in range(EC):
        for k in range(EC):
            nc.tensor.matmul(psb[:, 0, 2 * m:2 * m + 2],
                             f1[k][:, m * 128:(m + 1) * 128],
                             t1r[:, k:k + 1].to_broadcast((128, 2)),
                             start=(k == 0), stop=(k == EC - 1))
    nc.vector.tensor_tensor(out=b1p[:],
                            in0=psb[:, 0, 0:2 * EC:2], in1=bias["f1"],
                            op=OP.add)
    # fold BN1 scale into f1 (in place, per input-channel partition)
    for k in range(EC):
        with nc.allow_low_precision(reason="f32r weights"):
            nc.vector.tensor_scalar(out=f1[k][:], in0=f1[k][:],
                                    scalar1=s1[:, k:k + 1], scalar2=None,
                                    op0=OP.mult)

    # hn = s1*ht + t1 (BN1 output, residual only) -> xT slots
    hn = [const.tile([128, TOK], F32R, tag=f"xT{k}", name=f"hn{k}")
          for k in range(EC)]
    for m in range(EC):
        for i, (off, fl) in enumerate(_bank_slices(0, TOK)):
            src = ht[m][:, off:off + fl].bitcast(F32)
            dstv = hn[m][:, off:off + fl]
            with nc.allow_low_precision(reason="f32r hn"):
                nc.gpsimd.tensor_scalar(out=dstv, in0=src,
                                        scalar1=s1[:, m:m + 1],
                                        scalar2=t1[:, m:m + 1],
                                        op0=OP.mult, op1=OP.add)

    # ---------- FFN ----------
    h1 = [act.tile([128, TOK], BF16, tag=f"h1_{k}", name=f"h1_{k}")
          for k in range(EC)]
    for m in range(EC):
        for off, fl in _bank_slices(0, TOK):
            ps = ps_big.tile([128, 2, NT], F32, tag="sc", name="psf1")
            for k in range(EC):
                nc.tensor.matmul(ps[:, 0, 0:fl], f1[k][:, m * 128:(m + 1) * 128],
                                 ht[k][:, off:off + fl],
                                 start=(k == 0), stop=(k == EC - 1))
            with nc.allow_low_precision(reason="bf16 h1"):
                nc.scalar.activation(out=h1[m][:, off:off + fl],
                                     in_=ps[:, 0, 0:fl], func=AF.Relu,
                                     bias=b1p[:, m:m + 1], scale=1.0)
    ho = [act.tile([128, TOK], F32, tag=f"ht{k}", name=f"ho{k}")
          for k in range(EC)]
    acc2 = small.tile([128, EC, 4], F32, tag="acc2", name="acc2", bufs=1)
    sq2 = small.tile([128, EC, 4], F32, tag="sq2", name="sq2", bufs=1)
    for m in range(EC):
        for i, (off, fl) in enumerate(_bank_slices(0, TOK)):
            ps = ps_big.tile([128, 2, NT], F32, tag="sc", name="psf2")
            for k in range(EC):
                nc.tensor.matmul(ps[:, 0, 0:fl], f2[k][:, m * 128:(m + 1) * 128],
                                 h1[k][:, off:off + fl],
                                 start=(k == 0), stop=(k == EC - 1))
            dst = ho[m][:, off:off + fl]
            nc.vector.scalar_tensor_tensor(
                out=dst, in0=ps[:, 0, 0:fl], scalar=bias["f2"][:, m:m + 1],
                in1=hn[m][:, off:off + fl].bitcast(F32),
                op0=OP.add, op1=OP.add,
                accum_out=acc2[:, m, i:i + 1])
            # sumsq for BN2, alternating engines
            if i % 2 == 0:
                scr = small.tile([128, NT], F32, tag="sqd", name="sqd2", bufs=2)
                nc.vector.scalar_tensor_tensor(
                    out=scr[:, 0:fl], in0=dst, scalar=1.0, in1=dst,
                    op0=OP.mult, op1=OP.mult,
                    accum_out=sq2[:, m, i:i + 1])
            else:
                scr = small.tile([128, NT], F32, tag="sqa", name="sqa2", bufs=2)
                nc.scalar.activation(
                    out=scr[:, 0:fl], in_=dst, func=AF.Square,
                    accum_out=sq2[:, m, i:i + 1])

    # ---------- BN2 + output (pipelined per 512-token slice) ----------
    s2, t2 = _bn_params(nc, small, dram, acc2, sq2, bias["bn2_g"],
                        bias["bn2_b"], epst[:], "bn2", for_timing)
    for i, (off, fl) in enumerate(_bank_slices(0, TOK)):
        for m in range(EC):
            dst = ho[m][:, off:off + fl]
            eng = (i * EC + m) % 3
            if eng == 0:
                nc.vector.tensor_scalar(out=dst, in0=dst,
                                        scalar1=s2[:, m:m + 1],
                                        scalar2=t2[:, m:m + 1],
                                        op0=OP.mult, op1=OP.add)
            elif eng == 1:
                nc.scalar.activation(out=dst, in_=dst, func=AF.Identity,
                                     bias=t2[:, m:m + 1], scale=s2[:, m:m + 1])
            else:
                nc.gpsimd.tensor_scalar(out=dst, in0=dst,
                                        scalar1=s2[:, m:m + 1],
                                        scalar2=t2[:, m:m + 1],
                                        op0=OP.mult, op1=OP.add)
            nc.sync.dma_start(out=yT_d.ap()[m * 128:(m + 1) * 128, off:off + fl],
                              in_=dst)


def _bn_params(nc, small, dram, accs, sqs, g_sb, b_sb, epst, name,
               for_timing=False):
    """Per-channel scale/shift for training-mode BN over all B*N tokens from
    raw per-(m, slice) sums: reduce -> 8-core AllReduce -> mu/var ->
    sqrt+recip. Returns (s [128, EC], t [128, EC]) tiles."""
    ccin = dram.tile([128, 2 * EC], F32, tag=f"cci_{name}", name=f"cci_{name}")
    ccout = dram.tile([128, 2 * EC], F32, tag=f"cco_{name}", name=f"cco_{name}")
    su = small.tile([128, 2, EC], F32, tag=f"su_{name}", name=f"su_{name}")
    nc.vector.tensor_reduce(out=su[:, 0, :], in_=accs[:],
                            axis=mybir.AxisListType.X, op=OP.add)
    nc.vector.tensor_reduce(out=su[:, 1, :], in_=sqs[:],
                            axis=mybir.AxisListType.X, op=OP.add)
    nc.sync.dma_start(out=ccin[:], in_=su[:].rearrange("p a b -> p (a b)"))
    if for_timing:
        # TimelineSim cannot model collectives; substitute a same-shape copy
        nc.gpsimd.dma_start(out=ccout[:], in_=ccin[:])
    else:
        nc.gpsimd.collective_compute(
            "AllReduce", OP.add, replica_groups=[list(range(N_CORES))],
            ins=[ccin.opt()], outs=[ccout.opt()])
    gsa = small.tile([128, 2, EC], F32, tag=f"gs_{name}", name=f"gs_{name}")
    nc.sync.dma_start(out=gsa[:].rearrange("p a b -> p (a b)"), in_=ccout[:])
    mu = small.tile([128, EC], F32, tag=f"mu_{name}", name=f"mu_{name}", bufs=1)
    var = small.tile([128, EC], F32, tag=f"var_{name}", name=f"var_{name}",
                     bufs=1)
    nc.vector.tensor_scalar(out=mu[:], in0=gsa[:, 0, :],
                            scalar1=1.0 / N_GLOBAL, scalar2=None, op0=OP.mult)
    nc.vector.tensor_tensor(out=var[:], in0=mu[:], in1=mu[:], op=OP.mult)
    nc.vector.scalar_tensor_tensor(out=var[:], in0=gsa[:, 1, :],
                                   scalar=1.0 / N_GLOBAL, in1=var[:],
                                   op0=OP.mult, op1=OP.subtract)
    sq = small.tile([128, EC], F32, tag=f"sq_{name}", name=f"sq_{name}", bufs=1)
    nc.scalar.activation(out=sq[:], in_=var[:], func=AF.Sqrt, bias=epst,
                         scale=1.0)
    r0 = small.tile([128, EC], F32, tag=f"r0_{name}", name=f"r0_{name}", bufs=1)
    nc.vector.reciprocal(out=r0[:], in_=sq[:])
    s_all = small.tile([128, EC], F32, tag=f"s_{name}", name=f"s_{name}",
                       bufs=1)
    sh_all = small.tile([128, EC], F32, tag=f"sh_{name}", name=f"sh_{name}",
                        bufs=1)
    nc.vector.tensor_tensor(out=s_all[:], in0=r0[:], in1=g_sb, op=OP.mult)
    nc.vector.tensor_tensor(out=sh_all[:], in0=mu[:], in1=s_all[:], op=OP.mult)
    nc.vector.tensor_tensor(out=sh_all[:], in0=b_sb, in1=sh_all[:],
                            op=OP.subtract)
    return s_all, sh_all


_NC_CACHE = None


def _get_nc():
    global _NC_CACHE
    if _NC_CACHE is None:
        _NC_CACHE = build()
    return _NC_CACHE


def make_in_maps(inputs):
    import ml_dtypes
    f8 = ml_dtypes.float8_e4m3
    shared = {}
    for n in W8_NAMES:
        w = np.asarray(inputs[f"{n}_w"], dtype=np.float32)      # [E, E]
        # w8[p, g, jt, j] = W[j, (2g+jt)*128 + p]
        w8 = np.ascontiguousarray(
            w.T.reshape(2, 2, 128, E).transpose(2, 0, 1, 3)).astype(f8)
        shared[f"{n}_w8"] = w8.reshape(128, 4 * E)
    shared["f1_wT"] = np.ascontiguousarray(
        np.asarray(inputs["f1_w"], dtype=np.float32).T)
    shared["f2_wT"] = np.ascontiguousarray(
        np.asarray(inputs["f2_w"], dtype=np.float32).T).astype(
            ml_dtypes.bfloat16)
    for n in ["rv", "tv"]:
        shared[f"{n}_brep"] = np.ascontiguousarray(
            np.broadcast_to(np.asarray(inputs[f"{n}_b"], dtype=np.float32),
                            (128, E)))
    bpk = np.empty((128, len(ALL_B) * EC), dtype=np.float32)
    for i, n in enumerate(ALL_B):
        vec = inputs[f"{n}_b"] if n in W8_NAMES + ["f1", "f2"] else inputs[n]
        bpk[:, i * EC:(i + 1) * EC] = np.asarray(vec).reshape(EC, 128).T
    shared["bpk"] = bpk
    sel2 = np.zeros((98, 128), dtype=np.float32)
    for p in range(4):
        sel2[32 * p, 0:64] = IVS
        sel2[32 * p + 1, 64:128] = IVS
    shared["sel2"] = sel2

    x = np.asarray(inputs["x"], dtype=np.float32)
    in_maps = []
    for i in range(N_CORES):
        xc = x[BL * i:BL * (i + 1)]                      # [BL, N, E]
        xT = np.ascontiguousarray(xc.transpose(2, 0, 1).reshape(E, TOK))
        x8 = np.ascontiguousarray(
            xT.reshape(EC, 128, BL, N).transpose(1, 0, 2, 3)).astype(f8)
        x8r = np.ascontiguousarray(x8[:, :, :, 0:NR])
        x8t = np.ascontiguousarray(x8[:, :, :, NR:N])
        in_maps.append({"xT": xT,
                        "x8r": x8r.reshape(128, EC * 2 * NR),
                        "x8t": x8t.reshape(128, EC * 2 * NT), **shared})
    return in_maps


def assemble_output(results):
    y = np.empty((B, N, E), dtype=np.float32)
    for i in range(N_CORES):
        yT = results[i]["yT"]                            # [E, TOK]
        y[BL * i:BL * (i + 1)] = yT.reshape(E, BL, N).transpose(1, 2, 0)
    return y


def kernel(**inputs):
    nc = _get_nc()
    in_maps = make_in_maps(inputs)
    res = run_bass_kernel_spmd(nc, in_maps, core_ids=list(range(N_CORES)))
    return assemble_output(res.results)


if __name__ == "__main__":
    nc = build()
    print("build ok")


# revision 38
# speedup vs baseline: 1.2417x; 1.0561x over previous
"""Trainium2 Bass kernel for nn_EncoderBlock (dual self-attention + BN + FFN + BN).

Sharding: data-parallel over batch (16 batches -> 2 per core on 8 cores).

v2: the attention block runs in fp8e4m3 with DoubleRow matmuls (4x PE rate on
the qkv/out projections and AV), validated to ~8e-4 end-to-end rel err.
Scores stay bf16 (64-deep contraction can't DoubleRow). exp outputs fp8
directly from ACT with the softmax /8 range shift folded into the exp bias;
the ones-column (value 1/64) inside V yields denominators from the AV matmul.
K/Q/O biases are per-partition scalars in the evacuation ops (K's provably
cancels in softmax but is applied anyway); V's bias rides the existing
psum->v8 add. The out-projection evacuation is a fused scalar_tensor_tensor
(psum + bias + residual) whose accum_out doubles as the BN1 channel sums;
sumsq comes from a square pass split across DVE/ACT. BN1's scale is folded
into the F1 weights (in-place) so the FFN starts right after the AllReduce;
BN1's shift becomes an F1 bias correction via a tiny matvec. BN2 stats use
the same accum trick; the final normalize+store is pipelined per 512-column
slice across DVE/ACT/Pool with immediate per-slice DMA.
BatchNorm batch stats use a 4KB AllReduce across the 8 cores (twice).
"""

import numpy as np
import concourse.bass as bass
import concourse.bacc as bacc
import concourse.tile as tile
from concourse import mybir
from concourse.bass_utils import run_bass_kernel_spmd

dt = mybir.dt
F32 = dt.float32
F32R = dt.float32r
BF16 = dt.bfloat16
F8 = dt.float8e4
AF = mybir.ActivationFunctionType
OP = mybir.AluOpType
DR = mybir.MatmulPerfMode.DoubleRow

N_CORES = 8
B, N, E, H, DK = 16, 1024, 512, 8, 64
NR, NT = 256, 768          # robot / task sequence lengths
BL = B // N_CORES          # local batches per core
TOK = BL * N               # local tokens per core
EC = E // 128              # channel chunks of 128
N_GLOBAL = B * N           # BN stat count
EPS = 1e-5
LN8 = 2.0794415416798357   # exp range shift: ex = exp(s)/8
IVS = 1.0 / 64.0           # ones-column value (denominator scale)

W8_NAMES = ["rq", "rk", "rv", "ro", "tq", "tk", "tv", "to"]
ALL_B = W8_NAMES + ["f1", "f2", "bn1_g", "bn1_b", "bn2_g", "bn2_b"]


def _bank_slices(base, length, maxlen=512):
    """Split [base, base+length) into pieces (<=maxlen) that never cross a
    512-col PSUM bank boundary."""
    out = []
    cur = base
    end = base + length
    while cur < end:
        nb = (cur // 512 + 1) * 512
        fl = min(end, min(nb, cur + maxlen)) - cur
        out.append((cur - base, fl))
        cur += fl
    return out


def build(for_timing=False):
    nc = bacc.Bacc("TRN2", target_bir_lowering=False, debug=False,
                   num_devices=N_CORES)

    xT_d = nc.dram_tensor("xT", [E, TOK], F32, kind="ExternalInput")
    x8r_d = nc.dram_tensor("x8r", [128, EC * 2 * NR], F8, kind="ExternalInput")
    x8t_d = nc.dram_tensor("x8t", [128, EC * 2 * NT], F8, kind="ExternalInput")
    w8_d = {n: nc.dram_tensor(f"{n}_w8", [128, 4 * E], F8, kind="ExternalInput")
            for n in W8_NAMES}
    f_d = {"f1": nc.dram_tensor("f1_wT", [E, E], F32, kind="ExternalInput"),
           "f2": nc.dram_tensor("f2_wT", [E, E], BF16, kind="ExternalInput")}
    bpk_d = nc.dram_tensor("bpk", [128, len(ALL_B) * EC], F32,
                           kind="ExternalInput")
    vrep_d = {n: nc.dram_tensor(f"{n}_brep", [128, E], F32, kind="ExternalInput")
              for n in ["rv", "tv"]}
    sel2_d = nc.dram_tensor("sel2", [98, 128], F32, kind="ExternalInput")
    yT_d = nc.dram_tensor("yT", [E, TOK], F32, kind="ExternalOutput")

    from contextlib import ExitStack
    with tile.TileContext(nc) as tc, ExitStack() as es:
        const = es.enter_context(tc.tile_pool(name="const", bufs=1))
        wpool = es.enter_context(tc.tile_pool(name="w", bufs=1))
        act = es.enter_context(tc.tile_pool(name="act", bufs=1))
        attn = es.enter_context(tc.tile_pool(name="attn", bufs=2))
        expp = es.enter_context(tc.tile_pool(name="expp", bufs=2))
        small = es.enter_context(tc.tile_pool(name="small", bufs=2))
        dram = es.enter_context(tc.tile_pool(name="dram", bufs=1, space="DRAM"))
        ps_big = es.enter_context(tc.tile_pool(name="ps_big", bufs=2, space="PSUM"))
        ps_av = es.enter_context(tc.tile_pool(name="ps_av", bufs=2, space="PSUM"))
        _body(nc, const, wpool, act, attn, expp, small, dram, ps_big, ps_av,
              xT_d, x8r_d, x8t_d, w8_d, f_d, bpk_d, vrep_d, sel2_d, yT_d,
              for_timing)
    nc.finalize()
    return nc


class _Ctx:
    pass


def _body(nc, const, wpool, act, attn, expp, small, dram, ps_big, ps_av,
          xT_d, x8r_d, x8t_d, w8_d, f_d, bpk_d, vrep_d, sel2_d, yT_d,
          for_timing):
    # ---------- constants / inputs resident in SBUF ----------
    # DMA emission order == issue order on the sync queue: the first
    # projection needs rq/rk weights + robot x8 columns; bulk (xT, task x8,
    # FFN weights) trickles in behind.
    w8 = {}
    for n in ["rq", "rk"]:
        t = wpool.tile([128, 2, 2, E], F8, tag=f"w8{n}", name=f"w8{n}")
        nc.sync.dma_start(out=t[:], in_=w8_d[n].ap())
        w8[n] = t
    x8r = const.tile([128, EC, 2, NR], F8, tag="x8r", name="x8r")
    x8t = const.tile([128, EC, 2, NT], F8, tag="x8t", name="x8t")
    nc.sync.dma_start(out=x8r[:], in_=x8r_d.ap())
    bpk = const.tile([128, len(ALL_B) * EC], F32, tag="bpk", name="bpk")
    nc.sync.dma_start(out=bpk[:], in_=bpk_d.ap())
    bias = {n: bpk[:, i * EC:(i + 1) * EC] for i, n in enumerate(ALL_B)}
    for n in ["rv", "ro"]:
        t = wpool.tile([128, 2, 2, E], F8, tag=f"w8{n}", name=f"w8{n}")
        nc.sync.dma_start(out=t[:], in_=w8_d[n].ap())
        w8[n] = t
    vrep = {}
    t = const.tile([128, E], F32, tag="vr_rv", name="vr_rv")
    nc.sync.dma_start(out=t[:], in_=vrep_d["rv"].ap())
    vrep["rv"] = t
    sel2 = const.tile([98, 128], F32R, tag="sel2", name="sel2")
    nc.sync.dma_start(out=sel2[:], in_=sel2_d.ap().bitcast(F32R))
    # Bulk loads ride the Pool SWDGE train (no HWDGE contention with the
    # latency-critical z8/rows transfers on sync), ordered by need time:
    # xT robot (outproj R0 ~18us) -> task x8/weights (T0 proj ~30us) ->
    # xT task (outproj T0 ~110us) -> FFN weights (BN1 ~170us).
    xT = [const.tile([128, TOK], F32R, tag=f"xT{k}", name=f"xT{k}")
          for k in range(EC)]
    for k in range(EC):
        nc.sync.dma_start(out=xT[k][:, 0:NR],
                          in_=xT_d.ap()[k * 128:(k + 1) * 128, 0:NR].bitcast(F32R))
        nc.sync.dma_start(out=xT[k][:, N:N + NR],
                          in_=xT_d.ap()[k * 128:(k + 1) * 128, N:N + NR].bitcast(F32R))
    x8td = x8t_d.ap().rearrange("p (k b t) -> p k b t", k=EC, b=2)
    nc.gpsimd.dma_start(out=x8t[:, :, 0, :], in_=x8td[:, :, 0, :])
    nc.gpsimd.dma_start(out=x8t[:, :, 1, :], in_=x8td[:, :, 1, :])
    for n in ["tq", "tk", "tv", "to"]:
        t = wpool.tile([128, 2, 2, E], F8, tag=f"w8{n}", name=f"w8{n}")
        nc.gpsimd.dma_start(out=t[:], in_=w8_d[n].ap())
        w8[n] = t
    t = const.tile([128, E], F32, tag="vr_tv", name="vr_tv")
    nc.gpsimd.dma_start(out=t[:], in_=vrep_d["tv"].ap())
    vrep["tv"] = t
    for k in range(EC):
        nc.sync.dma_start(out=xT[k][:, NR:N],
                          in_=xT_d.ap()[k * 128:(k + 1) * 128, NR:N].bitcast(F32R))
        nc.sync.dma_start(out=xT[k][:, N + NR:],
                          in_=xT_d.ap()[k * 128:(k + 1) * 128, N + NR:].bitcast(F32R))
    # FFN weights (f1 f32r: folded in place later; f2 bf16 to match bf16 h1)
    f1 = [wpool.tile([128, E], F32R, tag=f"f1_{k}", name=f"f1_{k}")
          for k in range(EC)]
    f2 = [wpool.tile([128, E], BF16, tag=f"f2_{k}", name=f"f2_{k}")
          for k in range(EC)]
    for k in range(EC):
        nc.gpsimd.dma_start(out=f1[k][:],
                            in_=f_d["f1"].ap()[k * 128:(k + 1) * 128, :].bitcast(F32R))
    for k in range(EC):
        nc.gpsimd.dma_start(out=f2[k][:],
                            in_=f_d["f2"].ap()[k * 128:(k + 1) * 128, :])

    # prefetch the exp ACT table set while input DMAs are in flight
    warm = const.tile([1, 1], F32, tag="warm", name="warm")
    nc.vector.memset(warm[:], 0.0)
    nc.scalar.activation(out=warm[:], in_=warm[:], func=AF.Exp, scale=1.0)
    negln8 = const.tile([128, 1], F32, tag="negln8", name="negln8")
    nc.gpsimd.memset(negln8[:], -LN8)
    epst = const.tile([128, 1], F32, tag="epst", name="epst")
    nc.gpsimd.memset(epst[:], EPS)

    # h-tilde (pre-BN1 attention output) accumulated across parts/batches,
    # with per-(m, batch-part) channel sums / sumsq for BN1
    ht = [act.tile([128, TOK], F32R, tag=f"ht{k}", name=f"ht{k}")
          for k in range(EC)]
    acc1 = small.tile([128, EC, 4], F32, tag="acc1", name="acc1", bufs=1)
    sq1 = small.tile([128, EC, 4], F32, tag="sq1", name="sq1", bufs=1)

    # ---------- attention (fine-grained interleaved emission) ----------
    # Per part, per m-chunk: project q/k chunk m, interleave the PREVIOUS
    # part's output-projection chunk m, then run head pair m (scores -> exp
    # -> AV -> evac) and its denominator broadcast + z8 scale. The exp stream
    # on ACT paces everything; PE/DVE work rides underneath it. Robot parts
    # are DVE-bound instead, so their k/zu evacuations go to ACT.
    def make_state(P):
        st = _Ctx()
        st.qT = [attn.tile([128, NT], BF16, tag=f"qT{m}", name=f"qT{m}")
                 for m in range(EC)]
        st.kT = [attn.tile([128, NT], BF16, tag=f"kT{m}", name=f"kT{m}")
                 for m in range(EC)]
        st.v8 = attn.tile([128, 6, H, DK + 2], F8, tag="v8", name="v8")
        st.z8 = attn.tile([128, 4, NT], F8, tag="z8", name="z8")
        st.rows = small.tile([98, NT], F8, tag="rows", name="rows")
        st.rinv = small.tile([98, NT], F32R, tag="rinv", name="rinv")
        if P.merged:
            st.x8p = [x8r[:, :, b, :] for b in range(2)]
        else:
            st.x8p = [x8t[:, :, P.b, :]]
        with nc.allow_low_precision(reason="fp8 attention"):
            nc.vector.memset(st.v8[:, :, :, DK:DK + 1], IVS)
            nc.vector.memset(st.v8[:, :, :, DK + 1:DK + 2], 0.0)
        return st

    def emit_qk(P, st, which, m):
        wt, o_t, bname = ((P.wq, st.qT, P.wn[0]) if which == "q"
                          else (P.wk, st.kT, P.wn[1]))
        ps = ps_big.tile([128, 2, NT], F32, tag="sc", name="psq")
        for b in range(P.nb):
            for g in range(2):
                for off, fl in _bank_slices(b * P.np, P.np, 256):
                    nc.tensor.matmul(
                        ps[:, 0, b * P.np + off:b * P.np + off + fl],
                        wt[:, g, :, m * 128:(m + 1) * 128],
                        st.x8p[b][:, 2 * g:2 * g + 2, off:off + fl],
                        start=(g == 0), stop=(g == 1), perf_mode=DR)
        with nc.allow_low_precision(reason="bf16 qk"):
            nc.vector.tensor_scalar(
                out=o_t[m][:, 0:P.w], in0=ps[:, 0, 0:P.w],
                scalar1=bias[bname][:, m:m + 1], scalar2=None, op0=OP.add)

    def emit_v(P, st, b, t):
        ps = ps_big.tile([128, 2, NT], F32, tag="sc", name="psv")
        for g in range(2):
            for j0 in (0, 256):
                nc.tensor.matmul(
                    ps[:, 0, j0:j0 + 256],
                    st.x8p[b][:, 2 * g:2 * g + 2, t * 128:(t + 1) * 128],
                    P.wv[:, g, :, j0:j0 + 256],
                    start=(g == 0), stop=(g == 1), perf_mode=DR)
        with nc.allow_low_precision(reason="fp8 v"):
            nc.vector.tensor_tensor(
                out=st.v8[:, b * P.nk + t, :, 0:DK],
                in0=ps[:, 0, 0:E].rearrange("p (h d) -> p h d", h=H),
                in1=vrep[P.wn[2]][:].rearrange("p (h d) -> p h d", h=H),
                op=OP.add)

    def emit_scores(P, st, pair, j):
        exs = []
        for g in range(P.nk // 2):
            sc = ps_big.tile([128, 2, NT], F32, tag="sc", name="sc")
            for b in range(P.nb):
                qh = st.qT[pair][j * 64:j * 64 + 64,
                                 b * P.np:(b + 1) * P.np]
                for j2 in range(2):
                    kc = 2 * g + j2
                    kh = st.kT[pair][j * 64:j * 64 + 64,
                                     b * P.np + kc * 128:
                                     b * P.np + (kc + 1) * 128]
                    for off, fl in _bank_slices(j2 * NT + b * P.np, P.np):
                        nc.tensor.matmul(
                            sc[:, j2, b * P.np + off:b * P.np + off + fl],
                            kh, qh[:, off:off + fl],
                            start=True, stop=True)
            ex = expp.tile([128, 2, NT], F8, tag="exp", name="exp", bufs=6)
            with nc.allow_low_precision(reason="fp8 exp"):
                nc.scalar.activation(
                    out=ex[:, :, 0:P.w], in_=sc[:, :, 0:P.w],
                    func=AF.Exp, scale=0.125, bias=negln8[:])
            exs.append(ex)
        return exs

    def emit_avs(P, st, pair, j, exs, zst_box):
        h = 2 * pair + j
        zu = ps_av.tile([66, NT], F32, tag="av", name="av", bufs=1)
        for g in range(P.nk // 2):
            for b in range(P.nb):
                base = b * P.np
                for off, fl in _bank_slices(base, P.np, 256):
                    nc.tensor.matmul(
                        zu[:, base + off:base + off + fl],
                        st.v8[:, b * P.nk + 2 * g:b * P.nk + 2 * g + 2, h, :],
                        exs[g][:, :, base + off:base + off + fl],
                        start=(g == 0), stop=(g == P.nk // 2 - 1),
                        perf_mode=DR)
        if j == 0:
            zst_box[0] = expp.tile([65, 2, NT], F8, tag="zst", name="zst",
                                   bufs=2)
        zst = zst_box[0]
        with nc.allow_low_precision(reason="fp8 z"):
            nc.vector.tensor_copy(out=zst[:, j, 0:P.w], in_=zu[0:65, 0:P.w])
        nc.sync.dma_start(out=st.z8[64 * j:64 * (j + 1), pair, 0:P.w],
                          in_=zst[0:64, j, 0:P.w])
        if j == 1:
            nc.sync.dma_start(out=st.rows[32 * pair:32 * pair + 2, 0:P.w],
                              in_=zst[64:65, :, 0:P.w])
            with nc.allow_low_precision(reason="f32r feeds f32r mm"):
                nc.vector.reciprocal(
                    out=st.rinv[32 * pair:32 * pair + 2, 0:P.w],
                    in_=st.rows[32 * pair:32 * pair + 2, 0:P.w])

    def emit_pair_denom(P, st, pair):
        rinv = st.rinv[32 * pair:32 * pair + 2, 0:P.w]
        rep = ps_big.tile([128, 2, NT], F32, tag="sc", name="rep")
        tp = (96, 0) if pair == 3 else None
        for off, fl in _bank_slices(0, P.w):
            nc.tensor.matmul(rep[:, 0, off:off + fl],
                             sel2[32 * pair:32 * pair + 2, :],
                             rinv[:, off:off + fl], start=True, stop=True,
                             tile_position=tp)
        with nc.allow_low_precision(reason="fp8 z scale"):
            nc.vector.tensor_tensor(out=st.z8[:, pair, 0:P.w],
                                    in0=st.z8[:, pair, 0:P.w],
                                    in1=rep[:, 0, 0:P.w], op=OP.mult)

    def emit_outproj(P, st, m):
        ps = ps_big.tile([128, 2, NT], F32, tag="sc", name="pso")
        for b in range(P.nb):
            base = b * P.np
            for g in range(2):
                for off, fl in _bank_slices(base, P.np, 256):
                    nc.tensor.matmul(
                        ps[:, 0, base + off:base + off + fl],
                        P.wo[:, g, :, m * 128:(m + 1) * 128],
                        st.z8[:, 2 * g:2 * g + 2, base + off:base + off + fl],
                        start=(g == 0), stop=(g == 1), perf_mode=DR)
        if P.merged:
            dst = ht[m][:].rearrange("p (b n) -> p b n", b=2)[:, :, 0:NR]
            res = xT[m][:].rearrange("p (b n) -> p b n", b=2)[:, :, 0:NR]
            src_ps = ps[:, 0, 0:P.w].rearrange("p (b n) -> p b n", b=2)
        else:
            dst = ht[m][:, P.tok0:P.tok0 + P.np]
            res = xT[m][:, P.tok0:P.tok0 + P.np]
            src_ps = ps[:, 0, 0:P.np]
        with nc.allow_low_precision(reason="f32r ht"):
            nc.vector.scalar_tensor_tensor(
                out=dst, in0=src_ps,
                scalar=bias[P.wn[3]][:, m:m + 1],
                in1=res,
                op0=OP.add, op1=OP.add,
                accum_out=acc1[:, m, P.bp_idx:P.bp_idx + 1])
        dv = dst.bitcast(F32)
        if P.merged:
            def scr_out(scr):
                return scr[:, 0:P.w].rearrange("p (b n) -> p b n", b=2)
        else:
            def scr_out(scr):
                return scr[:, 0:P.np]
        if not P.sq_act:
            scr = small.tile([128, NT], F32, tag="sqd", name="sqd", bufs=2)
            nc.vector.scalar_tensor_tensor(
                out=scr_out(scr), in0=dv, scalar=1.0, in1=dv,
                op0=OP.mult, op1=OP.mult,
                accum_out=sq1[:, m, P.bp_idx:P.bp_idx + 1])
        else:
            scr = small.tile([128, NT], F32, tag="sqa", name="sqa", bufs=2)
            nc.scalar.activation(
                out=scr_out(scr), in_=dv, func=AF.Square,
                accum_out=sq1[:, m, P.bp_idx:P.bp_idx + 1])

    parts = []
    for bp_idx, (part, b) in enumerate([(0, 0), (1, 0), (1, 1)]):
        P = _Ctx()
        P.part = part
        P.b = b
        P.bp_idx = bp_idx
        P.merged = (part == 0)
        P.nb = 2 if P.merged else 1
        P.wn = ["rq", "rk", "rv", "ro"] if part == 0 else ["tq", "tk", "tv", "to"]
        P.np = NR if part == 0 else NT
        P.w = P.nb * P.np
        P.nk = P.np // 128
        P.tok0 = b * N + NR
        P.wq, P.wk, P.wv, P.wo = (w8[P.wn[0]], w8[P.wn[1]], w8[P.wn[2]],
                                  w8[P.wn[3]])
        parts.append(P)

    def v_sched(P):
        bts = [(b, t) for b in range(P.nb) for t in range(P.nk)]
        out = [[], [], [], []]
        for idx, bt in enumerate(bts):
            out[min(3, idx * 4 // len(bts))].append(bt)
        return out

    sts = {0: make_state(parts[0])}
    P0 = parts[0]
    for m in range(EC):
        emit_qk(P0, sts[0], "q", m)
        emit_qk(P0, sts[0], "k", m)
        for b, t in v_sched(P0)[m]:
            emit_v(P0, sts[0], b, t)
    pend = [None]

    def flush():
        if pend[0] is not None:
            pend[0]()
            pend[0] = None

    prev = None
    for i, P in enumerate(parts):
        st = sts[i]
        P.sq_act = (i == 2)    # outproj(T1) runs in the trailing window
        nxt = parts[i + 1] if i + 1 < len(parts) else None
        if nxt is not None:
            sts[i + 1] = make_state(nxt)
        zst_box = [None]
        for pair in range(4):
            for j in (0, 1):
                exs = emit_scores(P, st, pair, j)
                flush()

                def mk(P=P, st=st, pair=pair, j=j, exs=exs, zb=zst_box,
                       prev=prev, nxt=nxt, i=i):
                    def run():
                        emit_avs(P, st, pair, j, exs, zb)
                        if j == 1:
                            # per-pair boundary work, all dependency-ready:
                            # previous pair's denominators, the previous
                            # part's output projection, next part's proj
                            if pair >= 1:
                                emit_pair_denom(P, st, pair - 1)
                            elif prev is not None:
                                emit_pair_denom(prev[0], prev[1], 3)
                            if prev is not None:
                                emit_outproj(prev[0], prev[1], pair)
                            if nxt is not None:
                                emit_qk(nxt, sts[i + 1], "q", pair)
                                emit_qk(nxt, sts[i + 1], "k", pair)
                                for b, t in v_sched(nxt)[pair]:
                                    emit_v(nxt, sts[i + 1], b, t)
                    return run
                pend[0] = mk()
        prev = (P, st)
    flush()
    emit_pair_denom(prev[0], prev[1], 3)
    for m in range(EC):
        emit_outproj(prev[0], prev[1], m)

    # all exps done: swap the ACT table set to sqrt ahead of BN1
    warm2 = const.tile([1, 1], F32, tag="warm", name="warm2")
    nc.vector.memset(warm2[:], 1.0)
    nc.scalar.activation(out=warm2[:], in_=warm2[:], func=AF.Sqrt, scale=1.0)

    # ---------- BN1 (sums -> AllReduce -> params; fold into F1) ----------
    s1, t1 = _bn_params(nc, small, dram, acc1[:, :, 0:3],
                        sq1[:, :, 0:3], bias["bn1_g"],
                        bias["bn1_b"], epst[:], "bn1", for_timing)
    # b1' = f1_b + f1_w @ t1 (tiny matvec on original f1 tiles)
    b1p = small.tile([128, EC], F32, tag="b1p", name="b1p", bufs=1)
    t1r = small.tile([128, EC], F32R, tag="t1r", name="t1r", bufs=1)
    with nc.allow_low_precision(reason="f32r matvec input"):
        nc.vector.tensor_copy(out=t1r[:], in_=t1[:])
    psb = ps_big.tile([128, 2, NT], F32, tag="sc", name="psb1")
    for m in range(EC):
        for k in range(EC):
            nc.tensor.matmul(psb[:, 0, 2 * m:2 * m + 2],
                             f1[k][:, m * 128:(m + 1) * 128],
                             t1r[:, k:k + 1].to_broadcast((128, 2)),
                             start=(k == 0), stop=(k == EC - 1))
    nc.vector.tensor_tensor(out=b1p[:],
                            in0=psb[:, 0, 0:2 * EC:2], in1=bias["f1"],
                            op=OP.add)
    # fold BN1 scale into f1 (in place, per input-channel partition)
    for k in range(EC):
        with nc.allow_low_precision(reason="f32r weights"):
            nc.vector.tensor_scalar(out=f1[k][:], in0=f1[k][:],
                                    scalar1=s1[:, k:k + 1], scalar2=None,
                                    op0=OP.mult)

    # hn = s1*ht + t1 (BN1 output, residual only) -> xT slots
    hn = [const.tile([128, TOK], F32R, tag=f"xT{k}", name=f"hn{k}")
          for k in range(EC)]
    for m in range(EC):
        for i, (off, fl) in enumerate(_bank_slices(0, TOK)):
            src = ht[m][:, off:off + fl].bitcast(F32)
            dstv = hn[m][:, off:off + fl]
            with nc.allow_low_precision(reason="f32r hn"):
                nc.gpsimd.tensor_scalar(out=dstv, in0=src,
                                        scalar1=s1[:, m:m + 1],
                                        scalar2=t1[:, m:m + 1],
                                        op0=OP.mult, op1=OP.add)

    # ---------- FFN ----------
    h1 = [act.tile([128, TOK], BF16, tag=f"h1_{k}", name=f"h1_{k}")
          for k in range(EC)]
    for m in range(EC):
        for off, fl in _bank_slices(0, TOK):
            ps = ps_big.tile([128, 2, NT], F32, tag="sc", name="psf1")
            for k in range(EC):
                nc.tensor.matmul(ps[:, 0, 0:fl], f1[k][:, m * 128:(m + 1) * 128],
                                 ht[k][:, off:off + fl],
                                 start=(k == 0), stop=(k == EC - 1))
            with nc.allow_low_precision(reason="bf16 h1"):
                nc.scalar.activation(out=h1[m][:, off:off + fl],
                                     in_=ps[:, 0, 0:fl], func=AF.Relu,
                                     bias=b1p[:, m:m + 1], scale=1.0)
    ho = [act.tile([128, TOK], F32, tag=f"ht{k}", name=f"ho{k}")
          for k in range(EC)]
    acc2 = small.tile([128, EC, 4], F32, tag="acc2", name="acc2", bufs=1)
    sq2 = small.tile([128, EC, 4], F32, tag="sq2", name="sq2", bufs=1)
    for m in range(EC):
        for i, (off, fl) in enumerate(_bank_slices(0, TOK)):
            ps = ps_big.tile([128, 2, NT], F32, tag="sc", name="psf2")
            for k in range(EC):
                nc.tensor.matmul(ps[:, 0, 0:fl], f2[k][:, m * 128:(m + 1) * 128],
                                 h1[k][:, off:off + fl],
                                 start=(k == 0), stop=(k == EC - 1))
            dst = ho[m][:, off:off + fl]
            nc.vector.scalar_tensor_tensor(
                out=dst, in0=ps[:, 0, 0:fl], scalar=bias["f2"][:, m:m + 1],
                in1=hn[m][:, off:off + fl].bitcast(F32),
                op0=OP.add, op1=OP.add,
                accum_out=acc2[:, m, i:i + 1])
            # sumsq for BN2, alternating engines
            if i % 2 == 0:
                scr = small.tile([128, NT], F32, tag="sqd", name="sqd2", bufs=2)
                nc.vector.scalar_tensor_tensor(
                    out=scr[:, 0:fl], in0=dst, scalar=1.0, in1=dst,
                    op0=OP.mult, op1=OP.mult,
                    accum_out=sq2[:, m, i:i + 1])
            else:
                scr = small.tile([128, NT], F32, tag="sqa", name="sqa2", bufs=2)
                nc.scalar.activation(
                    out=scr[:, 0:fl], in_=dst, func=AF.Square,
                    accum_out=sq2[:, m, i:i + 1])

    # ---------- BN2 + output (pipelined per 512-token slice) ----------
    s2, t2 = _bn_params(nc, small, dram, acc2[:], sq2[:],
                        bias["bn2_g"],
                        bias["bn2_b"], epst[:], "bn2", for_timing)
    for i, (off, fl) in enumerate(_bank_slices(0, TOK)):
        for m in range(EC):
            dst = ho[m][:, off:off + fl]
            eng = (i * EC + m) % 3
            if eng == 0:
                nc.vector.tensor_scalar(out=dst, in0=dst,
                                        scalar1=s2[:, m:m + 1],
                                        scalar2=t2[:, m:m + 1],
                                        op0=OP.mult, op1=OP.add)
            elif eng == 1:
                nc.scalar.activation(out=dst, in_=dst, func=AF.Identity,
                                     bias=t2[:, m:m + 1], scale=s2[:, m:m + 1])
            else:
                nc.gpsimd.tensor_scalar(out=dst, in0=dst,
                                        scalar1=s2[:, m:m + 1],
                                        scalar2=t2[:, m:m + 1],
                                        op0=OP.mult, op1=OP.add)
            nc.sync.dma_start(out=yT_d.ap()[m * 128:(m + 1) * 128, off:off + fl],
                              in_=dst)


def _bn_params(nc, small, dram, accs, sqs, g_sb, b_sb, epst, name,
               for_timing=False):
    """Per-channel scale/shift for training-mode BN over all B*N tokens from
    raw per-(m, slice) sums: reduce -> 8-core AllReduce -> mu/var ->
    sqrt+recip. Returns (s [128, EC], t [128, EC]) tiles."""
    ccin = dram.tile([128, 2 * EC], F32, tag=f"cci_{name}", name=f"cci_{name}")
    ccout = dram.tile([128, 2 * EC], F32, tag=f"cco_{name}", name=f"cco_{name}")
    su = small.tile([128, 2, EC], F32, tag=f"su_{name}", name=f"su_{name}")
    nc.vector.tensor_reduce(out=su[:, 0, :], in_=accs,
                            axis=mybir.AxisListType.X, op=OP.add)
    nc.vector.tensor_reduce(out=su[:, 1, :], in_=sqs,
                            axis=mybir.AxisListType.X, op=OP.add)
    nc.sync.dma_start(out=ccin[:], in_=su[:].rearrange("p a b -> p (a b)"))
    if for_timing:
        # TimelineSim cannot model collectives; substitute a same-shape copy
        nc.gpsimd.dma_start(out=ccout[:], in_=ccin[:])
    else:
        nc.gpsimd.collective_compute(
            "AllReduce", OP.add, replica_groups=[list(range(N_CORES))],
            ins=[ccin.opt()], outs=[ccout.opt()])
    gsa = small.tile([128, 2, EC], F32, tag=f"gs_{name}", name=f"gs_{name}")
    nc.sync.dma_start(out=gsa[:].rearrange("p a b -> p (a b)"), in_=ccout[:])
    mu = small.tile([128, EC], F32, tag=f"mu_{name}", name=f"mu_{name}", bufs=1)
    var = small.tile([128, EC], F32, tag=f"var_{name}", name=f"var_{name}",
                     bufs=1)
    nc.vector.tensor_scalar(out=mu[:], in0=gsa[:, 0, :],
                            scalar1=1.0 / N_GLOBAL, scalar2=None, op0=OP.mult)
    nc.vector.tensor_tensor(out=var[:], in0=mu[:], in1=mu[:], op=OP.mult)
    nc.vector.scalar_tensor_tensor(out=var[:], in0=gsa[:, 1, :],
                                   scalar=1.0 / N_GLOBAL, in1=var[:],
                                   op0=OP.mult, op1=OP.subtract)
    sq = small.tile([128, EC], F32, tag=f"sq_{name}", name=f"sq_{name}", bufs=1)
    nc.scalar.activation(out=sq[:], in_=var[:], func=AF.Sqrt, bias=epst,
                         scale=1.0)
    r0 = small.tile([128, EC], F32, tag=f"r0_{name}", name=f"r0_{name}", bufs=1)
    nc.vector.reciprocal(out=r0[:], in_=sq[:])
    s_all = small.tile([128, EC], F32, tag=f"s_{name}", name=f"s_{name}",
                       bufs=1)
    sh_all = small.tile([128, EC], F32, tag=f"sh_{name}", name=f"sh_{name}",
                        bufs=1)
    nc.vector.tensor_tensor(out=s_all[:], in0=r0[:], in1=g_sb, op=OP.mult)
    nc.vector.tensor_tensor(out=sh_all[:], in0=mu[:], in1=s_all[:], op=OP.mult)
    nc.vector.tensor_tensor(out=sh_all[:], in0=b_sb, in1=sh_all[:],
                            op=OP.subtract)
    return s_all, sh_all


_NC_CACHE = None


def _get_nc():
    global _NC_CACHE
    if _NC_CACHE is None:
        _NC_CACHE = build()
    return _NC_CACHE


def make_in_maps(inputs):
    import ml_dtypes
    f8 = ml_dtypes.float8_e4m3
    shared = {}
    for n in W8_NAMES:
        w = np.asarray(inputs[f"{n}_w"], dtype=np.float32)      # [E, E]
        # w8[p, g, jt, j] = W[j, (2g+jt)*128 + p]
        w8 = np.ascontiguousarray(
            w.T.reshape(2, 2, 128, E).transpose(2, 0, 1, 3)).astype(f8)
        shared[f"{n}_w8"] = w8.reshape(128, 4 * E)
    shared["f1_wT"] = np.ascontiguousarray(
        np.asarray(inputs["f1_w"], dtype=np.float32).T)
    shared["f2_wT"] = np.ascontiguousarray(
        np.asarray(inputs["f2_w"], dtype=np.float32).T).astype(
            ml_dtypes.bfloat16)
    for n in ["rv", "tv"]:
        shared[f"{n}_brep"] = np.ascontiguousarray(
            np.broadcast_to(np.asarray(inputs[f"{n}_b"], dtype=np.float32),
                            (128, E)))
    bpk = np.empty((128, len(ALL_B) * EC), dtype=np.float32)
    for i, n in enumerate(ALL_B):
        vec = inputs[f"{n}_b"] if n in W8_NAMES + ["f1", "f2"] else inputs[n]
        bpk[:, i * EC:(i + 1) * EC] = np.asarray(vec).reshape(EC, 128).T
    shared["bpk"] = bpk
    sel2 = np.zeros((98, 128), dtype=np.float32)
    for p in range(4):
        sel2[32 * p, 0:64] = IVS
        sel2[32 * p + 1, 64:128] = IVS
    shared["sel2"] = sel2

    x = np.asarray(inputs["x"], dtype=np.float32)
    in_maps = []
    for i in range(N_CORES):
        xc = x[BL * i:BL * (i + 1)]                      # [BL, N, E]
        xT = np.ascontiguousarray(xc.transpose(2, 0, 1).reshape(E, TOK))
        x8 = np.ascontiguousarray(
            xT.reshape(EC, 128, BL, N).transpose(1, 0, 2, 3)).astype(f8)
        x8r = np.ascontiguousarray(x8[:, :, :, 0:NR])
        x8t = np.ascontiguousarray(x8[:, :, :, NR:N])
        in_maps.append({"xT": xT,
                        "x8r": x8r.reshape(128, EC * 2 * NR),
                        "x8t": x8t.reshape(128, EC * 2 * NT), **shared})
    return in_maps


def assemble_output(results):
    y = np.empty((B, N, E), dtype=np.float32)
    for i in range(N_CORES):
        yT = results[i]["yT"]                            # [E, TOK]
        y[BL * i:BL * (i + 1)] = yT.reshape(E, BL, N).transpose(1, 2, 0)
    return y


def kernel(**inputs):
    nc = _get_nc()
    in_maps = make_in_maps(inputs)
    res = run_bass_kernel_spmd(nc, in_maps, core_ids=list(range(N_CORES)))
    return assemble_output(res.results)


if __name__ == "__main__":
    nc = build()
    print("build ok")


# revision 41
# speedup vs baseline: 1.2657x; 1.0194x over previous
"""Trainium2 Bass kernel for nn_EncoderBlock (dual self-attention + BN + FFN + BN).

Sharding: data-parallel over batch (16 batches -> 2 per core on 8 cores).

v2: the attention block runs in fp8e4m3 with DoubleRow matmuls (4x PE rate on
the qkv/out projections and AV), validated to ~8e-4 end-to-end rel err.
Scores stay bf16 (64-deep contraction can't DoubleRow). exp outputs fp8
directly from ACT with the softmax /8 range shift folded into the exp bias;
the ones-column (value 1/64) inside V yields denominators from the AV matmul.
K/Q/O biases are per-partition scalars in the evacuation ops (K's provably
cancels in softmax but is applied anyway); V's bias rides the existing
psum->v8 add. The out-projection evacuation is a fused scalar_tensor_tensor
(psum + bias + residual) whose accum_out doubles as the BN1 channel sums;
sumsq comes from a square pass split across DVE/ACT. BN1's scale is folded
into the F1 weights (in-place) so the FFN starts right after the AllReduce;
BN1's shift becomes an F1 bias correction via a tiny matvec. BN2 stats use
the same accum trick; the final normalize+store is pipelined per 512-column
slice across DVE/ACT/Pool with immediate per-slice DMA.
BatchNorm batch stats use a 4KB AllReduce across the 8 cores (twice).
"""

import numpy as np
import concourse.bass as bass
import concourse.bacc as bacc
import concourse.tile as tile
from concourse import mybir
from concourse.bass_utils import run_bass_kernel_spmd

dt = mybir.dt
F32 = dt.float32
F32R = dt.float32r
BF16 = dt.bfloat16
F8 = dt.float8e4
AF = mybir.ActivationFunctionType
OP = mybir.AluOpType
DR = mybir.MatmulPerfMode.DoubleRow

N_CORES = 8
B, N, E, H, DK = 16, 1024, 512, 8, 64
NR, NT = 256, 768          # robot / task sequence lengths
BL = B // N_CORES          # local batches per core
TOK = BL * N               # local tokens per core
EC = E // 128              # channel chunks of 128
N_GLOBAL = B * N           # BN stat count
EPS = 1e-5
LN8 = 2.0794415416798357   # exp range shift: ex = exp(s)/8
IVS = 1.0 / 64.0           # ones-column value (denominator scale)

W8_NAMES = ["rq", "rk", "rv", "ro", "tq", "tk", "tv", "to"]
ALL_B = W8_NAMES + ["f1", "f2", "bn1_g", "bn1_b", "bn2_g", "bn2_b"]


def _bank_slices(base, length, maxlen=512):
    """Split [base, base+length) into pieces (<=maxlen) that never cross a
    512-col PSUM bank boundary."""
    out = []
    cur = base
    end = base + length
    while cur < end:
        nb = (cur // 512 + 1) * 512
        fl = min(end, min(nb, cur + maxlen)) - cur
        out.append((cur - base, fl))
        cur += fl
    return out


def build(for_timing=False):
    nc = bacc.Bacc("TRN2", target_bir_lowering=False, debug=False,
                   num_devices=N_CORES)

    xT_d = nc.dram_tensor("xT", [E, TOK], F32, kind="ExternalInput")
    x8r_d = nc.dram_tensor("x8r", [128, EC * 2 * NR], F8, kind="ExternalInput")
    x8t_d = nc.dram_tensor("x8t", [128, EC * 2 * NT], F8, kind="ExternalInput")
    w8_d = {n: nc.dram_tensor(f"{n}_w8", [128, 4 * E], F8, kind="ExternalInput")
            for n in W8_NAMES}
    f_d = {"f1": nc.dram_tensor("f1_wT", [E, E], F32, kind="ExternalInput"),
           "f2": nc.dram_tensor("f2_w8", [128, 4 * E], F8, kind="ExternalInput")}
    bpk_d = nc.dram_tensor("bpk", [128, len(ALL_B) * EC], F32,
                           kind="ExternalInput")
    vrep_d = {n: nc.dram_tensor(f"{n}_brep", [128, E], F32, kind="ExternalInput")
              for n in ["rv", "tv"]}
    sel2_d = nc.dram_tensor("sel2", [98, 128], F32, kind="ExternalInput")
    yT_d = nc.dram_tensor("yT", [E, TOK], F32, kind="ExternalOutput")

    from contextlib import ExitStack
    with tile.TileContext(nc) as tc, ExitStack() as es:
        const = es.enter_context(tc.tile_pool(name="const", bufs=1))
        wpool = es.enter_context(tc.tile_pool(name="w", bufs=1))
        act = es.enter_context(tc.tile_pool(name="act", bufs=1))
        attn = es.enter_context(tc.tile_pool(name="attn", bufs=2))
        expp = es.enter_context(tc.tile_pool(name="expp", bufs=2))
        small = es.enter_context(tc.tile_pool(name="small", bufs=2))
        dram = es.enter_context(tc.tile_pool(name="dram", bufs=1, space="DRAM"))
        ps_big = es.enter_context(tc.tile_pool(name="ps_big", bufs=2, space="PSUM"))
        ps_av = es.enter_context(tc.tile_pool(name="ps_av", bufs=2, space="PSUM"))
        _body(nc, const, wpool, act, attn, expp, small, dram, ps_big, ps_av,
              xT_d, x8r_d, x8t_d, w8_d, f_d, bpk_d, vrep_d, sel2_d, yT_d,
              for_timing)
    nc.finalize()
    return nc


class _Ctx:
    pass


def _body(nc, const, wpool, act, attn, expp, small, dram, ps_big, ps_av,
          xT_d, x8r_d, x8t_d, w8_d, f_d, bpk_d, vrep_d, sel2_d, yT_d,
          for_timing):
    # ---------- constants / inputs resident in SBUF ----------
    # DMA emission order == issue order on the sync queue: the first
    # projection needs rq/rk weights + robot x8 columns; bulk (xT, task x8,
    # FFN weights) trickles in behind.
    w8 = {}
    for n in ["rq", "rk"]:
        t = wpool.tile([128, 2, 2, E], F8, tag=f"w8{n}", name=f"w8{n}")
        nc.sync.dma_start(out=t[:], in_=w8_d[n].ap())
        w8[n] = t
    x8r = const.tile([128, EC, 2, NR], F8, tag="x8r", name="x8r")
    x8t = const.tile([128, EC, 2, NT], F8, tag="x8t", name="x8t")
    nc.sync.dma_start(out=x8r[:], in_=x8r_d.ap())
    bpk = const.tile([128, len(ALL_B) * EC], F32, tag="bpk", name="bpk")
    nc.sync.dma_start(out=bpk[:], in_=bpk_d.ap())
    bias = {n: bpk[:, i * EC:(i + 1) * EC] for i, n in enumerate(ALL_B)}
    for n in ["rv", "ro"]:
        t = wpool.tile([128, 2, 2, E], F8, tag=f"w8{n}", name=f"w8{n}")
        nc.sync.dma_start(out=t[:], in_=w8_d[n].ap())
        w8[n] = t
    vrep = {}
    t = const.tile([128, E], F32, tag="vr_rv", name="vr_rv")
    nc.sync.dma_start(out=t[:], in_=vrep_d["rv"].ap())
    vrep["rv"] = t
    sel2 = const.tile([98, 128], F32R, tag="sel2", name="sel2")
    nc.sync.dma_start(out=sel2[:], in_=sel2_d.ap().bitcast(F32R))
    # Bulk loads ride the Pool SWDGE train (no HWDGE contention with the
    # latency-critical z8/rows transfers on sync), ordered by need time:
    # xT robot (outproj R0 ~18us) -> task x8/weights (T0 proj ~30us) ->
    # xT task (outproj T0 ~110us) -> FFN weights (BN1 ~170us).
    xT = [const.tile([128, TOK], F32R, tag=f"xT{k}", name=f"xT{k}")
          for k in range(EC)]
    for k in range(EC):
        nc.sync.dma_start(out=xT[k][:, 0:NR],
                          in_=xT_d.ap()[k * 128:(k + 1) * 128, 0:NR].bitcast(F32R))
        nc.sync.dma_start(out=xT[k][:, N:N + NR],
                          in_=xT_d.ap()[k * 128:(k + 1) * 128, N:N + NR].bitcast(F32R))
    x8td = x8t_d.ap().rearrange("p (k b t) -> p k b t", k=EC, b=2)
    nc.gpsimd.dma_start(out=x8t[:, :, 0, :], in_=x8td[:, :, 0, :])
    nc.gpsimd.dma_start(out=x8t[:, :, 1, :], in_=x8td[:, :, 1, :])
    for n in ["tq", "tk", "tv", "to"]:
        t = wpool.tile([128, 2, 2, E], F8, tag=f"w8{n}", name=f"w8{n}")
        nc.gpsimd.dma_start(out=t[:], in_=w8_d[n].ap())
        w8[n] = t
    t = const.tile([128, E], F32, tag="vr_tv", name="vr_tv")
    nc.gpsimd.dma_start(out=t[:], in_=vrep_d["tv"].ap())
    vrep["tv"] = t
    for k in range(EC):
        nc.sync.dma_start(out=xT[k][:, NR:N],
                          in_=xT_d.ap()[k * 128:(k + 1) * 128, NR:N].bitcast(F32R))
        nc.sync.dma_start(out=xT[k][:, N + NR:],
                          in_=xT_d.ap()[k * 128:(k + 1) * 128, N + NR:].bitcast(F32R))
    # FFN weights (f1 f32r: folded in place later; f2 bf16 to match bf16 h1)
    f1 = [wpool.tile([128, E], F32R, tag=f"f1_{k}", name=f"f1_{k}")
          for k in range(EC)]
    f28 = wpool.tile([128, 2, 2, E], F8, tag="f28", name="f28")
    for k in range(EC):
        nc.gpsimd.dma_start(out=f1[k][:],
                            in_=f_d["f1"].ap()[k * 128:(k + 1) * 128, :].bitcast(F32R))
    nc.gpsimd.dma_start(out=f28[:], in_=f_d["f2"].ap())

    # prefetch the exp ACT table set while input DMAs are in flight
    warm = const.tile([1, 1], F32, tag="warm", name="warm")
    nc.vector.memset(warm[:], 0.0)
    nc.scalar.activation(out=warm[:], in_=warm[:], func=AF.Exp, scale=1.0)
    negln8 = const.tile([128, 1], F32, tag="negln8", name="negln8")
    nc.gpsimd.memset(negln8[:], -LN8)
    epst = const.tile([128, 1], F32, tag="epst", name="epst")
    nc.gpsimd.memset(epst[:], EPS)

    # h-tilde (pre-BN1 attention output) accumulated across parts/batches,
    # with per-(m, batch-part) channel sums / sumsq for BN1
    ht = [act.tile([128, TOK], F32R, tag=f"ht{k}", name=f"ht{k}")
          for k in range(EC)]
    acc1 = small.tile([128, EC, 4], F32, tag="acc1", name="acc1", bufs=1)
    sq1 = small.tile([128, EC, 4], F32, tag="sq1", name="sq1", bufs=1)

    # ---------- attention (fine-grained interleaved emission) ----------
    # Per part, per m-chunk: project q/k chunk m, interleave the PREVIOUS
    # part's output-projection chunk m, then run head pair m (scores -> exp
    # -> AV -> evac) and its denominator broadcast + z8 scale. The exp stream
    # on ACT paces everything; PE/DVE work rides underneath it. Robot parts
    # are DVE-bound instead, so their k/zu evacuations go to ACT.
    def make_state(P):
        st = _Ctx()
        st.qT = [attn.tile([128, NT], BF16, tag=f"qT{m}", name=f"qT{m}")
                 for m in range(EC)]
        st.kT = [attn.tile([128, NT], BF16, tag=f"kT{m}", name=f"kT{m}")
                 for m in range(EC)]
        st.v8 = attn.tile([128, 6, H, DK + 2], F8, tag="v8", name="v8")
        st.z8 = attn.tile([128, 4, NT], F8, tag="z8", name="z8")
        st.rows = small.tile([98, NT], F8, tag="rows", name="rows")
        st.rinv = small.tile([98, NT], F32R, tag="rinv", name="rinv")
        if P.merged:
            st.x8p = [x8r[:, :, b, :] for b in range(2)]
        else:
            st.x8p = [x8t[:, :, P.b, :]]
        with nc.allow_low_precision(reason="fp8 attention"):
            nc.vector.memset(st.v8[:, :, :, DK:DK + 1], IVS)
            nc.vector.memset(st.v8[:, :, :, DK + 1:DK + 2], 0.0)
        return st

    def emit_qk(P, st, which, m):
        wt, o_t, bname = ((P.wq, st.qT, P.wn[0]) if which == "q"
                          else (P.wk, st.kT, P.wn[1]))
        ps = ps_big.tile([128, 2, NT], F32, tag="sc", name="psq")
        for b in range(P.nb):
            for off, fl in _bank_slices(b * P.np, P.np, 256):
                for g in range(2):
                    nc.tensor.matmul(
                        ps[:, 0, b * P.np + off:b * P.np + off + fl],
                        wt[:, g, :, m * 128:(m + 1) * 128],
                        st.x8p[b][:, 2 * g:2 * g + 2, off:off + fl],
                        start=(g == 0), stop=(g == 1), perf_mode=DR)
        with nc.allow_low_precision(reason="bf16 qk"):
            nc.vector.tensor_scalar(
                out=o_t[m][:, 0:P.w], in0=ps[:, 0, 0:P.w],
                scalar1=bias[bname][:, m:m + 1], scalar2=None, op0=OP.add)

    def emit_v(P, st, b, t):
        ps = ps_big.tile([128, 2, NT], F32, tag="sc", name="psv")
        for j0 in (0, 256):
            for g in range(2):
                nc.tensor.matmul(
                    ps[:, 0, j0:j0 + 256],
                    st.x8p[b][:, 2 * g:2 * g + 2, t * 128:(t + 1) * 128],
                    P.wv[:, g, :, j0:j0 + 256],
                    start=(g == 0), stop=(g == 1), perf_mode=DR)
        with nc.allow_low_precision(reason="fp8 v"):
            nc.vector.tensor_tensor(
                out=st.v8[:, b * P.nk + t, :, 0:DK],
                in0=ps[:, 0, 0:E].rearrange("p (h d) -> p h d", h=H),
                in1=vrep[P.wn[2]][:].rearrange("p (h d) -> p h d", h=H),
                op=OP.add)

    def emit_scores(P, st, pair, j):
        exs = []
        for g in range(P.nk // 2):
            sc = ps_big.tile([128, 2, NT], F32, tag="sc", name="sc")
            for b in range(P.nb):
                qh = st.qT[pair][j * 64:j * 64 + 64,
                                 b * P.np:(b + 1) * P.np]
                for j2 in range(2):
                    kc = 2 * g + j2
                    kh = st.kT[pair][j * 64:j * 64 + 64,
                                     b * P.np + kc * 128:
                                     b * P.np + (kc + 1) * 128]
                    for off, fl in _bank_slices(j2 * NT + b * P.np, P.np):
                        nc.tensor.matmul(
                            sc[:, j2, b * P.np + off:b * P.np + off + fl],
                            kh, qh[:, off:off + fl],
                            start=True, stop=True)
            ex = expp.tile([128, 2, NT], F8, tag="exp", name="exp", bufs=6)
            with nc.allow_low_precision(reason="fp8 exp"):
                nc.scalar.activation(
                    out=ex[:, :, 0:P.w], in_=sc[:, :, 0:P.w],
                    func=AF.Exp, scale=0.125, bias=negln8[:])
            exs.append(ex)
        return exs

    def emit_avs(P, st, pair, j, exs, zst_box):
        h = 2 * pair + j
        zu = ps_av.tile([66, NT], F32, tag="av", name="av", bufs=1)
        for b in range(P.nb):
            base = b * P.np
            for off, fl in _bank_slices(base, P.np, 256):
                for g in range(P.nk // 2):
                    nc.tensor.matmul(
                        zu[:, base + off:base + off + fl],
                        st.v8[:, b * P.nk + 2 * g:b * P.nk + 2 * g + 2, h, :],
                        exs[g][:, :, base + off:base + off + fl],
                        start=(g == 0), stop=(g == P.nk // 2 - 1),
                        perf_mode=DR)
        if j == 0:
            zst_box[0] = expp.tile([65, 2, NT], F8, tag="zst", name="zst",
                                   bufs=2)
        zst = zst_box[0]
        with nc.allow_low_precision(reason="fp8 z"):
            nc.vector.tensor_copy(out=zst[:, j, 0:P.w], in_=zu[0:65, 0:P.w])
        nc.sync.dma_start(out=st.z8[64 * j:64 * (j + 1), pair, 0:P.w],
                          in_=zst[0:64, j, 0:P.w])
        if j == 1:
            nc.sync.dma_start(out=st.rows[32 * pair:32 * pair + 2, 0:P.w],
                              in_=zst[64:65, :, 0:P.w])
            with nc.allow_low_precision(reason="f32r feeds f32r mm"):
                nc.vector.reciprocal(
                    out=st.rinv[32 * pair:32 * pair + 2, 0:P.w],
                    in_=st.rows[32 * pair:32 * pair + 2, 0:P.w])

    def emit_pair_denom(P, st, pair):
        rinv = st.rinv[32 * pair:32 * pair + 2, 0:P.w]
        rep = ps_big.tile([128, 2, NT], F32, tag="sc", name="rep")
        tp = (96, 0) if pair == 3 else None
        for off, fl in _bank_slices(0, P.w):
            nc.tensor.matmul(rep[:, 0, off:off + fl],
                             sel2[32 * pair:32 * pair + 2, :],
                             rinv[:, off:off + fl], start=True, stop=True,
                             tile_position=tp)
        with nc.allow_low_precision(reason="fp8 z scale"):
            nc.vector.tensor_tensor(out=st.z8[:, pair, 0:P.w],
                                    in0=st.z8[:, pair, 0:P.w],
                                    in1=rep[:, 0, 0:P.w], op=OP.mult)

    def emit_outproj(P, st, m):
        ps = ps_big.tile([128, 2, NT], F32, tag="sc", name="pso")
        for b in range(P.nb):
            base = b * P.np
            for off, fl in _bank_slices(base, P.np, 256):
                for g in range(2):
                    nc.tensor.matmul(
                        ps[:, 0, base + off:base + off + fl],
                        P.wo[:, g, :, m * 128:(m + 1) * 128],
                        st.z8[:, 2 * g:2 * g + 2, base + off:base + off + fl],
                        start=(g == 0), stop=(g == 1), perf_mode=DR)
        if P.merged:
            dst = ht[m][:].rearrange("p (b n) -> p b n", b=2)[:, :, 0:NR]
            res = xT[m][:].rearrange("p (b n) -> p b n", b=2)[:, :, 0:NR]
            src_ps = ps[:, 0, 0:P.w].rearrange("p (b n) -> p b n", b=2)
        else:
            dst = ht[m][:, P.tok0:P.tok0 + P.np]
            res = xT[m][:, P.tok0:P.tok0 + P.np]
            src_ps = ps[:, 0, 0:P.np]
        with nc.allow_low_precision(reason="f32r ht"):
            nc.vector.scalar_tensor_tensor(
                out=dst, in0=src_ps,
                scalar=bias[P.wn[3]][:, m:m + 1],
                in1=res,
                op0=OP.add, op1=OP.add,
                accum_out=acc1[:, m, P.bp_idx:P.bp_idx + 1])
        dv = dst.bitcast(F32)
        if P.merged:
            def scr_out(scr):
                return scr[:, 0:P.w].rearrange("p (b n) -> p b n", b=2)
        else:
            def scr_out(scr):
                return scr[:, 0:P.np]
        if not P.sq_act:
            scr = small.tile([128, NT], F32, tag="sqd", name="sqd", bufs=2)
            nc.vector.scalar_tensor_tensor(
                out=scr_out(scr), in0=dv, scalar=1.0, in1=dv,
                op0=OP.mult, op1=OP.mult,
                accum_out=sq1[:, m, P.bp_idx:P.bp_idx + 1])
        else:
            scr = small.tile([128, NT], F32, tag="sqa", name="sqa", bufs=2)
            nc.scalar.activation(
                out=scr_out(scr), in_=dv, func=AF.Square,
                accum_out=sq1[:, m, P.bp_idx:P.bp_idx + 1])

    parts = []
    for bp_idx, (part, b) in enumerate([(0, 0), (1, 0), (1, 1)]):
        P = _Ctx()
        P.part = part
        P.b = b
        P.bp_idx = bp_idx
        P.merged = (part == 0)
        P.nb = 2 if P.merged else 1
        P.wn = ["rq", "rk", "rv", "ro"] if part == 0 else ["tq", "tk", "tv", "to"]
        P.np = NR if part == 0 else NT
        P.w = P.nb * P.np
        P.nk = P.np // 128
        P.tok0 = b * N + NR
        P.wq, P.wk, P.wv, P.wo = (w8[P.wn[0]], w8[P.wn[1]], w8[P.wn[2]],
                                  w8[P.wn[3]])
        parts.append(P)

    def v_sched(P):
        bts = [(b, t) for b in range(P.nb) for t in range(P.nk)]
        out = [[], [], [], []]
        for idx, bt in enumerate(bts):
            out[min(3, idx * 4 // len(bts))].append(bt)
        return out

    sts = {0: make_state(parts[0])}
    P0 = parts[0]
    for m in range(EC):
        emit_qk(P0, sts[0], "q", m)
        emit_qk(P0, sts[0], "k", m)
        for b, t in v_sched(P0)[m]:
            emit_v(P0, sts[0], b, t)
    pend = [None]

    def flush():
        if pend[0] is not None:
            pend[0]()
            pend[0] = None

    prev = None
    for i, P in enumerate(parts):
        st = sts[i]
        P.sq_act = (i == 2)    # outproj(T1) runs in the trailing window
        nxt = parts[i + 1] if i + 1 < len(parts) else None
        if nxt is not None:
            sts[i + 1] = make_state(nxt)
        zst_box = [None]
        for pair in range(4):
            for j in (0, 1):
                exs = emit_scores(P, st, pair, j)
                flush()

                def mk(P=P, st=st, pair=pair, j=j, exs=exs, zb=zst_box,
                       prev=prev, nxt=nxt, i=i):
                    def run():
                        emit_avs(P, st, pair, j, exs, zb)
                        if j == 1:
                            # per-pair boundary work, all dependency-ready:
                            # previous pair's denominators, the previous
                            # part's output projection, next part's proj
                            if pair >= 1:
                                emit_pair_denom(P, st, pair - 1)
                            elif prev is not None:
                                emit_pair_denom(prev[0], prev[1], 3)
                            if prev is not None:
                                emit_outproj(prev[0], prev[1], pair)
                            if nxt is not None:
                                emit_qk(nxt, sts[i + 1], "q", pair)
                                emit_qk(nxt, sts[i + 1], "k", pair)
                                for b, t in v_sched(nxt)[pair]:
                                    emit_v(nxt, sts[i + 1], b, t)
                    return run
                pend[0] = mk()
        prev = (P, st)
    flush()
    emit_pair_denom(prev[0], prev[1], 3)
    for m in range(EC):
        emit_outproj(prev[0], prev[1], m)

    # all exps done: swap the ACT table set to sqrt ahead of BN1
    warm2 = const.tile([1, 1], F32, tag="warm", name="warm2")
    nc.vector.memset(warm2[:], 1.0)
    nc.scalar.activation(out=warm2[:], in_=warm2[:], func=AF.Sqrt, scale=1.0)

    # ---------- BN1 (sums -> AllReduce -> params; fold into F1) ----------
    s1, t1 = _bn_params(nc, small, dram, acc1[:, :, 0:3],
                        sq1[:, :, 0:3], bias["bn1_g"],
                        bias["bn1_b"], epst[:], "bn1", for_timing)
    # b1' = f1_b + f1_w @ t1 (tiny matvec on original f1 tiles)
    b1p = small.tile([128, EC], F32, tag="b1p", name="b1p", bufs=1)
    t1r = small.tile([128, EC], F32R, tag="t1r", name="t1r", bufs=1)
    with nc.allow_low_precision(reason="f32r matvec input"):
        nc.vector.tensor_copy(out=t1r[:], in_=t1[:])
    psb = ps_big.tile([128, 2, NT], F32, tag="sc", name="psb1")
    for m in range(EC):
        for k in range(EC):
            nc.tensor.matmul(psb[:, 0, 2 * m:2 * m + 2],
                             f1[k][:, m * 128:(m + 1) * 128],
                             t1r[:, k:k + 1].to_broadcast((128, 2)),
                             start=(k == 0), stop=(k == EC - 1))
    nc.vector.tensor_tensor(out=b1p[:],
                            in0=psb[:, 0, 0:2 * EC:2], in1=bias["f1"],
                            op=OP.add)
    # fold BN1 scale into f1 (in place, per input-channel partition)
    for k in range(EC):
        with nc.allow_low_precision(reason="f32r weights"):
            nc.vector.tensor_scalar(out=f1[k][:], in0=f1[k][:],
                                    scalar1=s1[:, k:k + 1], scalar2=None,
                                    op0=OP.mult)

    # hn = s1*ht + t1 (BN1 output, residual only) -> xT slots
    hn = [const.tile([128, TOK], F32R, tag=f"xT{k}", name=f"hn{k}")
          for k in range(EC)]
    for m in range(EC):
        for i, (off, fl) in enumerate(_bank_slices(0, TOK)):
            src = ht[m][:, off:off + fl].bitcast(F32)
            dstv = hn[m][:, off:off + fl]
            with nc.allow_low_precision(reason="f32r hn"):
                nc.gpsimd.tensor_scalar(out=dstv, in0=src,
                                        scalar1=s1[:, m:m + 1],
                                        scalar2=t1[:, m:m + 1],
                                        op0=OP.mult, op1=OP.add)

    # ---------- FFN ----------
    h1 = act.tile([128, EC, TOK], F8, tag="h1", name="h1")
    for m in range(EC):
        for off, fl in _bank_slices(0, TOK):
            ps = ps_big.tile([128, 2, NT], F32, tag="sc", name="psf1")
            for k in range(EC):
                nc.tensor.matmul(ps[:, 0, 0:fl], f1[k][:, m * 128:(m + 1) * 128],
                                 ht[k][:, off:off + fl],
                                 start=(k == 0), stop=(k == EC - 1))
            with nc.allow_low_precision(reason="fp8 h1"):
                nc.scalar.activation(out=h1[:, m, off:off + fl],
                                     in_=ps[:, 0, 0:fl], func=AF.Relu,
                                     bias=b1p[:, m:m + 1], scale=1.0)
    ho = [act.tile([128, TOK], F32, tag=f"ht{k}", name=f"ho{k}")
          for k in range(EC)]
    acc2 = small.tile([128, EC, 4], F32, tag="acc2", name="acc2", bufs=1)
    sq2 = small.tile([128, EC, 4], F32, tag="sq2", name="sq2", bufs=1)
    for m in range(EC):
        for i, (off, fl) in enumerate(_bank_slices(0, TOK)):
            ps = ps_big.tile([128, 2, NT], F32, tag="sc", name="psf2")
            for c in range(0, fl, 256):
                cl = min(256, fl - c)
                for g in range(2):
                    nc.tensor.matmul(
                        ps[:, 0, c:c + cl],
                        f28[:, g, :, m * 128:(m + 1) * 128],
                        h1[:, 2 * g:2 * g + 2, off + c:off + c + cl],
                        start=(g == 0), stop=(g == 1), perf_mode=DR)
            dst = ho[m][:, off:off + fl]
            nc.vector.scalar_tensor_tensor(
                out=dst, in0=ps[:, 0, 0:fl], scalar=bias["f2"][:, m:m + 1],
                in1=hn[m][:, off:off + fl].bitcast(F32),
                op0=OP.add, op1=OP.add,
                accum_out=acc2[:, m, i:i + 1])
            # sumsq for BN2, alternating engines
            if i % 2 == 0:
                scr = small.tile([128, NT], F32, tag="sqd", name="sqd2", bufs=2)
                nc.vector.scalar_tensor_tensor(
                    out=scr[:, 0:fl], in0=dst, scalar=1.0, in1=dst,
                    op0=OP.mult, op1=OP.mult,
                    accum_out=sq2[:, m, i:i + 1])
            else:
                scr = small.tile([128, NT], F32, tag="sqa", name="sqa2", bufs=2)
                nc.scalar.activation(
                    out=scr[:, 0:fl], in_=dst, func=AF.Square,
                    accum_out=sq2[:, m, i:i + 1])

    # ---------- BN2 + output (pipelined per 512-token slice) ----------
    s2, t2 = _bn_params(nc, small, dram, acc2[:], sq2[:],
                        bias["bn2_g"],
                        bias["bn2_b"], epst[:], "bn2", for_timing)
    for i, (off, fl) in enumerate(_bank_slices(0, TOK)):
        for m in range(EC):
            dst = ho[m][:, off:off + fl]
            eng = (i * EC + m) % 3
            if eng == 0:
                nc.vector.tensor_scalar(out=dst, in0=dst,
                                        scalar1=s2[:, m:m + 1],
                                        scalar2=t2[:, m:m + 1],
                                        op0=OP.mult, op1=OP.add)
            elif eng == 1:
                nc.scalar.activation(out=dst, in_=dst, func=AF.Identity,
                                     bias=t2[:, m:m + 1], scale=s2[:, m:m + 1])
            else:
                nc.gpsimd.tensor_scalar(out=dst, in0=dst,
                                        scalar1=s2[:, m:m + 1],
                                        scalar2=t2[:, m:m + 1],
                                        op0=OP.mult, op1=OP.add)
            nc.sync.dma_start(out=yT_d.ap()[m * 128:(m + 1) * 128, off:off + fl],
                              in_=dst)


def _bn_params(nc, small, dram, accs, sqs, g_sb, b_sb, epst, name,
               for_timing=False):
    """Per-channel scale/shift for training-mode BN over all B*N tokens from
    raw per-(m, slice) sums: reduce -> 8-core AllReduce -> mu/var ->
    sqrt+recip. Returns (s [128, EC], t [128, EC]) tiles."""
    ccin = dram.tile([128, 2 * EC], F32, tag=f"cci_{name}", name=f"cci_{name}")
    ccout = dram.tile([128, 2 * EC], F32, tag=f"cco_{name}", name=f"cco_{name}")
    su = small.tile([128, 2, EC], F32, tag=f"su_{name}", name=f"su_{name}")
    nc.vector.tensor_reduce(out=su[:, 0, :], in_=accs,
                            axis=mybir.AxisListType.X, op=OP.add)
    nc.vector.tensor_reduce(out=su[:, 1, :], in_=sqs,
                            axis=mybir.AxisListType.X, op=OP.add)
    nc.sync.dma_start(out=ccin[:], in_=su[:].rearrange("p a b -> p (a b)"))
    if for_timing:
        # TimelineSim cannot model collectives; substitute a same-shape copy
        nc.gpsimd.dma_start(out=ccout[:], in_=ccin[:])
    else:
        nc.gpsimd.collective_compute(
            "AllReduce", OP.add, replica_groups=[list(range(N_CORES))],
            ins=[ccin.opt()], outs=[ccout.opt()])
    gsa = small.tile([128, 2, EC], F32, tag=f"gs_{name}", name=f"gs_{name}")
    nc.sync.dma_start(out=gsa[:].rearrange("p a b -> p (a b)"), in_=ccout[:])
    mu = small.tile([128, EC], F32, tag=f"mu_{name}", name=f"mu_{name}", bufs=1)
    var = small.tile([128, EC], F32, tag=f"var_{name}", name=f"var_{name}",
                     bufs=1)
    nc.vector.tensor_scalar(out=mu[:], in0=gsa[:, 0, :],
                            scalar1=1.0 / N_GLOBAL, scalar2=None, op0=OP.mult)
    nc.vector.tensor_tensor(out=var[:], in0=mu[:], in1=mu[:], op=OP.mult)
    nc.vector.scalar_tensor_tensor(out=var[:], in0=gsa[:, 1, :],
                                   scalar=1.0 / N_GLOBAL, in1=var[:],
                                   op0=OP.mult, op1=OP.subtract)
    sq = small.tile([128, EC], F32, tag=f"sq_{name}", name=f"sq_{name}", bufs=1)
    nc.scalar.activation(out=sq[:], in_=var[:], func=AF.Sqrt, bias=epst,
                         scale=1.0)
    r0 = small.tile([128, EC], F32, tag=f"r0_{name}", name=f"r0_{name}", bufs=1)
    nc.vector.reciprocal(out=r0[:], in_=sq[:])
    s_all = small.tile([128, EC], F32, tag=f"s_{name}", name=f"s_{name}",
                       bufs=1)
    sh_all = small.tile([128, EC], F32, tag=f"sh_{name}", name=f"sh_{name}",
                        bufs=1)
    nc.vector.tensor_tensor(out=s_all[:], in0=r0[:], in1=g_sb, op=OP.mult)
    nc.vector.tensor_tensor(out=sh_all[:], in0=mu[:], in1=s_all[:], op=OP.mult)
    nc.vector.tensor_tensor(out=sh_all[:], in0=b_sb, in1=sh_all[:],
                            op=OP.subtract)
    return s_all, sh_all


_NC_CACHE = None


def _get_nc():
    global _NC_CACHE
    if _NC_CACHE is None:
        _NC_CACHE = build()
    return _NC_CACHE


def make_in_maps(inputs):
    import ml_dtypes
    f8 = ml_dtypes.float8_e4m3
    shared = {}
    for n in W8_NAMES:
        w = np.asarray(inputs[f"{n}_w"], dtype=np.float32)      # [E, E]
        # w8[p, g, jt, j] = W[j, (2g+jt)*128 + p]
        w8 = np.ascontiguousarray(
            w.T.reshape(2, 2, 128, E).transpose(2, 0, 1, 3)).astype(f8)
        shared[f"{n}_w8"] = w8.reshape(128, 4 * E)
    shared["f1_wT"] = np.ascontiguousarray(
        np.asarray(inputs["f1_w"], dtype=np.float32).T)
    w2 = np.asarray(inputs["f2_w"], dtype=np.float32)
    shared["f2_w8"] = np.ascontiguousarray(
        w2.T.reshape(2, 2, 128, E).transpose(2, 0, 1, 3)).astype(
            f8).reshape(128, 4 * E)
    for n in ["rv", "tv"]:
        shared[f"{n}_brep"] = np.ascontiguousarray(
            np.broadcast_to(np.asarray(inputs[f"{n}_b"], dtype=np.float32),
                            (128, E)))
    bpk = np.empty((128, len(ALL_B) * EC), dtype=np.float32)
    for i, n in enumerate(ALL_B):
        vec = inputs[f"{n}_b"] if n in W8_NAMES + ["f1", "f2"] else inputs[n]
        bpk[:, i * EC:(i + 1) * EC] = np.asarray(vec).reshape(EC, 128).T
    shared["bpk"] = bpk
    sel2 = np.zeros((98, 128), dtype=np.float32)
    for p in range(4):
        sel2[32 * p, 0:64] = IVS
        sel2[32 * p + 1, 64:128] = IVS
    shared["sel2"] = sel2

    x = np.asarray(inputs["x"], dtype=np.float32)
    in_maps = []
    for i in range(N_CORES):
        xc = x[BL * i:BL * (i + 1)]                      # [BL, N, E]
        xT = np.ascontiguousarray(xc.transpose(2, 0, 1).reshape(E, TOK))
        x8 = np.ascontiguousarray(
            xT.reshape(EC, 128, BL, N).transpose(1, 0, 2, 3)).astype(f8)
        x8r = np.ascontiguousarray(x8[:, :, :, 0:NR])
        x8t = np.ascontiguousarray(x8[:, :, :, NR:N])
        in_maps.append({"xT": xT,
                        "x8r": x8r.reshape(128, EC * 2 * NR),
                        "x8t": x8t.reshape(128, EC * 2 * NT), **shared})
    return in_maps


def assemble_output(results):
    y = np.empty((B, N, E), dtype=np.float32)
    for i in range(N_CORES):
        yT = results[i]["yT"]                            # [E, TOK]
        y[BL * i:BL * (i + 1)] = yT.reshape(E, BL, N).transpose(1, 2, 0)
    return y


def kernel(**inputs):
    nc = _get_nc()
    in_maps = make_in_maps(inputs)
    res = run_bass_kernel_spmd(nc, in_maps, core_ids=list(range(N_CORES)))
    return assemble_output(res.results)


if __name__ == "__main__":
    nc = build()
    print("build ok")


# revision 43
# speedup vs baseline: 1.2883x; 1.0179x over previous
"""Trainium2 Bass kernel for nn_EncoderBlock (dual self-attention + BN + FFN + BN).

Sharding: data-parallel over batch (16 batches -> 2 per core on 8 cores).

v2: the attention block runs in fp8e4m3 with DoubleRow matmuls (4x PE rate on
the qkv/out projections and AV), validated to ~8e-4 end-to-end rel err.
Scores stay bf16 (64-deep contraction can't DoubleRow). exp outputs fp8
directly from ACT with the softmax /8 range shift folded into the exp bias;
the ones-column (value 1/64) inside V yields denominators from the AV matmul.
K/Q/O biases are per-partition scalars in the evacuation ops (K's provably
cancels in softmax but is applied anyway); V's bias rides the existing
psum->v8 add. The out-projection evacuation is a fused scalar_tensor_tensor
(psum + bias + residual) whose accum_out doubles as the BN1 channel sums;
sumsq comes from a square pass split across DVE/ACT. BN1's scale is folded
into the F1 weights (in-place) so the FFN starts right after the AllReduce;
BN1's shift becomes an F1 bias correction via a tiny matvec. BN2 stats use
the same accum trick; the final normalize+store is pipelined per 512-column
slice across DVE/ACT/Pool with immediate per-slice DMA.
BatchNorm batch stats use a 4KB AllReduce across the 8 cores (twice).
"""

import numpy as np
import concourse.bass as bass
import concourse.bacc as bacc
import concourse.tile as tile
from concourse import mybir
from concourse.bass_utils import run_bass_kernel_spmd

dt = mybir.dt
F32 = dt.float32
F32R = dt.float32r
BF16 = dt.bfloat16
F8 = dt.float8e4
AF = mybir.ActivationFunctionType
OP = mybir.AluOpType
DR = mybir.MatmulPerfMode.DoubleRow

N_CORES = 8
B, N, E, H, DK = 16, 1024, 512, 8, 64
NR, NT = 256, 768          # robot / task sequence lengths
BL = B // N_CORES          # local batches per core
TOK = BL * N               # local tokens per core
EC = E // 128              # channel chunks of 128
N_GLOBAL = B * N           # BN stat count
EPS = 1e-5
LN8 = 2.0794415416798357   # exp range shift: ex = exp(s)/8
IVS = 1.0 / 64.0           # ones-column value (denominator scale)

W8_NAMES = ["rq", "rk", "rv", "ro", "tq", "tk", "tv", "to"]
ALL_B = W8_NAMES + ["f1", "f2", "bn1_g", "bn1_b", "bn2_g", "bn2_b"]


def _bank_slices(base, length, maxlen=512):
    """Split [base, base+length) into pieces (<=maxlen) that never cross a
    512-col PSUM bank boundary."""
    out = []
    cur = base
    end = base + length
    while cur < end:
        nb = (cur // 512 + 1) * 512
        fl = min(end, min(nb, cur + maxlen)) - cur
        out.append((cur - base, fl))
        cur += fl
    return out


def build(for_timing=False):
    nc = bacc.Bacc("TRN2", target_bir_lowering=False, debug=False,
                   num_devices=N_CORES)

    xT_d = nc.dram_tensor("xT", [E, TOK], F32, kind="ExternalInput")
    x8r_d = nc.dram_tensor("x8r", [128, EC * 2 * NR], F8, kind="ExternalInput")
    x8t_d = nc.dram_tensor("x8t", [128, EC * 2 * NT], F8, kind="ExternalInput")
    w8_d = {n: nc.dram_tensor(f"{n}_w8", [128, 4 * E], F8, kind="ExternalInput")
            for n in W8_NAMES}
    f_d = {"f1": nc.dram_tensor("f1_wT", [E, E], F32, kind="ExternalInput"),
           "f2": nc.dram_tensor("f2_w8", [128, 4 * E], F8, kind="ExternalInput")}
    bpk_d = nc.dram_tensor("bpk", [128, len(ALL_B) * EC], F32,
                           kind="ExternalInput")
    vrep_d = {n: nc.dram_tensor(f"{n}_brep", [128, E], F32, kind="ExternalInput")
              for n in ["rv", "tv"]}
    sel2_d = nc.dram_tensor("sel2", [98, 128], F32, kind="ExternalInput")
    yT_d = nc.dram_tensor("yT", [E, TOK], F32, kind="ExternalOutput")

    from contextlib import ExitStack
    with tile.TileContext(nc) as tc, ExitStack() as es:
        const = es.enter_context(tc.tile_pool(name="const", bufs=1))
        wpool = es.enter_context(tc.tile_pool(name="w", bufs=1))
        act = es.enter_context(tc.tile_pool(name="act", bufs=1))
        attn = es.enter_context(tc.tile_pool(name="attn", bufs=2))
        expp = es.enter_context(tc.tile_pool(name="expp", bufs=2))
        small = es.enter_context(tc.tile_pool(name="small", bufs=2))
        dram = es.enter_context(tc.tile_pool(name="dram", bufs=1, space="DRAM"))
        ps_big = es.enter_context(tc.tile_pool(name="ps_big", bufs=2, space="PSUM"))
        ps_av = es.enter_context(tc.tile_pool(name="ps_av", bufs=2, space="PSUM"))
        _body(nc, const, wpool, act, attn, expp, small, dram, ps_big, ps_av,
              xT_d, x8r_d, x8t_d, w8_d, f_d, bpk_d, vrep_d, sel2_d, yT_d,
              for_timing)
    nc.finalize()
    return nc


class _Ctx:
    pass


def _body(nc, const, wpool, act, attn, expp, small, dram, ps_big, ps_av,
          xT_d, x8r_d, x8t_d, w8_d, f_d, bpk_d, vrep_d, sel2_d, yT_d,
          for_timing):
    # ---------- constants / inputs resident in SBUF ----------
    # DMA emission order == issue order on the sync queue: the first
    # projection needs rq/rk weights + robot x8 columns; bulk (xT, task x8,
    # FFN weights) trickles in behind.
    w8 = {}
    for n in ["rq", "rk"]:
        t = wpool.tile([128, 2, 2, E], F8, tag=f"w8{n}", name=f"w8{n}")
        nc.sync.dma_start(out=t[:], in_=w8_d[n].ap())
        w8[n] = t
    x8r = const.tile([128, EC, 2, NR], F8, tag="x8r", name="x8r")
    x8t = const.tile([128, EC, 2, NT], F8, tag="x8t", name="x8t")
    nc.sync.dma_start(out=x8r[:], in_=x8r_d.ap())
    bpk = const.tile([128, len(ALL_B) * EC], F32, tag="bpk", name="bpk")
    nc.sync.dma_start(out=bpk[:], in_=bpk_d.ap())
    bias = {n: bpk[:, i * EC:(i + 1) * EC] for i, n in enumerate(ALL_B)}
    for n in ["rv", "ro"]:
        t = wpool.tile([128, 2, 2, E], F8, tag=f"w8{n}", name=f"w8{n}")
        nc.sync.dma_start(out=t[:], in_=w8_d[n].ap())
        w8[n] = t
    vrep = {}
    t = const.tile([128, E], F32, tag="vr_rv", name="vr_rv")
    nc.sync.dma_start(out=t[:], in_=vrep_d["rv"].ap())
    vrep["rv"] = t
    sel2 = const.tile([98, 128], F32R, tag="sel2", name="sel2")
    nc.sync.dma_start(out=sel2[:], in_=sel2_d.ap().bitcast(F32R))
    # Bulk loads ride the Pool SWDGE train (no HWDGE contention with the
    # latency-critical z8/rows transfers on sync), ordered by need time:
    # xT robot (outproj R0 ~18us) -> task x8/weights (T0 proj ~30us) ->
    # xT task (outproj T0 ~110us) -> FFN weights (BN1 ~170us).
    xT = [const.tile([128, TOK], F32R, tag=f"xT{k}", name=f"xT{k}")
          for k in range(EC)]
    for k in range(EC):
        nc.sync.dma_start(out=xT[k][:, 0:NR],
                          in_=xT_d.ap()[k * 128:(k + 1) * 128, 0:NR].bitcast(F32R))
        nc.sync.dma_start(out=xT[k][:, N:N + NR],
                          in_=xT_d.ap()[k * 128:(k + 1) * 128, N:N + NR].bitcast(F32R))
    x8td = x8t_d.ap().rearrange("p (k b t) -> p k b t", k=EC, b=2)
    nc.gpsimd.dma_start(out=x8t[:, :, 0, :], in_=x8td[:, :, 0, :])
    nc.gpsimd.dma_start(out=x8t[:, :, 1, :], in_=x8td[:, :, 1, :])
    for n in ["tq", "tk", "tv", "to"]:
        t = wpool.tile([128, 2, 2, E], F8, tag=f"w8{n}", name=f"w8{n}")
        nc.gpsimd.dma_start(out=t[:], in_=w8_d[n].ap())
        w8[n] = t
    t = const.tile([128, E], F32, tag="vr_tv", name="vr_tv")
    nc.gpsimd.dma_start(out=t[:], in_=vrep_d["tv"].ap())
    vrep["tv"] = t
    for k in range(EC):
        nc.sync.dma_start(out=xT[k][:, NR:N],
                          in_=xT_d.ap()[k * 128:(k + 1) * 128, NR:N].bitcast(F32R))
        nc.sync.dma_start(out=xT[k][:, N + NR:],
                          in_=xT_d.ap()[k * 128:(k + 1) * 128, N + NR:].bitcast(F32R))
    # FFN weights (f1 f32r: folded in place later; f2 bf16 to match bf16 h1)
    f1 = [wpool.tile([128, E], F32R, tag=f"f1_{k}", name=f"f1_{k}")
          for k in range(EC)]
    f28 = wpool.tile([128, 2, 2, E], F8, tag="f28", name="f28")
    for k in range(EC):
        nc.gpsimd.dma_start(out=f1[k][:],
                            in_=f_d["f1"].ap()[k * 128:(k + 1) * 128, :].bitcast(F32R))
    nc.gpsimd.dma_start(out=f28[:], in_=f_d["f2"].ap())

    # prefetch the exp ACT table set while input DMAs are in flight
    warm = const.tile([1, 1], F32, tag="warm", name="warm")
    nc.vector.memset(warm[:], 0.0)
    nc.scalar.activation(out=warm[:], in_=warm[:], func=AF.Exp, scale=1.0)
    negln8 = const.tile([128, 1], F32, tag="negln8", name="negln8")
    nc.gpsimd.memset(negln8[:], -LN8)
    epst = const.tile([128, 1], F32, tag="epst", name="epst")
    nc.gpsimd.memset(epst[:], EPS)

    # h-tilde (pre-BN1 attention output) accumulated across parts/batches,
    # with per-(m, batch-part) channel sums / sumsq for BN1
    ht = [act.tile([128, TOK], F32R, tag=f"ht{k}", name=f"ht{k}")
          for k in range(EC)]
    acc1 = small.tile([128, EC, 4], F32, tag="acc1", name="acc1", bufs=1)
    sq1 = small.tile([128, EC, 4], F32, tag="sq1", name="sq1", bufs=1)

    # ---------- attention (fine-grained interleaved emission) ----------
    # Per part, per m-chunk: project q/k chunk m, interleave the PREVIOUS
    # part's output-projection chunk m, then run head pair m (scores -> exp
    # -> AV -> evac) and its denominator broadcast + z8 scale. The exp stream
    # on ACT paces everything; PE/DVE work rides underneath it. Robot parts
    # are DVE-bound instead, so their k/zu evacuations go to ACT.
    def make_state(P):
        st = _Ctx()
        st.qT = [attn.tile([128, NT], BF16, tag=f"qT{m}", name=f"qT{m}")
                 for m in range(EC)]
        st.kT = [attn.tile([128, NT], BF16, tag=f"kT{m}", name=f"kT{m}")
                 for m in range(EC)]
        st.v8 = attn.tile([128, 6, H, DK + 2], F8, tag="v8", name="v8")
        st.z8 = attn.tile([128, 4, NT], F8, tag="z8", name="z8")
        st.rows = small.tile([98, NT], F8, tag="rows", name="rows")
        st.rinv = small.tile([98, NT], F32R, tag="rinv", name="rinv")
        if P.merged:
            st.x8p = [x8r[:, :, b, :] for b in range(2)]
        else:
            st.x8p = [x8t[:, :, P.b, :]]
        with nc.allow_low_precision(reason="fp8 attention"):
            nc.vector.memset(st.v8[:, :, :, DK:DK + 1], IVS)
            nc.vector.memset(st.v8[:, :, :, DK + 1:DK + 2], 0.0)
        return st

    def emit_qk(P, st, which, m):
        wt, o_t, bname = ((P.wq, st.qT, P.wn[0]) if which == "q"
                          else (P.wk, st.kT, P.wn[1]))
        ps = ps_big.tile([128, 2, NT], F32, tag="sc", name="psq")
        for b in range(P.nb):
            for off, fl in _bank_slices(b * P.np, P.np, 256):
                for g in range(2):
                    nc.tensor.matmul(
                        ps[:, 0, b * P.np + off:b * P.np + off + fl],
                        wt[:, g, :, m * 128:(m + 1) * 128],
                        st.x8p[b][:, 2 * g:2 * g + 2, off:off + fl],
                        start=(g == 0), stop=(g == 1), perf_mode=DR)
        with nc.allow_low_precision(reason="bf16 qk"):
            if P.merged and which == "k":
                nc.scalar.activation(
                    out=o_t[m][:, 0:P.w], in_=ps[:, 0, 0:P.w],
                    func=AF.Identity, bias=bias[bname][:, m:m + 1], scale=1.0)
            else:
                nc.vector.tensor_scalar(
                    out=o_t[m][:, 0:P.w], in0=ps[:, 0, 0:P.w],
                    scalar1=bias[bname][:, m:m + 1], scalar2=None, op0=OP.add)

    def emit_v(P, st, b, t):
        ps = ps_big.tile([128, 2, NT], F32, tag="sc", name="psv")
        for j0 in (0, 256):
            for g in range(2):
                nc.tensor.matmul(
                    ps[:, 0, j0:j0 + 256],
                    st.x8p[b][:, 2 * g:2 * g + 2, t * 128:(t + 1) * 128],
                    P.wv[:, g, :, j0:j0 + 256],
                    start=(g == 0), stop=(g == 1), perf_mode=DR)
        with nc.allow_low_precision(reason="fp8 v"):
            nc.vector.tensor_tensor(
                out=st.v8[:, b * P.nk + t, :, 0:DK],
                in0=ps[:, 0, 0:E].rearrange("p (h d) -> p h d", h=H),
                in1=vrep[P.wn[2]][:].rearrange("p (h d) -> p h d", h=H),
                op=OP.add)

    def emit_scores(P, st, pair, j):
        exs = []
        for g in range(P.nk // 2):
            sc = ps_big.tile([128, 2, NT], F32, tag="sc", name="sc")
            for b in range(P.nb):
                qh = st.qT[pair][j * 64:j * 64 + 64,
                                 b * P.np:(b + 1) * P.np]
                for j2 in range(2):
                    kc = 2 * g + j2
                    kh = st.kT[pair][j * 64:j * 64 + 64,
                                     b * P.np + kc * 128:
                                     b * P.np + (kc + 1) * 128]
                    for off, fl in _bank_slices(j2 * NT + b * P.np, P.np):
                        nc.tensor.matmul(
                            sc[:, j2, b * P.np + off:b * P.np + off + fl],
                            kh, qh[:, off:off + fl],
                            start=True, stop=True)
            ex = expp.tile([128, 2, NT], F8, tag="exp", name="exp", bufs=6)
            with nc.allow_low_precision(reason="fp8 exp"):
                nc.scalar.activation(
                    out=ex[:, :, 0:P.w], in_=sc[:, :, 0:P.w],
                    func=AF.Exp, scale=0.125, bias=negln8[:])
            exs.append(ex)
        return exs

    def emit_avs(P, st, pair, j, exs, zst_box):
        h = 2 * pair + j
        zu = ps_av.tile([66, NT], F32, tag="av", name="av", bufs=1)
        for b in range(P.nb):
            base = b * P.np
            for off, fl in _bank_slices(base, P.np, 256):
                for g in range(P.nk // 2):
                    nc.tensor.matmul(
                        zu[:, base + off:base + off + fl],
                        st.v8[:, b * P.nk + 2 * g:b * P.nk + 2 * g + 2, h, :],
                        exs[g][:, :, base + off:base + off + fl],
                        start=(g == 0), stop=(g == P.nk // 2 - 1),
                        perf_mode=DR)
        if j == 0:
            zst_box[0] = expp.tile([65, 2, NT], F8, tag="zst", name="zst",
                                   bufs=2)
        zst = zst_box[0]
        with nc.allow_low_precision(reason="fp8 z"):
            if P.merged:
                nc.scalar.activation(out=zst[:, j, 0:P.w],
                                     in_=zu[0:65, 0:P.w], func=AF.Copy)
            else:
                nc.vector.tensor_copy(out=zst[:, j, 0:P.w],
                                      in_=zu[0:65, 0:P.w])
        nc.sync.dma_start(out=st.z8[64 * j:64 * (j + 1), pair, 0:P.w],
                          in_=zst[0:64, j, 0:P.w])
        if j == 1:
            nc.sync.dma_start(out=st.rows[32 * pair:32 * pair + 2, 0:P.w],
                              in_=zst[64:65, :, 0:P.w])
            with nc.allow_low_precision(reason="f32r feeds f32r mm"):
                nc.vector.reciprocal(
                    out=st.rinv[32 * pair:32 * pair + 2, 0:P.w],
                    in_=st.rows[32 * pair:32 * pair + 2, 0:P.w])

    def emit_pair_denom(P, st, pair):
        rinv = st.rinv[32 * pair:32 * pair + 2, 0:P.w]
        rep = ps_big.tile([128, 2, NT], F32, tag="sc", name="rep")
        tp = (96, 0) if pair == 3 else None
        for off, fl in _bank_slices(0, P.w):
            nc.tensor.matmul(rep[:, 0, off:off + fl],
                             sel2[32 * pair:32 * pair + 2, :],
                             rinv[:, off:off + fl], start=True, stop=True,
                             tile_position=tp)
        with nc.allow_low_precision(reason="fp8 z scale"):
            nc.vector.tensor_tensor(out=st.z8[:, pair, 0:P.w],
                                    in0=st.z8[:, pair, 0:P.w],
                                    in1=rep[:, 0, 0:P.w], op=OP.mult)

    def emit_outproj(P, st, m):
        ps = ps_big.tile([128, 2, NT], F32, tag="sc", name="pso")
        for b in range(P.nb):
            base = b * P.np
            for off, fl in _bank_slices(base, P.np, 256):
                for g in range(2):
                    nc.tensor.matmul(
                        ps[:, 0, base + off:base + off + fl],
                        P.wo[:, g, :, m * 128:(m + 1) * 128],
                        st.z8[:, 2 * g:2 * g + 2, base + off:base + off + fl],
                        start=(g == 0), stop=(g == 1), perf_mode=DR)
        if P.merged:
            dst = ht[m][:].rearrange("p (b n) -> p b n", b=2)[:, :, 0:NR]
            res = xT[m][:].rearrange("p (b n) -> p b n", b=2)[:, :, 0:NR]
            src_ps = ps[:, 0, 0:P.w].rearrange("p (b n) -> p b n", b=2)
        else:
            dst = ht[m][:, P.tok0:P.tok0 + P.np]
            res = xT[m][:, P.tok0:P.tok0 + P.np]
            src_ps = ps[:, 0, 0:P.np]
        with nc.allow_low_precision(reason="f32r ht"):
            nc.vector.scalar_tensor_tensor(
                out=dst, in0=src_ps,
                scalar=bias[P.wn[3]][:, m:m + 1],
                in1=res,
                op0=OP.add, op1=OP.add,
                accum_out=acc1[:, m, P.bp_idx:P.bp_idx + 1])
        dv = dst.bitcast(F32)
        if P.merged:
            def scr_out(scr):
                return scr[:, 0:P.w].rearrange("p (b n) -> p b n", b=2)
        else:
            def scr_out(scr):
                return scr[:, 0:P.np]
        if not P.sq_act:
            scr = small.tile([128, NT], F32, tag="sqd", name="sqd", bufs=2)
            nc.vector.scalar_tensor_tensor(
                out=scr_out(scr), in0=dv, scalar=1.0, in1=dv,
                op0=OP.mult, op1=OP.mult,
                accum_out=sq1[:, m, P.bp_idx:P.bp_idx + 1])
        else:
            scr = small.tile([128, NT], F32, tag="sqa", name="sqa", bufs=2)
            nc.scalar.activation(
                out=scr_out(scr), in_=dv, func=AF.Square,
                accum_out=sq1[:, m, P.bp_idx:P.bp_idx + 1])

    parts = []
    for bp_idx, (part, b) in enumerate([(0, 0), (1, 0), (1, 1)]):
        P = _Ctx()
        P.part = part
        P.b = b
        P.bp_idx = bp_idx
        P.merged = (part == 0)
        P.nb = 2 if P.merged else 1
        P.wn = ["rq", "rk", "rv", "ro"] if part == 0 else ["tq", "tk", "tv", "to"]
        P.np = NR if part == 0 else NT
        P.w = P.nb * P.np
        P.nk = P.np // 128
        P.tok0 = b * N + NR
        P.wq, P.wk, P.wv, P.wo = (w8[P.wn[0]], w8[P.wn[1]], w8[P.wn[2]],
                                  w8[P.wn[3]])
        parts.append(P)

    def v_sched(P):
        bts = [(b, t) for b in range(P.nb) for t in range(P.nk)]
        out = [[], [], [], []]
        for idx, bt in enumerate(bts):
            out[min(3, idx * 4 // len(bts))].append(bt)
        return out

    sts = {0: make_state(parts[0])}
    P0 = parts[0]
    for m in range(EC):
        emit_qk(P0, sts[0], "q", m)
        emit_qk(P0, sts[0], "k", m)
        for b, t in v_sched(P0)[m]:
            emit_v(P0, sts[0], b, t)
    pend = [None]

    def flush():
        if pend[0] is not None:
            pend[0]()
            pend[0] = None

    prev = None
    for i, P in enumerate(parts):
        st = sts[i]
        P.sq_act = (i == 2)    # outproj(T1) runs in the trailing window
        nxt = parts[i + 1] if i + 1 < len(parts) else None
        if nxt is not None:
            sts[i + 1] = make_state(nxt)
        zst_box = [None]
        for pair in range(4):
            for j in (0, 1):
                exs = emit_scores(P, st, pair, j)
                flush()

                def mk(P=P, st=st, pair=pair, j=j, exs=exs, zb=zst_box,
                       prev=prev, nxt=nxt, i=i):
                    def run():
                        emit_avs(P, st, pair, j, exs, zb)
                        if j == 1:
                            # per-pair boundary work, all dependency-ready:
                            # previous pair's denominators, the previous
                            # part's output projection, next part's proj
                            if pair >= 1:
                                emit_pair_denom(P, st, pair - 1)
                            elif prev is not None:
                                emit_pair_denom(prev[0], prev[1], 3)
                            if prev is not None:
                                emit_outproj(prev[0], prev[1], pair)
                            if nxt is not None:
                                emit_qk(nxt, sts[i + 1], "q", pair)
                                emit_qk(nxt, sts[i + 1], "k", pair)
                                for b, t in v_sched(nxt)[pair]:
                                    emit_v(nxt, sts[i + 1], b, t)
                    return run
                pend[0] = mk()
        prev = (P, st)
    flush()
    emit_pair_denom(prev[0], prev[1], 3)
    for m in range(EC):
        emit_outproj(prev[0], prev[1], m)

    # all exps done: swap the ACT table set to sqrt ahead of BN1
    warm2 = const.tile([1, 1], F32, tag="warm", name="warm2")
    nc.vector.memset(warm2[:], 1.0)
    nc.scalar.activation(out=warm2[:], in_=warm2[:], func=AF.Sqrt, scale=1.0)

    # ---------- BN1 (sums -> AllReduce -> params; fold into F1) ----------
    s1, t1 = _bn_params(nc, small, dram, acc1[:, :, 0:3],
                        sq1[:, :, 0:3], bias["bn1_g"],
                        bias["bn1_b"], epst[:], "bn1", for_timing)
    # b1' = f1_b + f1_w @ t1 (tiny matvec on original f1 tiles)
    b1p = small.tile([128, EC], F32, tag="b1p", name="b1p", bufs=1)
    t1r = small.tile([128, EC], F32R, tag="t1r", name="t1r", bufs=1)
    with nc.allow_low_precision(reason="f32r matvec input"):
        nc.vector.tensor_copy(out=t1r[:], in_=t1[:])
    psb = ps_big.tile([128, 2, NT], F32, tag="sc", name="psb1")
    for m in range(EC):
        for k in range(EC):
            nc.tensor.matmul(psb[:, 0, 2 * m:2 * m + 2],
                             f1[k][:, m * 128:(m + 1) * 128],
                             t1r[:, k:k + 1].to_broadcast((128, 2)),
                             start=(k == 0), stop=(k == EC - 1))
    nc.vector.tensor_tensor(out=b1p[:],
                            in0=psb[:, 0, 0:2 * EC:2], in1=bias["f1"],
                            op=OP.add)
    # fold BN1 scale into f1 (in place, per input-channel partition)
    for k in range(EC):
        with nc.allow_low_precision(reason="f32r weights"):
            nc.vector.tensor_scalar(out=f1[k][:], in0=f1[k][:],
                                    scalar1=s1[:, k:k + 1], scalar2=None,
                                    op0=OP.mult)

    # hn = s1*ht + t1 (BN1 output, residual only) -> xT slots
    hn = [const.tile([128, TOK], F32R, tag=f"xT{k}", name=f"hn{k}")
          for k in range(EC)]
    for m in range(EC):
        for i, (off, fl) in enumerate(_bank_slices(0, TOK)):
            src = ht[m][:, off:off + fl].bitcast(F32)
            dstv = hn[m][:, off:off + fl]
            with nc.allow_low_precision(reason="f32r hn"):
                nc.gpsimd.tensor_scalar(out=dstv, in0=src,
                                        scalar1=s1[:, m:m + 1],
                                        scalar2=t1[:, m:m + 1],
                                        op0=OP.mult, op1=OP.add)

    # ---------- FFN ----------
    h1 = act.tile([128, EC, TOK], F8, tag="h1", name="h1")
    for m in range(EC):
        for off, fl in _bank_slices(0, TOK):
            ps = ps_big.tile([128, 2, NT], F32, tag="sc", name="psf1")
            for k in range(EC):
                nc.tensor.matmul(ps[:, 0, 0:fl], f1[k][:, m * 128:(m + 1) * 128],
                                 ht[k][:, off:off + fl],
                                 start=(k == 0), stop=(k == EC - 1))
            with nc.allow_low_precision(reason="fp8 h1"):
                nc.scalar.activation(out=h1[:, m, off:off + fl],
                                     in_=ps[:, 0, 0:fl], func=AF.Relu,
                                     bias=b1p[:, m:m + 1], scale=1.0)
    ho = [act.tile([128, TOK], F32, tag=f"ht{k}", name=f"ho{k}")
          for k in range(EC)]
    acc2 = small.tile([128, EC, 4], F32, tag="acc2", name="acc2", bufs=1)
    sq2 = small.tile([128, EC, 4], F32, tag="sq2", name="sq2", bufs=1)
    for m in range(EC):
        for i, (off, fl) in enumerate(_bank_slices(0, TOK)):
            ps = ps_big.tile([128, 2, NT], F32, tag="sc", name="psf2")
            for c in range(0, fl, 256):
                cl = min(256, fl - c)
                for g in range(2):
                    nc.tensor.matmul(
                        ps[:, 0, c:c + cl],
                        f28[:, g, :, m * 128:(m + 1) * 128],
                        h1[:, 2 * g:2 * g + 2, off + c:off + c + cl],
                        start=(g == 0), stop=(g == 1), perf_mode=DR)
            dst = ho[m][:, off:off + fl]
            nc.vector.scalar_tensor_tensor(
                out=dst, in0=ps[:, 0, 0:fl], scalar=bias["f2"][:, m:m + 1],
                in1=hn[m][:, off:off + fl].bitcast(F32),
                op0=OP.add, op1=OP.add,
                accum_out=acc2[:, m, i:i + 1])
            # sumsq for BN2 on ACT (DVE is saturated by the STT evacs here)
            scr = small.tile([128, NT], F32, tag="sqa", name="sqa2", bufs=2)
            nc.scalar.activation(
                out=scr[:, 0:fl], in_=dst, func=AF.Square,
                accum_out=sq2[:, m, i:i + 1])

    # ---------- BN2 + output (pipelined per 512-token slice) ----------
    s2, t2 = _bn_params(nc, small, dram, acc2[:], sq2[:],
                        bias["bn2_g"],
                        bias["bn2_b"], epst[:], "bn2", for_timing)
    for i, (off, fl) in enumerate(_bank_slices(0, TOK)):
        for m in range(EC):
            dst = ho[m][:, off:off + fl]
            eng = (i * EC + m) % 3
            if eng == 0:
                nc.vector.tensor_scalar(out=dst, in0=dst,
                                        scalar1=s2[:, m:m + 1],
                                        scalar2=t2[:, m:m + 1],
                                        op0=OP.mult, op1=OP.add)
            elif eng == 1:
                nc.scalar.activation(out=dst, in_=dst, func=AF.Identity,
                                     bias=t2[:, m:m + 1], scale=s2[:, m:m + 1])
            else:
                nc.gpsimd.tensor_scalar(out=dst, in0=dst,
                                        scalar1=s2[:, m:m + 1],
                                        scalar2=t2[:, m:m + 1],
                                        op0=OP.mult, op1=OP.add)
            nc.sync.dma_start(out=yT_d.ap()[m * 128:(m + 1) * 128, off:off + fl],
                              in_=dst)


def _bn_params(nc, small, dram, accs, sqs, g_sb, b_sb, epst, name,
               for_timing=False):
    """Per-channel scale/shift for training-mode BN over all B*N tokens from
    raw per-(m, slice) sums: reduce -> 8-core AllReduce -> mu/var ->
    sqrt+recip. Returns (s [128, EC], t [128, EC]) tiles."""
    ccin = dram.tile([128, 2 * EC], F32, tag=f"cci_{name}", name=f"cci_{name}")
    ccout = dram.tile([128, 2 * EC], F32, tag=f"cco_{name}", name=f"cco_{name}")
    su = small.tile([128, 2, EC], F32, tag=f"su_{name}", name=f"su_{name}")
    nc.vector.tensor_reduce(out=su[:, 0, :], in_=accs,
                            axis=mybir.AxisListType.X, op=OP.add)
    nc.vector.tensor_reduce(out=su[:, 1, :], in_=sqs,
                            axis=mybir.AxisListType.X, op=OP.add)
    nc.sync.dma_start(out=ccin[:], in_=su[:].rearrange("p a b -> p (a b)"))
    if for_timing:
        # TimelineSim cannot model collectives; substitute a same-shape copy
        nc.gpsimd.dma_start(out=ccout[:], in_=ccin[:])
    else:
        nc.gpsimd.collective_compute(
            "AllReduce", OP.add, replica_groups=[list(range(N_CORES))],
            ins=[ccin.opt()], outs=[ccout.opt()])
    gsa = small.tile([128, 2, EC], F32, tag=f"gs_{name}", name=f"gs_{name}")
    nc.sync.dma_start(out=gsa[:].rearrange("p a b -> p (a b)"), in_=ccout[:])
    mu = small.tile([128, EC], F32, tag=f"mu_{name}", name=f"mu_{name}", bufs=1)
    var = small.tile([128, EC], F32, tag=f"var_{name}", name=f"var_{name}",
                     bufs=1)
    nc.vector.tensor_scalar(out=mu[:], in0=gsa[:, 0, :],
                            scalar1=1.0 / N_GLOBAL, scalar2=None, op0=OP.mult)
    nc.vector.tensor_tensor(out=var[:], in0=mu[:], in1=mu[:], op=OP.mult)
    nc.vector.scalar_tensor_tensor(out=var[:], in0=gsa[:, 1, :],
                                   scalar=1.0 / N_GLOBAL, in1=var[:],
                                   op0=OP.mult, op1=OP.subtract)
    sq = small.tile([128, EC], F32, tag=f"sq_{name}", name=f"sq_{name}", bufs=1)
    nc.scalar.activation(out=sq[:], in_=var[:], func=AF.Sqrt, bias=epst,
                         scale=1.0)
    r0 = small.tile([128, EC], F32, tag=f"r0_{name}", name=f"r0_{name}", bufs=1)
    nc.vector.reciprocal(out=r0[:], in_=sq[:])
    s_all = small.tile([128, EC], F32, tag=f"s_{name}", name=f"s_{name}",
                       bufs=1)
    sh_all = small.tile([128, EC], F32, tag=f"sh_{name}", name=f"sh_{name}",
                        bufs=1)
    nc.vector.tensor_tensor(out=s_all[:], in0=r0[:], in1=g_sb, op=OP.mult)
    nc.vector.tensor_tensor(out=sh_all[:], in0=mu[:], in1=s_all[:], op=OP.mult)
    nc.vector.tensor_tensor(out=sh_all[:], in0=b_sb, in1=sh_all[:],
                            op=OP.subtract)
    return s_all, sh_all


_NC_CACHE = None


def _get_nc():
    global _NC_CACHE
    if _NC_CACHE is None:
        _NC_CACHE = build()
    return _NC_CACHE


def make_in_maps(inputs):
    import ml_dtypes
    f8 = ml_dtypes.float8_e4m3
    shared = {}
    for n in W8_NAMES:
        w = np.asarray(inputs[f"{n}_w"], dtype=np.float32)      # [E, E]
        # w8[p, g, jt, j] = W[j, (2g+jt)*128 + p]
        w8 = np.ascontiguousarray(
            w.T.reshape(2, 2, 128, E).transpose(2, 0, 1, 3)).astype(f8)
        shared[f"{n}_w8"] = w8.reshape(128, 4 * E)
    shared["f1_wT"] = np.ascontiguousarray(
        np.asarray(inputs["f1_w"], dtype=np.float32).T)
    w2 = np.asarray(inputs["f2_w"], dtype=np.float32)
    shared["f2_w8"] = np.ascontiguousarray(
        w2.T.reshape(2, 2, 128, E).transpose(2, 0, 1, 3)).astype(
            f8).reshape(128, 4 * E)
    for n in ["rv", "tv"]:
        shared[f"{n}_brep"] = np.ascontiguousarray(
            np.broadcast_to(np.asarray(inputs[f"{n}_b"], dtype=np.float32),
                            (128, E)))
    bpk = np.empty((128, len(ALL_B) * EC), dtype=np.float32)
    for i, n in enumerate(ALL_B):
        vec = inputs[f"{n}_b"] if n in W8_NAMES + ["f1", "f2"] else inputs[n]
        bpk[:, i * EC:(i + 1) * EC] = np.asarray(vec).reshape(EC, 128).T
    shared["bpk"] = bpk
    sel2 = np.zeros((98, 128), dtype=np.float32)
    for p in range(4):
        sel2[32 * p, 0:64] = IVS
        sel2[32 * p + 1, 64:128] = IVS
    shared["sel2"] = sel2

    x = np.asarray(inputs["x"], dtype=np.float32)
    in_maps = []
    for i in range(N_CORES):
        xc = x[BL * i:BL * (i + 1)]                      # [BL, N, E]
        xT = np.ascontiguousarray(xc.transpose(2, 0, 1).reshape(E, TOK))
        x8 = np.ascontiguousarray(
            xT.reshape(EC, 128, BL, N).transpose(1, 0, 2, 3)).astype(f8)
        x8r = np.ascontiguousarray(x8[:, :, :, 0:NR])
        x8t = np.ascontiguousarray(x8[:, :, :, NR:N])
        in_maps.append({"xT": xT,
                        "x8r": x8r.reshape(128, EC * 2 * NR),
                        "x8t": x8t.reshape(128, EC * 2 * NT), **shared})
    return in_maps


def assemble_output(results):
    y = np.empty((B, N, E), dtype=np.float32)
    for i in range(N_CORES):
        yT = results[i]["yT"]                            # [E, TOK]
        y[BL * i:BL * (i + 1)] = yT.reshape(E, BL, N).transpose(1, 2, 0)
    return y


def kernel(**inputs):
    nc = _get_nc()
    in_maps = make_in_maps(inputs)
    res = run_bass_kernel_spmd(nc, in_maps, core_ids=list(range(N_CORES)))
    return assemble_output(res.results)


if __name__ == "__main__":
    nc = build()
    print("build ok")


# revision 47
# speedup vs baseline: 1.2903x; 1.0015x over previous
"""Trainium2 Bass kernel for nn_EncoderBlock (dual self-attention + BN + FFN + BN).

Sharding: data-parallel over batch (16 batches -> 2 per core on 8 cores).

v2: the attention block runs in fp8e4m3 with DoubleRow matmuls (4x PE rate on
the qkv/out projections and AV), validated to ~8e-4 end-to-end rel err.
Scores stay bf16 (64-deep contraction can't DoubleRow). exp outputs fp8
directly from ACT with the softmax /8 range shift folded into the exp bias;
the ones-column (value 1/64) inside V yields denominators from the AV matmul.
K/Q/O biases are per-partition scalars in the evacuation ops (K's provably
cancels in softmax but is applied anyway); V's bias rides the existing
psum->v8 add. The out-projection evacuation is a fused scalar_tensor_tensor
(psum + bias + residual) whose accum_out doubles as the BN1 channel sums;
sumsq comes from a square pass split across DVE/ACT. BN1's scale is folded
into the F1 weights (in-place) so the FFN starts right after the AllReduce;
BN1's shift becomes an F1 bias correction via a tiny matvec. BN2 stats use
the same accum trick; the final normalize+store is pipelined per 512-column
slice across DVE/ACT/Pool with immediate per-slice DMA.
BatchNorm batch stats use a 4KB AllReduce across the 8 cores (twice).
"""

import numpy as np
import concourse.bass as bass
import concourse.bacc as bacc
import concourse.tile as tile
from concourse import mybir
from concourse.bass_utils import run_bass_kernel_spmd

dt = mybir.dt
F32 = dt.float32
F32R = dt.float32r
BF16 = dt.bfloat16
F8 = dt.float8e4
AF = mybir.ActivationFunctionType
OP = mybir.AluOpType
DR = mybir.MatmulPerfMode.DoubleRow

N_CORES = 8
B, N, E, H, DK = 16, 1024, 512, 8, 64
NR, NT = 256, 768          # robot / task sequence lengths
BL = B // N_CORES          # local batches per core
TOK = BL * N               # local tokens per core
EC = E // 128              # channel chunks of 128
N_GLOBAL = B * N           # BN stat count
EPS = 1e-5
LN8 = 2.0794415416798357   # exp range shift: ex = exp(s)/8
IVS = 1.0 / 64.0           # ones-column value (denominator scale)

W8_NAMES = ["rq", "rk", "rv", "ro", "tq", "tk", "tv", "to"]
ALL_B = W8_NAMES + ["f1", "f2", "bn1_g", "bn1_b", "bn2_g", "bn2_b"]


def _bank_slices(base, length, maxlen=512):
    """Split [base, base+length) into pieces (<=maxlen) that never cross a
    512-col PSUM bank boundary."""
    out = []
    cur = base
    end = base + length
    while cur < end:
        nb = (cur // 512 + 1) * 512
        fl = min(end, min(nb, cur + maxlen)) - cur
        out.append((cur - base, fl))
        cur += fl
    return out


def build(for_timing=False):
    nc = bacc.Bacc("TRN2", target_bir_lowering=False, debug=False,
                   num_devices=N_CORES)

    xT_d = nc.dram_tensor("xT", [E, TOK], F32, kind="ExternalInput")
    x8r_d = nc.dram_tensor("x8r", [128, EC * 2 * NR], F8, kind="ExternalInput")
    x8t_d = nc.dram_tensor("x8t", [128, EC * 2 * NT], F8, kind="ExternalInput")
    w8_d = {n: nc.dram_tensor(f"{n}_w8", [128, 4 * E], F8, kind="ExternalInput")
            for n in W8_NAMES}
    f_d = {"f1": nc.dram_tensor("f1_wT", [E, E], F32, kind="ExternalInput"),
           "f2": nc.dram_tensor("f2_w8", [128, 4 * E], F8, kind="ExternalInput")}
    bpk_d = nc.dram_tensor("bpk", [128, len(ALL_B) * EC], F32,
                           kind="ExternalInput")
    vrep_d = {n: nc.dram_tensor(f"{n}_brep", [128, E], F32, kind="ExternalInput")
              for n in ["rv", "tv"]}
    sel2_d = nc.dram_tensor("sel2", [98, 128], F32, kind="ExternalInput")
    ones1_d = nc.dram_tensor("ones1", [1, 128], F32, kind="ExternalInput")
    yT_d = nc.dram_tensor("yT", [E, TOK], F32, kind="ExternalOutput")

    from contextlib import ExitStack
    with tile.TileContext(nc) as tc, ExitStack() as es:
        const = es.enter_context(tc.tile_pool(name="const", bufs=1))
        wpool = es.enter_context(tc.tile_pool(name="w", bufs=1))
        act = es.enter_context(tc.tile_pool(name="act", bufs=1))
        attn = es.enter_context(tc.tile_pool(name="attn", bufs=2))
        expp = es.enter_context(tc.tile_pool(name="expp", bufs=2))
        small = es.enter_context(tc.tile_pool(name="small", bufs=2))
        dram = es.enter_context(tc.tile_pool(name="dram", bufs=1, space="DRAM"))
        ps_big = es.enter_context(tc.tile_pool(name="ps_big", bufs=2, space="PSUM"))
        ps_av = es.enter_context(tc.tile_pool(name="ps_av", bufs=2, space="PSUM"))
        _body(nc, const, wpool, act, attn, expp, small, dram, ps_big, ps_av,
              xT_d, x8r_d, x8t_d, w8_d, f_d, bpk_d, vrep_d, sel2_d, ones1_d,
              yT_d, for_timing)
    nc.finalize()
    return nc


class _Ctx:
    pass


def _body(nc, const, wpool, act, attn, expp, small, dram, ps_big, ps_av,
          xT_d, x8r_d, x8t_d, w8_d, f_d, bpk_d, vrep_d, sel2_d, ones1_d,
          yT_d, for_timing):
    # ---------- constants / inputs resident in SBUF ----------
    # DMA emission order == issue order on the sync queue: the first
    # projection needs rq/rk weights + robot x8 columns; bulk (xT, task x8,
    # FFN weights) trickles in behind.
    w8 = {}
    for n in ["rq", "rk"]:
        t = wpool.tile([128, 2, 2, E], F8, tag=f"w8{n}", name=f"w8{n}")
        nc.sync.dma_start(out=t[:], in_=w8_d[n].ap())
        w8[n] = t
    x8r = const.tile([128, EC, 2, NR], F8, tag="x8r", name="x8r")
    x8t = const.tile([128, EC, 2, NT], F8, tag="x8t", name="x8t")
    nc.sync.dma_start(out=x8r[:], in_=x8r_d.ap())
    bpk = const.tile([128, len(ALL_B) * EC], F32, tag="bpk", name="bpk")
    nc.sync.dma_start(out=bpk[:], in_=bpk_d.ap())
    bias = {n: bpk[:, i * EC:(i + 1) * EC] for i, n in enumerate(ALL_B)}
    for n in ["rv", "ro"]:
        t = wpool.tile([128, 2, 2, E], F8, tag=f"w8{n}", name=f"w8{n}")
        nc.sync.dma_start(out=t[:], in_=w8_d[n].ap())
        w8[n] = t
    vrep = {}
    t = const.tile([128, E], F32R, tag="vr_rv", name="vr_rv")
    nc.sync.dma_start(out=t[:], in_=vrep_d["rv"].ap().bitcast(F32R))
    vrep["rv"] = t
    sel2 = const.tile([98, 128], F32R, tag="sel2", name="sel2")
    nc.sync.dma_start(out=sel2[:], in_=sel2_d.ap().bitcast(F32R))
    ones1 = const.tile([1, 128], F32R, tag="ones1", name="ones1")
    nc.sync.dma_start(out=ones1[:], in_=ones1_d.ap().bitcast(F32R))
    # Bulk loads ride the Pool SWDGE train (no HWDGE contention with the
    # latency-critical z8/rows transfers on sync), ordered by need time:
    # xT robot (outproj R0 ~18us) -> task x8/weights (T0 proj ~30us) ->
    # xT task (outproj T0 ~110us) -> FFN weights (BN1 ~170us).
    xT = [const.tile([128, TOK], F32R, tag=f"xT{k}", name=f"xT{k}")
          for k in range(EC)]
    for k in range(EC):
        nc.sync.dma_start(out=xT[k][:, 0:NR],
                          in_=xT_d.ap()[k * 128:(k + 1) * 128, 0:NR].bitcast(F32R))
        nc.sync.dma_start(out=xT[k][:, N:N + NR],
                          in_=xT_d.ap()[k * 128:(k + 1) * 128, N:N + NR].bitcast(F32R))
    x8td = x8t_d.ap().rearrange("p (k b t) -> p k b t", k=EC, b=2)
    nc.gpsimd.dma_start(out=x8t[:, :, 0, :], in_=x8td[:, :, 0, :])
    nc.gpsimd.dma_start(out=x8t[:, :, 1, :], in_=x8td[:, :, 1, :])
    for n in ["tq", "tk", "tv", "to"]:
        t = wpool.tile([128, 2, 2, E], F8, tag=f"w8{n}", name=f"w8{n}")
        nc.gpsimd.dma_start(out=t[:], in_=w8_d[n].ap())
        w8[n] = t
    t = const.tile([128, E], F32R, tag="vr_tv", name="vr_tv")
    nc.gpsimd.dma_start(out=t[:], in_=vrep_d["tv"].ap().bitcast(F32R))
    vrep["tv"] = t
    for k in range(EC):
        nc.sync.dma_start(out=xT[k][:, NR:N],
                          in_=xT_d.ap()[k * 128:(k + 1) * 128, NR:N].bitcast(F32R))
        nc.sync.dma_start(out=xT[k][:, N + NR:],
                          in_=xT_d.ap()[k * 128:(k + 1) * 128, N + NR:].bitcast(F32R))
    # FFN weights (f1 f32r: folded in place later; f2 bf16 to match bf16 h1)
    f1 = [wpool.tile([128, E], F32R, tag=f"f1_{k}", name=f"f1_{k}")
          for k in range(EC)]
    f28 = wpool.tile([128, 2, 2, E], F8, tag="f28", name="f28")
    for k in range(EC):
        nc.gpsimd.dma_start(out=f1[k][:],
                            in_=f_d["f1"].ap()[k * 128:(k + 1) * 128, :].bitcast(F32R))
    nc.gpsimd.dma_start(out=f28[:], in_=f_d["f2"].ap())

    # prefetch the exp ACT table set while input DMAs are in flight
    warm = const.tile([1, 1], F32, tag="warm", name="warm")
    nc.vector.memset(warm[:], 0.0)
    nc.scalar.activation(out=warm[:], in_=warm[:], func=AF.Exp, scale=1.0)
    negln8 = const.tile([128, 1], F32, tag="negln8", name="negln8")
    nc.gpsimd.memset(negln8[:], -LN8)
    epst = const.tile([128, 1], F32, tag="epst", name="epst")
    nc.gpsimd.memset(epst[:], EPS)


    # h-tilde (pre-BN1 attention output) accumulated across parts/batches,
    # with per-(m, batch-part) channel sums / sumsq for BN1
    ht = [act.tile([128, TOK], F32R, tag=f"ht{k}", name=f"ht{k}")
          for k in range(EC)]
    acc1 = small.tile([128, EC, 4], F32, tag="acc1", name="acc1", bufs=1)
    sq1 = small.tile([128, EC, 4], F32, tag="sq1", name="sq1", bufs=1)

    # ---------- attention (fine-grained interleaved emission) ----------
    # Per part, per m-chunk: project q/k chunk m, interleave the PREVIOUS
    # part's output-projection chunk m, then run head pair m (scores -> exp
    # -> AV -> evac) and its denominator broadcast + z8 scale. The exp stream
    # on ACT paces everything; PE/DVE work rides underneath it. Robot parts
    # are DVE-bound instead, so their k/zu evacuations go to ACT.
    def make_state(P):
        st = _Ctx()
        st.qT = [attn.tile([128, NT], BF16, tag=f"qT{m}", name=f"qT{m}")
                 for m in range(EC)]
        st.kT = [attn.tile([128, NT], BF16, tag=f"kT{m}", name=f"kT{m}")
                 for m in range(EC)]
        st.v8 = attn.tile([128, 6, H, DK + 2], F8, tag="v8", name="v8")
        st.z8 = attn.tile([128, 4, NT], F8, tag="z8", name="z8")
        st.rows = small.tile([98, NT], F8, tag="rows", name="rows")
        st.rinv = small.tile([98, NT], F32R, tag="rinv", name="rinv")
        if P.merged:
            st.x8p = [x8r[:, :, b, :] for b in range(2)]
        else:
            st.x8p = [x8t[:, :, P.b, :]]
        with nc.allow_low_precision(reason="fp8 attention"):
            nc.vector.memset(st.v8[:, :, :, DK:DK + 1], IVS)
            nc.vector.memset(st.v8[:, :, :, DK + 1:DK + 2], 0.0)
        return st

    def emit_qk(P, st, which, m):
        wt, o_t, bname = ((P.wq, st.qT, P.wn[0]) if which == "q"
                          else (P.wk, st.kT, P.wn[1]))
        ps = ps_big.tile([128, 2, NT], F32, tag="sc", name="psq")
        for b in range(P.nb):
            for off, fl in _bank_slices(b * P.np, P.np, 256):
                for g in range(2):
                    nc.tensor.matmul(
                        ps[:, 0, b * P.np + off:b * P.np + off + fl],
                        wt[:, g, :, m * 128:(m + 1) * 128],
                        st.x8p[b][:, 2 * g:2 * g + 2, off:off + fl],
                        start=(g == 0), stop=(g == 1), perf_mode=DR)
        with nc.allow_low_precision(reason="bf16 qk"):
            if P.merged and which == "k":
                nc.scalar.activation(
                    out=o_t[m][:, 0:P.w], in_=ps[:, 0, 0:P.w],
                    func=AF.Identity, bias=bias[bname][:, m:m + 1], scale=1.0)
            else:
                nc.vector.tensor_scalar(
                    out=o_t[m][:, 0:P.w], in0=ps[:, 0, 0:P.w],
                    scalar1=bias[bname][:, m:m + 1], scalar2=None, op0=OP.add)

    def emit_v(P, st, b, t):
        ps = ps_big.tile([128, 2, NT], F32, tag="sc", name="psv")
        for j0 in (0, 256):
            for g in range(2):
                nc.tensor.matmul(
                    ps[:, 0, j0:j0 + 256],
                    st.x8p[b][:, 2 * g:2 * g + 2, t * 128:(t + 1) * 128],
                    P.wv[:, g, :, j0:j0 + 256],
                    start=(g == 0), stop=(g == 1), perf_mode=DR)
            # V bias via a K=1 ones-row matmul so the evac is a pure copy
            # that can ride the otherwise-idle ACT slots
            nc.tensor.matmul(
                ps[:, 0, j0:j0 + 256], ones1[:],
                vrep[P.wn[2]][0:1, j0:j0 + 256],
                start=False, stop=False, skip_group_check=True)
        with nc.allow_low_precision(reason="fp8 v"):
            nc.scalar.activation(
                out=st.v8[:, b * P.nk + t, :, 0:DK],
                in_=ps[:, 0, 0:E].rearrange("p (h d) -> p h d", h=H),
                func=AF.Copy)

    def emit_scores(P, st, pair, j):
        exs = []
        for g in range(P.nk // 2):
            sc = ps_big.tile([128, 2, NT], F32, tag="sc", name="sc")
            for b in range(P.nb):
                qh = st.qT[pair][j * 64:j * 64 + 64,
                                 b * P.np:(b + 1) * P.np]
                for j2 in range(2):
                    kc = 2 * g + j2
                    kh = st.kT[pair][j * 64:j * 64 + 64,
                                     b * P.np + kc * 128:
                                     b * P.np + (kc + 1) * 128]
                    for off, fl in _bank_slices(j2 * NT + b * P.np, P.np):
                        nc.tensor.matmul(
                            sc[:, j2, b * P.np + off:b * P.np + off + fl],
                            kh, qh[:, off:off + fl],
                            start=True, stop=True)
            ex = expp.tile([128, 2, NT], F8, tag="exp", name="exp", bufs=9)
            with nc.allow_low_precision(reason="fp8 exp"):
                nc.scalar.activation(
                    out=ex[:, :, 0:P.w], in_=sc[:, :, 0:P.w],
                    func=AF.Exp, scale=0.125, bias=negln8[:])
            exs.append(ex)
        return exs

    def emit_avs(P, st, pair, j, exs, zst_box):
        h = 2 * pair + j
        zu = ps_av.tile([66, NT], F32, tag="av", name="av", bufs=1)
        for b in range(P.nb):
            base = b * P.np
            for off, fl in _bank_slices(base, P.np, 256):
                for g in range(P.nk // 2):
                    nc.tensor.matmul(
                        zu[:, base + off:base + off + fl],
                        st.v8[:, b * P.nk + 2 * g:b * P.nk + 2 * g + 2, h, :],
                        exs[g][:, :, base + off:base + off + fl],
                        start=(g == 0), stop=(g == P.nk // 2 - 1),
                        perf_mode=DR)
        if j == 0:
            zst_box[0] = expp.tile([65, 2, NT], F8, tag="zst", name="zst",
                                   bufs=3)
        zst = zst_box[0]
        with nc.allow_low_precision(reason="fp8 z"):
            if P.merged:
                nc.scalar.activation(out=zst[:, j, 0:P.w],
                                     in_=zu[0:65, 0:P.w], func=AF.Copy)
            else:
                nc.vector.tensor_copy(out=zst[:, j, 0:P.w],
                                      in_=zu[0:65, 0:P.w])
        nc.sync.dma_start(out=st.z8[64 * j:64 * (j + 1), pair, 0:P.w],
                          in_=zst[0:64, j, 0:P.w])
        if j == 1:
            nc.sync.dma_start(out=st.rows[32 * pair:32 * pair + 2, 0:P.w],
                              in_=zst[64:65, :, 0:P.w])
            with nc.allow_low_precision(reason="f32r feeds f32r mm"):
                nc.vector.reciprocal(
                    out=st.rinv[32 * pair:32 * pair + 2, 0:P.w],
                    in_=st.rows[32 * pair:32 * pair + 2, 0:P.w])

    def emit_pair_denom(P, st, pair):
        rinv = st.rinv[32 * pair:32 * pair + 2, 0:P.w]
        rep = ps_big.tile([128, 2, NT], F32, tag="sc", name="rep")
        tp = (96, 0) if pair == 3 else None
        for off, fl in _bank_slices(0, P.w):
            nc.tensor.matmul(rep[:, 0, off:off + fl],
                             sel2[32 * pair:32 * pair + 2, :],
                             rinv[:, off:off + fl], start=True, stop=True,
                             tile_position=tp)
        with nc.allow_low_precision(reason="fp8 z scale"):
            nc.vector.tensor_tensor(out=st.z8[:, pair, 0:P.w],
                                    in0=st.z8[:, pair, 0:P.w],
                                    in1=rep[:, 0, 0:P.w], op=OP.mult)

    def emit_outproj(P, st, m):
        ps = ps_big.tile([128, 2, NT], F32, tag="sc", name="pso")
        for b in range(P.nb):
            base = b * P.np
            for off, fl in _bank_slices(base, P.np, 256):
                for g in range(2):
                    nc.tensor.matmul(
                        ps[:, 0, base + off:base + off + fl],
                        P.wo[:, g, :, m * 128:(m + 1) * 128],
                        st.z8[:, 2 * g:2 * g + 2, base + off:base + off + fl],
                        start=(g == 0), stop=(g == 1), perf_mode=DR)
        if P.merged:
            dst = ht[m][:].rearrange("p (b n) -> p b n", b=2)[:, :, 0:NR]
            res = xT[m][:].rearrange("p (b n) -> p b n", b=2)[:, :, 0:NR]
            src_ps = ps[:, 0, 0:P.w].rearrange("p (b n) -> p b n", b=2)
        else:
            dst = ht[m][:, P.tok0:P.tok0 + P.np]
            res = xT[m][:, P.tok0:P.tok0 + P.np]
            src_ps = ps[:, 0, 0:P.np]
        with nc.allow_low_precision(reason="f32r ht"):
            nc.vector.scalar_tensor_tensor(
                out=dst, in0=src_ps,
                scalar=bias[P.wn[3]][:, m:m + 1],
                in1=res,
                op0=OP.add, op1=OP.add,
                accum_out=acc1[:, m, P.bp_idx:P.bp_idx + 1])
        dv = dst.bitcast(F32)
        if P.merged:
            def scr_out(scr):
                return scr[:, 0:P.w].rearrange("p (b n) -> p b n", b=2)
        else:
            def scr_out(scr):
                return scr[:, 0:P.np]
        if not P.sq_act:
            scr = small.tile([128, NT], F32, tag="sqd", name="sqd", bufs=2)
            nc.vector.scalar_tensor_tensor(
                out=scr_out(scr), in0=dv, scalar=1.0, in1=dv,
                op0=OP.mult, op1=OP.mult,
                accum_out=sq1[:, m, P.bp_idx:P.bp_idx + 1])
        else:
            scr = small.tile([128, NT], F32, tag="sqa", name="sqa", bufs=2)
            nc.scalar.activation(
                out=scr_out(scr), in_=dv, func=AF.Square,
                accum_out=sq1[:, m, P.bp_idx:P.bp_idx + 1])

    parts = []
    for bp_idx, (part, b) in enumerate([(0, 0), (1, 0), (1, 1)]):
        P = _Ctx()
        P.part = part
        P.b = b
        P.bp_idx = bp_idx
        P.merged = (part == 0)
        P.nb = 2 if P.merged else 1
        P.wn = ["rq", "rk", "rv", "ro"] if part == 0 else ["tq", "tk", "tv", "to"]
        P.np = NR if part == 0 else NT
        P.w = P.nb * P.np
        P.nk = P.np // 128
        P.tok0 = b * N + NR
        P.wq, P.wk, P.wv, P.wo = (w8[P.wn[0]], w8[P.wn[1]], w8[P.wn[2]],
                                  w8[P.wn[3]])
        parts.append(P)

    def v_sched(P):
        bts = [(b, t) for b in range(P.nb) for t in range(P.nk)]
        out = [[], [], [], []]
        for idx, bt in enumerate(bts):
            out[min(3, idx * 4 // len(bts))].append(bt)
        return out

    sts = {0: make_state(parts[0])}
    P0 = parts[0]
    for m in range(EC):
        emit_qk(P0, sts[0], "q", m)
        emit_qk(P0, sts[0], "k", m)
        for b, t in v_sched(P0)[m]:
            emit_v(P0, sts[0], b, t)
    pend = [None]

    def flush():
        if pend[0] is not None:
            pend[0]()
            pend[0] = None

    prev = None
    for i, P in enumerate(parts):
        st = sts[i]
        P.sq_act = (i == 2)    # outproj(T1) runs in the trailing window
        nxt = parts[i + 1] if i + 1 < len(parts) else None
        if nxt is not None:
            sts[i + 1] = make_state(nxt)
        zst_box = [None]
        for pair in range(4):
            for j in (0, 1):
                exs = emit_scores(P, st, pair, j)
                flush()

                def mk(P=P, st=st, pair=pair, j=j, exs=exs, zb=zst_box,
                       prev=prev, nxt=nxt, i=i):
                    def run():
                        emit_avs(P, st, pair, j, exs, zb)
                        if j == 1:
                            # per-pair boundary work, all dependency-ready:
                            # previous pair's denominators, the previous
                            # part's output projection, next part's proj
                            if pair >= 1:
                                emit_pair_denom(P, st, pair - 1)
                            elif prev is not None:
                                emit_pair_denom(prev[0], prev[1], 3)
                            if prev is not None:
                                emit_outproj(prev[0], prev[1], pair)
                            if nxt is not None:
                                emit_qk(nxt, sts[i + 1], "q", pair)
                                emit_qk(nxt, sts[i + 1], "k", pair)
                                for b, t in v_sched(nxt)[pair]:
                                    emit_v(nxt, sts[i + 1], b, t)
                    return run
                pend[0] = mk()
        prev = (P, st)
    flush()
    emit_pair_denom(prev[0], prev[1], 3)
    for m in range(EC):
        emit_outproj(prev[0], prev[1], m)

    # all exps done: swap the ACT table set to sqrt ahead of BN1
    warm2 = const.tile([1, 1], F32, tag="warm", name="warm2")
    nc.vector.memset(warm2[:], 1.0)
    nc.scalar.activation(out=warm2[:], in_=warm2[:], func=AF.Sqrt, scale=1.0)

    # ---------- BN1 (sums -> AllReduce -> params; fold into F1) ----------
    s1, t1 = _bn_params(nc, small, dram, acc1[:, :, 0:3],
                        sq1[:, :, 0:3], bias["bn1_g"],
                        bias["bn1_b"], epst[:], "bn1", for_timing)
    # b1' = f1_b + f1_w @ t1 (tiny matvec on original f1 tiles)
    b1p = small.tile([128, EC], F32, tag="b1p", name="b1p", bufs=1)
    t1r = small.tile([128, EC], F32R, tag="t1r", name="t1r", bufs=1)
    with nc.allow_low_precision(reason="f32r matvec input"):
        nc.vector.tensor_copy(out=t1r[:], in_=t1[:])
    psb = ps_big.tile([128, 2, NT], F32, tag="sc", name="psb1")
    for m in range(EC):
        for k in range(EC):
            nc.tensor.matmul(psb[:, 0, 2 * m:2 * m + 2],
                             f1[k][:, m * 128:(m + 1) * 128],
                             t1r[:, k:k + 1].to_broadcast((128, 2)),
                             start=(k == 0), stop=(k == EC - 1))
    nc.vector.tensor_tensor(out=b1p[:],
                            in0=psb[:, 0, 0:2 * EC:2], in1=bias["f1"],
                            op=OP.add)
    # fold BN1 scale into f1 (in place, per input-channel partition)
    for k in range(EC):
        with nc.allow_low_precision(reason="f32r weights"):
            nc.vector.tensor_scalar(out=f1[k][:], in0=f1[k][:],
                                    scalar1=s1[:, k:k + 1], scalar2=None,
                                    op0=OP.mult)

    # hn = s1*ht + t1 (BN1 output, residual only) -> xT slots
    hn = [const.tile([128, TOK], F32R, tag=f"xT{k}", name=f"hn{k}")
          for k in range(EC)]
    for m in range(EC):
        for i, (off, fl) in enumerate(_bank_slices(0, TOK)):
            src = ht[m][:, off:off + fl].bitcast(F32)
            dstv = hn[m][:, off:off + fl]
            with nc.allow_low_precision(reason="f32r hn"):
                nc.gpsimd.tensor_scalar(out=dstv, in0=src,
                                        scalar1=s1[:, m:m + 1],
                                        scalar2=t1[:, m:m + 1],
                                        op0=OP.mult, op1=OP.add)

    # ---------- FFN ----------
    h1 = act.tile([128, EC, TOK], F8, tag="h1", name="h1")
    for m in range(EC):
        for off, fl in _bank_slices(0, TOK):
            ps = ps_big.tile([128, 2, NT], F32, tag="sc", name="psf1")
            for k in range(EC):
                nc.tensor.matmul(ps[:, 0, 0:fl], f1[k][:, m * 128:(m + 1) * 128],
                                 ht[k][:, off:off + fl],
                                 start=(k == 0), stop=(k == EC - 1))
            with nc.allow_low_precision(reason="fp8 h1"):
                nc.scalar.activation(out=h1[:, m, off:off + fl],
                                     in_=ps[:, 0, 0:fl], func=AF.Relu,
                                     bias=b1p[:, m:m + 1], scale=1.0)
    ho = [act.tile([128, TOK], F32, tag=f"ht{k}", name=f"ho{k}")
          for k in range(EC)]
    acc2 = small.tile([128, EC, 4], F32, tag="acc2", name="acc2", bufs=1)
    sq2 = small.tile([128, EC, 4], F32, tag="sq2", name="sq2", bufs=1)
    for m in range(EC):
        for i, (off, fl) in enumerate(_bank_slices(0, TOK)):
            ps = ps_big.tile([128, 2, NT], F32, tag="sc", name="psf2")
            for c in range(0, fl, 256):
                cl = min(256, fl - c)
                for g in range(2):
                    nc.tensor.matmul(
                        ps[:, 0, c:c + cl],
                        f28[:, g, :, m * 128:(m + 1) * 128],
                        h1[:, 2 * g:2 * g + 2, off + c:off + c + cl],
                        start=(g == 0), stop=(g == 1), perf_mode=DR)
            dst = ho[m][:, off:off + fl]
            nc.vector.scalar_tensor_tensor(
                out=dst, in0=ps[:, 0, 0:fl], scalar=bias["f2"][:, m:m + 1],
                in1=hn[m][:, off:off + fl].bitcast(F32),
                op0=OP.add, op1=OP.add,
                accum_out=acc2[:, m, i:i + 1])
            # sumsq for BN2 on ACT (DVE is saturated by the STT evacs here)
            scr = small.tile([128, NT], F32, tag="sqa", name="sqa2", bufs=2)
            nc.scalar.activation(
                out=scr[:, 0:fl], in_=dst, func=AF.Square,
                accum_out=sq2[:, m, i:i + 1])

    # ---------- BN2 + output (pipelined per 512-token slice) ----------
    s2, t2 = _bn_params(nc, small, dram, acc2[:], sq2[:],
                        bias["bn2_g"],
                        bias["bn2_b"], epst[:], "bn2", for_timing)
    for i, (off, fl) in enumerate(_bank_slices(0, TOK)):
        for m in range(EC):
            dst = ho[m][:, off:off + fl]
            eng = (i * EC + m) % 3
            if eng == 0:
                nc.vector.tensor_scalar(out=dst, in0=dst,
                                        scalar1=s2[:, m:m + 1],
                                        scalar2=t2[:, m:m + 1],
                                        op0=OP.mult, op1=OP.add)
            elif eng == 1:
                nc.scalar.activation(out=dst, in_=dst, func=AF.Identity,
                                     bias=t2[:, m:m + 1], scale=s2[:, m:m + 1])
            else:
                nc.gpsimd.tensor_scalar(out=dst, in0=dst,
                                        scalar1=s2[:, m:m + 1],
                                        scalar2=t2[:, m:m + 1],
                                        op0=OP.mult, op1=OP.add)
            nc.sync.dma_start(out=yT_d.ap()[m * 128:(m + 1) * 128, off:off + fl],
                              in_=dst)


def _bn_params(nc, small, dram, accs, sqs, g_sb, b_sb, epst, name,
               for_timing=False):
    """Per-channel scale/shift for training-mode BN over all B*N tokens from
    raw per-(m, slice) sums: reduce -> 8-core AllReduce -> mu/var ->
    sqrt+recip. Returns (s [128, EC], t [128, EC]) tiles."""
    ccin = dram.tile([128, 2 * EC], F32, tag=f"cci_{name}", name=f"cci_{name}")
    ccout = dram.tile([128, 2 * EC], F32, tag=f"cco_{name}", name=f"cco_{name}")
    su = small.tile([128, 2, EC], F32, tag=f"su_{name}", name=f"su_{name}")
    nc.vector.tensor_reduce(out=su[:, 0, :], in_=accs,
                            axis=mybir.AxisListType.X, op=OP.add)
    nc.vector.tensor_reduce(out=su[:, 1, :], in_=sqs,
                            axis=mybir.AxisListType.X, op=OP.add)
    nc.sync.dma_start(out=ccin[:], in_=su[:].rearrange("p a b -> p (a b)"))
    if for_timing:
        # TimelineSim cannot model collectives; substitute a same-shape copy
        nc.gpsimd.dma_start(out=ccout[:], in_=ccin[:])
    else:
        nc.gpsimd.collective_compute(
            "AllReduce", OP.add, replica_groups=[list(range(N_CORES))],
            ins=[ccin.opt()], outs=[ccout.opt()])
    gsa = small.tile([128, 2, EC], F32, tag=f"gs_{name}", name=f"gs_{name}")
    nc.sync.dma_start(out=gsa[:].rearrange("p a b -> p (a b)"), in_=ccout[:])
    mu = small.tile([128, EC], F32, tag=f"mu_{name}", name=f"mu_{name}", bufs=1)
    var = small.tile([128, EC], F32, tag=f"var_{name}", name=f"var_{name}",
                     bufs=1)
    nc.vector.tensor_scalar(out=mu[:], in0=gsa[:, 0, :],
                            scalar1=1.0 / N_GLOBAL, scalar2=None, op0=OP.mult)
    nc.vector.tensor_tensor(out=var[:], in0=mu[:], in1=mu[:], op=OP.mult)
    nc.vector.scalar_tensor_tensor(out=var[:], in0=gsa[:, 1, :],
                                   scalar=1.0 / N_GLOBAL, in1=var[:],
                                   op0=OP.mult, op1=OP.subtract)
    sq = small.tile([128, EC], F32, tag=f"sq_{name}", name=f"sq_{name}", bufs=1)
    nc.scalar.activation(out=sq[:], in_=var[:], func=AF.Sqrt, bias=epst,
                         scale=1.0)
    r0 = small.tile([128, EC], F32, tag=f"r0_{name}", name=f"r0_{name}", bufs=1)
    nc.vector.reciprocal(out=r0[:], in_=sq[:])
    s_all = small.tile([128, EC], F32, tag=f"s_{name}", name=f"s_{name}",
                       bufs=1)
    sh_all = small.tile([128, EC], F32, tag=f"sh_{name}", name=f"sh_{name}",
                        bufs=1)
    nc.vector.tensor_tensor(out=s_all[:], in0=r0[:], in1=g_sb, op=OP.mult)
    nc.vector.tensor_tensor(out=sh_all[:], in0=mu[:], in1=s_all[:], op=OP.mult)
    nc.vector.tensor_tensor(out=sh_all[:], in0=b_sb, in1=sh_all[:],
                            op=OP.subtract)
    return s_all, sh_all


_NC_CACHE = None


def _get_nc():
    global _NC_CACHE
    if _NC_CACHE is None:
        _NC_CACHE = build()
    return _NC_CACHE


def make_in_maps(inputs):
    import ml_dtypes
    f8 = ml_dtypes.float8_e4m3
    shared = {}
    for n in W8_NAMES:
        w = np.asarray(inputs[f"{n}_w"], dtype=np.float32)      # [E, E]
        # w8[p, g, jt, j] = W[j, (2g+jt)*128 + p]
        w8 = np.ascontiguousarray(
            w.T.reshape(2, 2, 128, E).transpose(2, 0, 1, 3)).astype(f8)
        shared[f"{n}_w8"] = w8.reshape(128, 4 * E)
    shared["f1_wT"] = np.ascontiguousarray(
        np.asarray(inputs["f1_w"], dtype=np.float32).T)
    w2 = np.asarray(inputs["f2_w"], dtype=np.float32)
    shared["f2_w8"] = np.ascontiguousarray(
        w2.T.reshape(2, 2, 128, E).transpose(2, 0, 1, 3)).astype(
            f8).reshape(128, 4 * E)
    for n in ["rv", "tv"]:
        shared[f"{n}_brep"] = np.ascontiguousarray(
            np.broadcast_to(np.asarray(inputs[f"{n}_b"], dtype=np.float32),
                            (128, E)))
    bpk = np.empty((128, len(ALL_B) * EC), dtype=np.float32)
    for i, n in enumerate(ALL_B):
        vec = inputs[f"{n}_b"] if n in W8_NAMES + ["f1", "f2"] else inputs[n]
        bpk[:, i * EC:(i + 1) * EC] = np.asarray(vec).reshape(EC, 128).T
    shared["bpk"] = bpk
    sel2 = np.zeros((98, 128), dtype=np.float32)
    for p in range(4):
        sel2[32 * p, 0:64] = IVS
        sel2[32 * p + 1, 64:128] = IVS
    shared["sel2"] = sel2
    shared["ones1"] = np.ones((1, 128), dtype=np.float32)

    x = np.asarray(inputs["x"], dtype=np.float32)
    in_maps = []
    for i in range(N_CORES):
        xc = x[BL * i:BL * (i + 1)]                      # [BL, N, E]
        xT = np.ascontiguousarray(xc.transpose(2, 0, 1).reshape(E, TOK))
        x8 = np.ascontiguousarray(
            xT.reshape(EC, 128, BL, N).transpose(1, 0, 2, 3)).astype(f8)
        x8r = np.ascontiguousarray(x8[:, :, :, 0:NR])
        x8t = np.ascontiguousarray(x8[:, :, :, NR:N])
        in_maps.append({"xT": xT,
                        "x8r": x8r.reshape(128, EC * 2 * NR),
                        "x8t": x8t.reshape(128, EC * 2 * NT), **shared})
    return in_maps


def assemble_output(results):
    y = np.empty((B, N, E), dtype=np.float32)
    for i in range(N_CORES):
        yT = results[i]["yT"]                            # [E, TOK]
        y[BL * i:BL * (i + 1)] = yT.reshape(E, BL, N).transpose(1, 2, 0)
    return y


def kernel(**inputs):
    nc = _get_nc()
    in_maps = make_in_maps(inputs)
    res = run_bass_kernel_spmd(nc, in_maps, core_ids=list(range(N_CORES)))
    return assemble_output(res.results)


if __name__ == "__main__":
    nc = build()
    print("build ok")


# revision 49
# speedup vs baseline: 1.3016x; 1.0088x over previous
"""Trainium2 Bass kernel for nn_EncoderBlock (dual self-attention + BN + FFN + BN).

Sharding: data-parallel over batch (16 batches -> 2 per core on 8 cores).

v2: the attention block runs in fp8e4m3 with DoubleRow matmuls (4x PE rate on
the qkv/out projections and AV), validated to ~8e-4 end-to-end rel err.
Scores stay bf16 (64-deep contraction can't DoubleRow). exp outputs fp8
directly from ACT with the softmax /8 range shift folded into the exp bias;
the ones-column (value 1/64) inside V yields denominators from the AV matmul.
K/Q/O biases are per-partition scalars in the evacuation ops (K's provably
cancels in softmax but is applied anyway); V's bias rides the existing
psum->v8 add. The out-projection evacuation is a fused scalar_tensor_tensor
(psum + bias + residual) whose accum_out doubles as the BN1 channel sums;
sumsq comes from a square pass split across DVE/ACT. BN1's scale is folded
into the F1 weights (in-place) so the FFN starts right after the AllReduce;
BN1's shift becomes an F1 bias correction via a tiny matvec. BN2 stats use
the same accum trick; the final normalize+store is pipelined per 512-column
slice across DVE/ACT/Pool with immediate per-slice DMA.
BatchNorm batch stats use a 4KB AllReduce across the 8 cores (twice).
"""

import numpy as np
import concourse.bass as bass
import concourse.bacc as bacc
import concourse.tile as tile
from concourse import mybir
from concourse.bass_utils import run_bass_kernel_spmd

dt = mybir.dt
F32 = dt.float32
F32R = dt.float32r
BF16 = dt.bfloat16
F8 = dt.float8e4
AF = mybir.ActivationFunctionType
OP = mybir.AluOpType
DR = mybir.MatmulPerfMode.DoubleRow

N_CORES = 8
B, N, E, H, DK = 16, 1024, 512, 8, 64
NR, NT = 256, 768          # robot / task sequence lengths
BL = B // N_CORES          # local batches per core
TOK = BL * N               # local tokens per core
EC = E // 128              # channel chunks of 128
N_GLOBAL = B * N           # BN stat count
EPS = 1e-5
LN8 = 2.0794415416798357   # exp range shift: ex = exp(s)/8
IVS = 1.0 / 64.0           # ones-column value (denominator scale)

W8_NAMES = ["rq", "rk", "rv", "ro", "tq", "tk", "tv", "to"]
ALL_B = W8_NAMES + ["f1", "f2", "bn1_g", "bn1_b", "bn2_g", "bn2_b"]


def _bank_slices(base, length, maxlen=512):
    """Split [base, base+length) into pieces (<=maxlen) that never cross a
    512-col PSUM bank boundary."""
    out = []
    cur = base
    end = base + length
    while cur < end:
        nb = (cur // 512 + 1) * 512
        fl = min(end, min(nb, cur + maxlen)) - cur
        out.append((cur - base, fl))
        cur += fl
    return out


def build(for_timing=False):
    nc = bacc.Bacc("TRN2", target_bir_lowering=False, debug=False,
                   num_devices=N_CORES)

    xT_d = nc.dram_tensor("xT", [E, TOK], F32, kind="ExternalInput")
    x8r_d = nc.dram_tensor("x8r", [128, EC * 2 * NR], F8, kind="ExternalInput")
    x8t_d = nc.dram_tensor("x8t", [128, EC * 2 * NT], F8, kind="ExternalInput")
    w8_d = {n: nc.dram_tensor(f"{n}_w8", [128, 4 * E], F8, kind="ExternalInput")
            for n in W8_NAMES}
    f_d = {"f1": nc.dram_tensor("f1_wT", [E, E], F32, kind="ExternalInput"),
           "f2": nc.dram_tensor("f2_w8", [128, 4 * E], F8, kind="ExternalInput")}
    bpk_d = nc.dram_tensor("bpk", [128, len(ALL_B) * EC], F32,
                           kind="ExternalInput")
    vrep_d = {n: nc.dram_tensor(f"{n}_brep", [128, E], F32, kind="ExternalInput")
              for n in ["rv", "tv"]}
    sel2_d = nc.dram_tensor("sel2", [98, 128], F32, kind="ExternalInput")
    ones1_d = nc.dram_tensor("ones1", [1, 128], F32, kind="ExternalInput")
    yT_d = nc.dram_tensor("yT", [E, TOK], F32, kind="ExternalOutput")

    from contextlib import ExitStack
    with tile.TileContext(nc) as tc, ExitStack() as es:
        const = es.enter_context(tc.tile_pool(name="const", bufs=1))
        wpool = es.enter_context(tc.tile_pool(name="w", bufs=1))
        act = es.enter_context(tc.tile_pool(name="act", bufs=1))
        attn = es.enter_context(tc.tile_pool(name="attn", bufs=2))
        expp = es.enter_context(tc.tile_pool(name="expp", bufs=2))
        small = es.enter_context(tc.tile_pool(name="small", bufs=2))
        dram = es.enter_context(tc.tile_pool(name="dram", bufs=1, space="DRAM"))
        ps_big = es.enter_context(tc.tile_pool(name="ps_big", bufs=2, space="PSUM"))
        ps_av = es.enter_context(tc.tile_pool(name="ps_av", bufs=2, space="PSUM"))
        _body(nc, const, wpool, act, attn, expp, small, dram, ps_big, ps_av,
              xT_d, x8r_d, x8t_d, w8_d, f_d, bpk_d, vrep_d, sel2_d, ones1_d,
              yT_d, for_timing)
    nc.finalize()
    return nc


class _Ctx:
    pass


def _body(nc, const, wpool, act, attn, expp, small, dram, ps_big, ps_av,
          xT_d, x8r_d, x8t_d, w8_d, f_d, bpk_d, vrep_d, sel2_d, ones1_d,
          yT_d, for_timing):
    # ---------- constants / inputs resident in SBUF ----------
    # DMA emission order == issue order on the sync queue: the first
    # projection needs rq/rk weights + robot x8 columns; bulk (xT, task x8,
    # FFN weights) trickles in behind.
    w8 = {}
    for n in ["rq", "rk"]:
        t = wpool.tile([128, 2, 2, E], F8, tag=f"w8{n}", name=f"w8{n}")
        nc.sync.dma_start(out=t[:], in_=w8_d[n].ap())
        w8[n] = t
    x8r = const.tile([128, EC, 2, NR], F8, tag="x8r", name="x8r")
    x8t = const.tile([128, EC, 2, NT], F8, tag="x8t", name="x8t")
    nc.sync.dma_start(out=x8r[:], in_=x8r_d.ap())
    bpk = const.tile([128, len(ALL_B) * EC], F32, tag="bpk", name="bpk")
    nc.sync.dma_start(out=bpk[:], in_=bpk_d.ap())
    bias = {n: bpk[:, i * EC:(i + 1) * EC] for i, n in enumerate(ALL_B)}
    for n in ["rv", "ro"]:
        t = wpool.tile([128, 2, 2, E], F8, tag=f"w8{n}", name=f"w8{n}")
        nc.sync.dma_start(out=t[:], in_=w8_d[n].ap())
        w8[n] = t
    vrep = {}
    t = const.tile([128, E], F32R, tag="vr_rv", name="vr_rv")
    nc.sync.dma_start(out=t[:], in_=vrep_d["rv"].ap().bitcast(F32R))
    vrep["rv"] = t
    sel2 = const.tile([98, 128], F32R, tag="sel2", name="sel2")
    nc.sync.dma_start(out=sel2[:], in_=sel2_d.ap().bitcast(F32R))
    ones1 = const.tile([1, 128], F32R, tag="ones1", name="ones1")
    nc.sync.dma_start(out=ones1[:], in_=ones1_d.ap().bitcast(F32R))
    # Bulk loads ride the Pool SWDGE train (no HWDGE contention with the
    # latency-critical z8/rows transfers on sync), ordered by need time:
    # xT robot (outproj R0 ~18us) -> task x8/weights (T0 proj ~30us) ->
    # xT task (outproj T0 ~110us) -> FFN weights (BN1 ~170us).
    xT = [const.tile([128, TOK], F32R, tag=f"xT{k}", name=f"xT{k}")
          for k in range(EC)]
    for k in range(EC):
        nc.sync.dma_start(out=xT[k][:, 0:NR],
                          in_=xT_d.ap()[k * 128:(k + 1) * 128, 0:NR].bitcast(F32R))
        nc.sync.dma_start(out=xT[k][:, N:N + NR],
                          in_=xT_d.ap()[k * 128:(k + 1) * 128, N:N + NR].bitcast(F32R))
    x8td = x8t_d.ap().rearrange("p (k b t) -> p k b t", k=EC, b=2)
    nc.gpsimd.dma_start(out=x8t[:, :, 0, :], in_=x8td[:, :, 0, :])
    nc.gpsimd.dma_start(out=x8t[:, :, 1, :], in_=x8td[:, :, 1, :])
    for n in ["tq", "tk", "tv", "to"]:
        t = wpool.tile([128, 2, 2, E], F8, tag=f"w8{n}", name=f"w8{n}")
        nc.gpsimd.dma_start(out=t[:], in_=w8_d[n].ap())
        w8[n] = t
    t = const.tile([128, E], F32R, tag="vr_tv", name="vr_tv")
    nc.gpsimd.dma_start(out=t[:], in_=vrep_d["tv"].ap().bitcast(F32R))
    vrep["tv"] = t
    for k in range(EC):
        nc.sync.dma_start(out=xT[k][:, NR:N],
                          in_=xT_d.ap()[k * 128:(k + 1) * 128, NR:N].bitcast(F32R))
        nc.sync.dma_start(out=xT[k][:, N + NR:],
                          in_=xT_d.ap()[k * 128:(k + 1) * 128, N + NR:].bitcast(F32R))
    # FFN weights (f1 f32r: folded in place later; f2 bf16 to match bf16 h1)
    f1 = [wpool.tile([128, E], F32R, tag=f"f1_{k}", name=f"f1_{k}")
          for k in range(EC)]
    f28 = wpool.tile([128, 2, 2, E], F8, tag="f28", name="f28")
    for k in range(EC):
        nc.gpsimd.dma_start(out=f1[k][:],
                            in_=f_d["f1"].ap()[k * 128:(k + 1) * 128, :].bitcast(F32R))
    nc.gpsimd.dma_start(out=f28[:], in_=f_d["f2"].ap())

    # prefetch the exp ACT table set while input DMAs are in flight
    warm = const.tile([1, 1], F32, tag="warm", name="warm")
    nc.vector.memset(warm[:], 0.0)
    nc.scalar.activation(out=warm[:], in_=warm[:], func=AF.Exp, scale=1.0)
    negln8 = const.tile([128, 1], F32, tag="negln8", name="negln8")
    nc.gpsimd.memset(negln8[:], -LN8)
    epst = const.tile([128, 1], F32, tag="epst", name="epst")
    nc.gpsimd.memset(epst[:], EPS)


    # h-tilde (pre-BN1 attention output) accumulated across parts/batches,
    # with per-(m, batch-part) channel sums / sumsq for BN1
    ht = [act.tile([128, TOK], F32R, tag=f"ht{k}", name=f"ht{k}")
          for k in range(EC)]
    acc1 = small.tile([128, EC, 4], F32, tag="acc1", name="acc1", bufs=1)
    sq1 = small.tile([128, EC, 4], F32, tag="sq1", name="sq1", bufs=1)

    # ---------- attention (fine-grained interleaved emission) ----------
    # Per part, per m-chunk: project q/k chunk m, interleave the PREVIOUS
    # part's output-projection chunk m, then run head pair m (scores -> exp
    # -> AV -> evac) and its denominator broadcast + z8 scale. The exp stream
    # on ACT paces everything; PE/DVE work rides underneath it. Robot parts
    # are DVE-bound instead, so their k/zu evacuations go to ACT.
    def make_state(P):
        st = _Ctx()
        st.qT = [attn.tile([128, NT], BF16, tag=f"qT{m}", name=f"qT{m}")
                 for m in range(EC)]
        st.kT = [attn.tile([128, NT], BF16, tag=f"kT{m}", name=f"kT{m}")
                 for m in range(EC)]
        st.v8 = attn.tile([128, 6, H, DK + 2], F8, tag="v8", name="v8")
        st.z8 = attn.tile([128, 4, NT], F8, tag="z8", name="z8")
        st.rows = small.tile([98, NT], F8, tag="rows", name="rows")
        st.rinv = small.tile([98, NT], F32R, tag="rinv", name="rinv")
        if P.merged:
            st.x8p = [x8r[:, :, b, :] for b in range(2)]
        else:
            st.x8p = [x8t[:, :, P.b, :]]
        with nc.allow_low_precision(reason="fp8 attention"):
            nc.vector.memset(st.v8[:, :, :, DK:DK + 1], IVS)
            nc.vector.memset(st.v8[:, :, :, DK + 1:DK + 2], 0.0)
        return st

    def emit_qk(P, st, m):
        ps = ps_big.tile([128, 2, NT], F32, tag="sc", name="psq")
        for r, (wt, o_t, bname) in enumerate(
                [(P.wq, st.qT, P.wn[0]), (P.wk, st.kT, P.wn[1])]):
            for b in range(P.nb):
                for off, fl in _bank_slices(b * P.np, P.np, 256):
                    for g in range(2):
                        nc.tensor.matmul(
                            ps[:, r, b * P.np + off:b * P.np + off + fl],
                            wt[:, g, :, m * 128:(m + 1) * 128],
                            st.x8p[b][:, 2 * g:2 * g + 2, off:off + fl],
                            start=(g == 0), stop=(g == 1), perf_mode=DR)
            with nc.allow_low_precision(reason="bf16 qk"):
                if P.merged and r == 1:
                    nc.scalar.activation(
                        out=o_t[m][:, 0:P.w], in_=ps[:, r, 0:P.w],
                        func=AF.Identity, bias=bias[bname][:, m:m + 1],
                        scale=1.0)
                else:
                    nc.vector.tensor_scalar(
                        out=o_t[m][:, 0:P.w], in0=ps[:, r, 0:P.w],
                        scalar1=bias[bname][:, m:m + 1], scalar2=None,
                        op0=OP.add)

    def emit_v(P, st, b, t):
        ps = ps_big.tile([128, 2, NT], F32, tag="sc", name="psv")
        for j0 in (0, 256):
            for g in range(2):
                nc.tensor.matmul(
                    ps[:, 0, j0:j0 + 256],
                    st.x8p[b][:, 2 * g:2 * g + 2, t * 128:(t + 1) * 128],
                    P.wv[:, g, :, j0:j0 + 256],
                    start=(g == 0), stop=(g == 1), perf_mode=DR)
            # V bias via a K=1 ones-row matmul so the evac is a pure copy
            # that can ride the otherwise-idle ACT slots
            nc.tensor.matmul(
                ps[:, 0, j0:j0 + 256], ones1[:],
                vrep[P.wn[2]][0:1, j0:j0 + 256],
                start=False, stop=False, skip_group_check=True)
        with nc.allow_low_precision(reason="fp8 v"):
            nc.scalar.activation(
                out=st.v8[:, b * P.nk + t, :, 0:DK],
                in_=ps[:, 0, 0:E].rearrange("p (h d) -> p h d", h=H),
                func=AF.Copy)

    def emit_scores(P, st, pair, j):
        exs = []
        for g in range(P.nk // 2):
            sc = ps_big.tile([128, 2, NT], F32, tag="sc", name="sc")
            for b in range(P.nb):
                qh = st.qT[pair][j * 64:j * 64 + 64,
                                 b * P.np:(b + 1) * P.np]
                for j2 in range(2):
                    kc = 2 * g + j2
                    kh = st.kT[pair][j * 64:j * 64 + 64,
                                     b * P.np + kc * 128:
                                     b * P.np + (kc + 1) * 128]
                    for off, fl in _bank_slices(j2 * NT + b * P.np, P.np):
                        nc.tensor.matmul(
                            sc[:, j2, b * P.np + off:b * P.np + off + fl],
                            kh, qh[:, off:off + fl],
                            start=True, stop=True)
            ex = expp.tile([128, 2, NT], F8, tag="exp", name="exp", bufs=9)
            with nc.allow_low_precision(reason="fp8 exp"):
                nc.scalar.activation(
                    out=ex[:, :, 0:P.w], in_=sc[:, :, 0:P.w],
                    func=AF.Exp, scale=0.125, bias=negln8[:])
            exs.append(ex)
        return exs

    def emit_avs(P, st, pair, j, exs, zst_box):
        h = 2 * pair + j
        zu = ps_av.tile([66, NT], F32, tag="av", name="av", bufs=1)
        for b in range(P.nb):
            base = b * P.np
            for off, fl in _bank_slices(base, P.np, 256):
                for g in range(P.nk // 2):
                    nc.tensor.matmul(
                        zu[:, base + off:base + off + fl],
                        st.v8[:, b * P.nk + 2 * g:b * P.nk + 2 * g + 2, h, :],
                        exs[g][:, :, base + off:base + off + fl],
                        start=(g == 0), stop=(g == P.nk // 2 - 1),
                        perf_mode=DR)
        if j == 0:
            zst_box[0] = expp.tile([65, 2, NT], F8, tag="zst", name="zst",
                                   bufs=3)
        zst = zst_box[0]
        with nc.allow_low_precision(reason="fp8 z"):
            if P.merged:
                nc.scalar.activation(out=zst[:, j, 0:P.w],
                                     in_=zu[0:65, 0:P.w], func=AF.Copy)
            else:
                nc.vector.tensor_copy(out=zst[:, j, 0:P.w],
                                      in_=zu[0:65, 0:P.w])
        nc.sync.dma_start(out=st.z8[64 * j:64 * (j + 1), pair, 0:P.w],
                          in_=zst[0:64, j, 0:P.w])
        if j == 1:
            nc.sync.dma_start(out=st.rows[32 * pair:32 * pair + 2, 0:P.w],
                              in_=zst[64:65, :, 0:P.w])
            with nc.allow_low_precision(reason="f32r feeds f32r mm"):
                nc.vector.reciprocal(
                    out=st.rinv[32 * pair:32 * pair + 2, 0:P.w],
                    in_=st.rows[32 * pair:32 * pair + 2, 0:P.w])

    def emit_pair_denom(P, st, pair):
        rinv = st.rinv[32 * pair:32 * pair + 2, 0:P.w]
        rep = ps_big.tile([128, 2, NT], F32, tag="sc", name="rep")
        tp = (96, 0) if pair == 3 else None
        for off, fl in _bank_slices(0, P.w):
            nc.tensor.matmul(rep[:, 0, off:off + fl],
                             sel2[32 * pair:32 * pair + 2, :],
                             rinv[:, off:off + fl], start=True, stop=True,
                             tile_position=tp)
        with nc.allow_low_precision(reason="fp8 z scale"):
            nc.vector.tensor_tensor(out=st.z8[:, pair, 0:P.w],
                                    in0=st.z8[:, pair, 0:P.w],
                                    in1=rep[:, 0, 0:P.w], op=OP.mult)

    def emit_outproj(P, st, m):
        ps = ps_big.tile([128, 2, NT], F32, tag="sc", name="pso")
        for b in range(P.nb):
            base = b * P.np
            for off, fl in _bank_slices(base, P.np, 256):
                for g in range(2):
                    nc.tensor.matmul(
                        ps[:, 0, base + off:base + off + fl],
                        P.wo[:, g, :, m * 128:(m + 1) * 128],
                        st.z8[:, 2 * g:2 * g + 2, base + off:base + off + fl],
                        start=(g == 0), stop=(g == 1), perf_mode=DR)
        if P.merged:
            dst = ht[m][:].rearrange("p (b n) -> p b n", b=2)[:, :, 0:NR]
            res = xT[m][:].rearrange("p (b n) -> p b n", b=2)[:, :, 0:NR]
            src_ps = ps[:, 0, 0:P.w].rearrange("p (b n) -> p b n", b=2)
        else:
            dst = ht[m][:, P.tok0:P.tok0 + P.np]
            res = xT[m][:, P.tok0:P.tok0 + P.np]
            src_ps = ps[:, 0, 0:P.np]
        with nc.allow_low_precision(reason="f32r ht"):
            nc.vector.scalar_tensor_tensor(
                out=dst, in0=src_ps,
                scalar=bias[P.wn[3]][:, m:m + 1],
                in1=res,
                op0=OP.add, op1=OP.add,
                accum_out=acc1[:, m, P.bp_idx:P.bp_idx + 1])
        dv = dst.bitcast(F32)
        if P.merged:
            def scr_out(scr):
                return scr[:, 0:P.w].rearrange("p (b n) -> p b n", b=2)
        else:
            def scr_out(scr):
                return scr[:, 0:P.np]
        if not P.sq_act:
            scr = small.tile([128, NT], F32, tag="sqd", name="sqd", bufs=2)
            nc.vector.scalar_tensor_tensor(
                out=scr_out(scr), in0=dv, scalar=1.0, in1=dv,
                op0=OP.mult, op1=OP.mult,
                accum_out=sq1[:, m, P.bp_idx:P.bp_idx + 1])
        else:
            scr = small.tile([128, NT], F32, tag="sqa", name="sqa", bufs=2)
            nc.scalar.activation(
                out=scr_out(scr), in_=dv, func=AF.Square,
                accum_out=sq1[:, m, P.bp_idx:P.bp_idx + 1])

    parts = []
    for bp_idx, (part, b) in enumerate([(0, 0), (1, 0), (1, 1)]):
        P = _Ctx()
        P.part = part
        P.b = b
        P.bp_idx = bp_idx
        P.merged = (part == 0)
        P.nb = 2 if P.merged else 1
        P.wn = ["rq", "rk", "rv", "ro"] if part == 0 else ["tq", "tk", "tv", "to"]
        P.np = NR if part == 0 else NT
        P.w = P.nb * P.np
        P.nk = P.np // 128
        P.tok0 = b * N + NR
        P.wq, P.wk, P.wv, P.wo = (w8[P.wn[0]], w8[P.wn[1]], w8[P.wn[2]],
                                  w8[P.wn[3]])
        parts.append(P)

    def v_sched(P):
        bts = [(b, t) for b in range(P.nb) for t in range(P.nk)]
        out = [[], [], [], []]
        for idx, bt in enumerate(bts):
            out[min(3, idx * 4 // len(bts))].append(bt)
        return out

    sts = {0: make_state(parts[0])}
    P0 = parts[0]
    for m in range(EC):
        emit_qk(P0, sts[0], m)
        for b, t in v_sched(P0)[m]:
            emit_v(P0, sts[0], b, t)
    pend = [None]

    def flush():
        if pend[0] is not None:
            pend[0]()
            pend[0] = None

    prev = None
    for i, P in enumerate(parts):
        st = sts[i]
        P.sq_act = True    # sumsq rides the ACT stall windows
        nxt = parts[i + 1] if i + 1 < len(parts) else None
        if nxt is not None:
            sts[i + 1] = make_state(nxt)
        zst_box = [None]
        for pair in range(4):
            for j in (0, 1):
                exs = emit_scores(P, st, pair, j)
                flush()

                def mk(P=P, st=st, pair=pair, j=j, exs=exs, zb=zst_box,
                       prev=prev, nxt=nxt, i=i):
                    def run():
                        emit_avs(P, st, pair, j, exs, zb)
                        if j == 0:
                            # mid-pair: previous part's output projection
                            # (its final denominator first, once)
                            if prev is not None:
                                if pair == 0:
                                    emit_pair_denom(prev[0], prev[1], 3)
                                emit_outproj(prev[0], prev[1], pair)
                        else:
                            # pair boundary: own denominators one pair late,
                            # next part's projections
                            if pair >= 1:
                                emit_pair_denom(P, st, pair - 1)
                            if nxt is not None:
                                emit_qk(nxt, sts[i + 1], pair)
                                for b, t in v_sched(nxt)[pair]:
                                    emit_v(nxt, sts[i + 1], b, t)
                    return run
                pend[0] = mk()
        prev = (P, st)
    flush()
    emit_pair_denom(prev[0], prev[1], 3)
    for m in range(EC):
        emit_outproj(prev[0], prev[1], m)

    # all exps done: swap the ACT table set to sqrt ahead of BN1
    warm2 = const.tile([1, 1], F32, tag="warm", name="warm2")
    nc.vector.memset(warm2[:], 1.0)
    nc.scalar.activation(out=warm2[:], in_=warm2[:], func=AF.Sqrt, scale=1.0)

    # ---------- BN1 (sums -> AllReduce -> params; fold into F1) ----------
    s1, t1 = _bn_params(nc, small, dram, acc1[:, :, 0:3],
                        sq1[:, :, 0:3], bias["bn1_g"],
                        bias["bn1_b"], epst[:], "bn1", for_timing)
    # b1' = f1_b + f1_w @ t1 (tiny matvec on original f1 tiles)
    b1p = small.tile([128, EC], F32, tag="b1p", name="b1p", bufs=1)
    t1r = small.tile([128, EC], F32R, tag="t1r", name="t1r", bufs=1)
    with nc.allow_low_precision(reason="f32r matvec input"):
        nc.vector.tensor_copy(out=t1r[:], in_=t1[:])
    psb = ps_big.tile([128, 2, NT], F32, tag="sc", name="psb1")
    for m in range(EC):
        for k in range(EC):
            nc.tensor.matmul(psb[:, 0, 2 * m:2 * m + 2],
                             f1[k][:, m * 128:(m + 1) * 128],
                             t1r[:, k:k + 1].to_broadcast((128, 2)),
                             start=(k == 0), stop=(k == EC - 1))
    nc.vector.tensor_tensor(out=b1p[:],
                            in0=psb[:, 0, 0:2 * EC:2], in1=bias["f1"],
                            op=OP.add)
    # fold BN1 scale into f1 (in place, per input-channel partition)
    for k in range(EC):
        with nc.allow_low_precision(reason="f32r weights"):
            nc.vector.tensor_scalar(out=f1[k][:], in0=f1[k][:],
                                    scalar1=s1[:, k:k + 1], scalar2=None,
                                    op0=OP.mult)

    # hn = s1*ht + t1 (BN1 output, residual only) -> xT slots
    hn = [const.tile([128, TOK], F32R, tag=f"xT{k}", name=f"hn{k}")
          for k in range(EC)]
    for m in range(EC):
        for i, (off, fl) in enumerate(_bank_slices(0, TOK)):
            src = ht[m][:, off:off + fl].bitcast(F32)
            dstv = hn[m][:, off:off + fl]
            with nc.allow_low_precision(reason="f32r hn"):
                nc.gpsimd.tensor_scalar(out=dstv, in0=src,
                                        scalar1=s1[:, m:m + 1],
                                        scalar2=t1[:, m:m + 1],
                                        op0=OP.mult, op1=OP.add)

    # ---------- FFN ----------
    h1 = act.tile([128, EC, TOK], F8, tag="h1", name="h1")
    for m in range(EC):
        for off, fl in _bank_slices(0, TOK):
            ps = ps_big.tile([128, 2, NT], F32, tag="sc", name="psf1")
            for k in range(EC):
                nc.tensor.matmul(ps[:, 0, 0:fl], f1[k][:, m * 128:(m + 1) * 128],
                                 ht[k][:, off:off + fl],
                                 start=(k == 0), stop=(k == EC - 1))
            with nc.allow_low_precision(reason="fp8 h1"):
                nc.scalar.activation(out=h1[:, m, off:off + fl],
                                     in_=ps[:, 0, 0:fl], func=AF.Relu,
                                     bias=b1p[:, m:m + 1], scale=1.0)
    ho = [act.tile([128, TOK], F32, tag=f"ht{k}", name=f"ho{k}")
          for k in range(EC)]
    acc2 = small.tile([128, EC, 4], F32, tag="acc2", name="acc2", bufs=1)
    sq2 = small.tile([128, EC, 4], F32, tag="sq2", name="sq2", bufs=1)
    for m in range(EC):
        for i, (off, fl) in enumerate(_bank_slices(0, TOK)):
            ps = ps_big.tile([128, 2, NT], F32, tag="sc", name="psf2")
            for c in range(0, fl, 256):
                cl = min(256, fl - c)
                for g in range(2):
                    nc.tensor.matmul(
                        ps[:, 0, c:c + cl],
                        f28[:, g, :, m * 128:(m + 1) * 128],
                        h1[:, 2 * g:2 * g + 2, off + c:off + c + cl],
                        start=(g == 0), stop=(g == 1), perf_mode=DR)
            dst = ho[m][:, off:off + fl]
            nc.vector.scalar_tensor_tensor(
                out=dst, in0=ps[:, 0, 0:fl], scalar=bias["f2"][:, m:m + 1],
                in1=hn[m][:, off:off + fl].bitcast(F32),
                op0=OP.add, op1=OP.add,
                accum_out=acc2[:, m, i:i + 1])
            # sumsq for BN2 on ACT (DVE is saturated by the STT evacs here)
            scr = small.tile([128, NT], F32, tag="sqa", name="sqa2", bufs=2)
            nc.scalar.activation(
                out=scr[:, 0:fl], in_=dst, func=AF.Square,
                accum_out=sq2[:, m, i:i + 1])

    # ---------- BN2 + output (pipelined per 512-token slice) ----------
    s2, t2 = _bn_params(nc, small, dram, acc2[:], sq2[:],
                        bias["bn2_g"],
                        bias["bn2_b"], epst[:], "bn2", for_timing)
    for i, (off, fl) in enumerate(_bank_slices(0, TOK)):
        for m in range(EC):
            dst = ho[m][:, off:off + fl]
            eng = (i * EC + m) % 3
            if eng == 0:
                nc.vector.tensor_scalar(out=dst, in0=dst,
                                        scalar1=s2[:, m:m + 1],
                                        scalar2=t2[:, m:m + 1],
                                        op0=OP.mult, op1=OP.add)
            elif eng == 1:
                nc.scalar.activation(out=dst, in_=dst, func=AF.Identity,
                                     bias=t2[:, m:m + 1], scale=s2[:, m:m + 1])
            else:
                nc.gpsimd.tensor_scalar(out=dst, in0=dst,
                                        scalar1=s2[:, m:m + 1],
                                        scalar2=t2[:, m:m + 1],
                                        op0=OP.mult, op1=OP.add)
            nc.sync.dma_start(out=yT_d.ap()[m * 128:(m + 1) * 128, off:off + fl],
                              in_=dst)


def _bn_params(nc, small, dram, accs, sqs, g_sb, b_sb, epst, name,
               for_timing=False):
    """Per-channel scale/shift for training-mode BN over all B*N tokens from
    raw per-(m, slice) sums: reduce -> 8-core AllReduce -> mu/var ->
    sqrt+recip. Returns (s [128, EC], t [128, EC]) tiles."""
    ccin = dram.tile([128, 2 * EC], F32, tag=f"cci_{name}", name=f"cci_{name}")
    ccout = dram.tile([128, 2 * EC], F32, tag=f"cco_{name}", name=f"cco_{name}")
    su = small.tile([128, 2, EC], F32, tag=f"su_{name}", name=f"su_{name}")
    nc.vector.tensor_reduce(out=su[:, 0, :], in_=accs,
                            axis=mybir.AxisListType.X, op=OP.add)
    nc.vector.tensor_reduce(out=su[:, 1, :], in_=sqs,
                            axis=mybir.AxisListType.X, op=OP.add)
    nc.sync.dma_start(out=ccin[:], in_=su[:].rearrange("p a b -> p (a b)"))
    if for_timing:
        # TimelineSim cannot model collectives; substitute a same-shape copy
        nc.gpsimd.dma_start(out=ccout[:], in_=ccin[:])
    else:
        nc.gpsimd.collective_compute(
            "AllReduce", OP.add, replica_groups=[list(range(N_CORES))],
            ins=[ccin.opt()], outs=[ccout.opt()])
    gsa = small.tile([128, 2, EC], F32, tag=f"gs_{name}", name=f"gs_{name}")
    nc.sync.dma_start(out=gsa[:].rearrange("p a b -> p (a b)"), in_=ccout[:])
    mu = small.tile([128, EC], F32, tag=f"mu_{name}", name=f"mu_{name}", bufs=1)
    var = small.tile([128, EC], F32, tag=f"var_{name}", name=f"var_{name}",
                     bufs=1)
    nc.vector.tensor_scalar(out=mu[:], in0=gsa[:, 0, :],
                            scalar1=1.0 / N_GLOBAL, scalar2=None, op0=OP.mult)
    nc.vector.tensor_tensor(out=var[:], in0=mu[:], in1=mu[:], op=OP.mult)
    nc.vector.scalar_tensor_tensor(out=var[:], in0=gsa[:, 1, :],
                                   scalar=1.0 / N_GLOBAL, in1=var[:],
                                   op0=OP.mult, op1=OP.subtract)
    sq = small.tile([128, EC], F32, tag=f"sq_{name}", name=f"sq_{name}", bufs=1)
    nc.scalar.activation(out=sq[:], in_=var[:], func=AF.Sqrt, bias=epst,
                         scale=1.0)
    r0 = small.tile([128, EC], F32, tag=f"r0_{name}", name=f"r0_{name}", bufs=1)
    nc.vector.reciprocal(out=r0[:], in_=sq[:])
    s_all = small.tile([128, EC], F32, tag=f"s_{name}", name=f"s_{name}",
                       bufs=1)
    sh_all = small.tile([128, EC], F32, tag=f"sh_{name}", name=f"sh_{name}",
                        bufs=1)
    nc.vector.tensor_tensor(out=s_all[:], in0=r0[:], in1=g_sb, op=OP.mult)
    nc.vector.tensor_tensor(out=sh_all[:], in0=mu[:], in1=s_all[:], op=OP.mult)
    nc.vector.tensor_tensor(out=sh_all[:], in0=b_sb, in1=sh_all[:],
                            op=OP.subtract)
    return s_all, sh_all


_NC_CACHE = None


def _get_nc():
    global _NC_CACHE
    if _NC_CACHE is None:
        _NC_CACHE = build()
    return _NC_CACHE


def make_in_maps(inputs):
    import ml_dtypes
    f8 = ml_dtypes.float8_e4m3
    shared = {}
    for n in W8_NAMES:
        w = np.asarray(inputs[f"{n}_w"], dtype=np.float32)      # [E, E]
        # w8[p, g, jt, j] = W[j, (2g+jt)*128 + p]
        w8 = np.ascontiguousarray(
            w.T.reshape(2, 2, 128, E).transpose(2, 0, 1, 3)).astype(f8)
        shared[f"{n}_w8"] = w8.reshape(128, 4 * E)
    shared["f1_wT"] = np.ascontiguousarray(
        np.asarray(inputs["f1_w"], dtype=np.float32).T)
    w2 = np.asarray(inputs["f2_w"], dtype=np.float32)
    shared["f2_w8"] = np.ascontiguousarray(
        w2.T.reshape(2, 2, 128, E).transpose(2, 0, 1, 3)).astype(
            f8).reshape(128, 4 * E)
    for n in ["rv", "tv"]:
        shared[f"{n}_brep"] = np.ascontiguousarray(
            np.broadcast_to(np.asarray(inputs[f"{n}_b"], dtype=np.float32),
                            (128, E)))
    bpk = np.empty((128, len(ALL_B) * EC), dtype=np.float32)
    for i, n in enumerate(ALL_B):
        vec = inputs[f"{n}_b"] if n in W8_NAMES + ["f1", "f2"] else inputs[n]
        bpk[:, i * EC:(i + 1) * EC] = np.asarray(vec).reshape(EC, 128).T
    shared["bpk"] = bpk
    sel2 = np.zeros((98, 128), dtype=np.float32)
    for p in range(4):
        sel2[32 * p, 0:64] = IVS
        sel2[32 * p + 1, 64:128] = IVS
    shared["sel2"] = sel2
    shared["ones1"] = np.ones((1, 128), dtype=np.float32)

    x = np.asarray(inputs["x"], dtype=np.float32)
    in_maps = []
    for i in range(N_CORES):
        xc = x[BL * i:BL * (i + 1)]                      # [BL, N, E]
        xT = np.ascontiguousarray(xc.transpose(2, 0, 1).reshape(E, TOK))
        x8 = np.ascontiguousarray(
            xT.reshape(EC, 128, BL, N).transpose(1, 0, 2, 3)).astype(f8)
        x8r = np.ascontiguousarray(x8[:, :, :, 0:NR])
        x8t = np.ascontiguousarray(x8[:, :, :, NR:N])
        in_maps.append({"xT": xT,
                        "x8r": x8r.reshape(128, EC * 2 * NR),
                        "x8t": x8t.reshape(128, EC * 2 * NT), **shared})
    return in_maps


def assemble_output(results):
    y = np.empty((B, N, E), dtype=np.float32)
    for i in range(N_CORES):
        yT = results[i]["yT"]                            # [E, TOK]
        y[BL * i:BL * (i + 1)] = yT.reshape(E, BL, N).transpose(1, 2, 0)
    return y


def kernel(**inputs):
    nc = _get_nc()
    in_maps = make_in_maps(inputs)
    res = run_bass_kernel_spmd(nc, in_maps, core_ids=list(range(N_CORES)))
    return assemble_output(res.results)


if __name__ == "__main__":
    nc = build()
    print("build ok")
